# revision 6
# baseline (speedup 1.0000x reference)
"""Transformer-XL relative attention (B=2, L=2048, D=1024, H=16) on 8 TRN2
NeuronCores.

Sharding: data-parallel over batch x tensor-parallel over heads.  Core
c = 4*b + g handles batch b, head group g (4 heads).  Wq/Wk/Wv are
column-sharded, Wo row-sharded; each core emits a partial [2048,1024]
output (bf16) which the host sums per batch in f32 (+bo).

Per-core layouts (bf16 in SBUF, head pair = h//2, row = 64*(h%2)+e):
  q1t/q2t/kt : [128, 2*2048]  e-tile h//2 at cols [2048*(h//2)], free = seq
  peht       : [128, 2*2048]  relative-position keys pe[1:2049] @ r_kernel
  vh         : [128, 16*260]  per key-tile: 4 heads' V (64 cols each) + a
                              ones column per head (free row-sum trick)
  at (A^T)   : [128, 2*2048]  normalized attention output, transposed

Rel-shift: for query block bi (rows ri..ri+127) R_s[rr, x] holds
Q2 . peh[xmin + x] (xmin = 1920 - ri); the score chunk at columns
[c0, c0+wc) needs staged[rr, cc] = R_s[rr, 127 - rr + c0 + cc] -- an
anti-diagonal flat access pattern (offset 127+c0, ap=[[PITCH-1,128],[1,wc]])
which only DMA engines can execute (SBUF->SBUF).  R_s columns beyond the
causal edge are padded with -1e9; the anti-diagonal read maps exactly the
strict upper triangle onto that pad, so exp() yields the causal zeros with
no separate masking pass.

P^T (pn -> pt) is done by the DMA crossbar (dma_start_transpose): one
batched InstDmaTransposeAnt per (head, query block) transposes each
128-col block of pn into its pt strip via a 3D strided dst AP, freeing
the PE of 544 transpose matmuls and the DVE of the psT evictions.

The rel pipeline is software-pipelined one query-block ahead: the rel
matmuls + eviction + pad + anti-diag staging DMA for block i+1 are
emitted before the content/ident/exp chunks of block i, hiding the DMA
completion-semaphore latency (900 ns) behind a full block of PE work.
"""

import numpy as np
import ml_dtypes
import concourse.bass as bass
import concourse.mybir as mybir
import concourse.tile as tile
from concourse import bacc
from concourse import bass_utils
from concourse.bass_interp import get_hw_module
from concourse.masks import make_identity

BF = mybir.dt.bfloat16
F32 = mybir.dt.float32
EXP = mybir.ActivationFunctionType.Exp
IDENT = mybir.ActivationFunctionType.Identity
MULT = mybir.AluOpType.mult
ADD = mybir.AluOpType.add

L = 2048          # sequence length
RS_W = 2176       # R_s tile width (2048 + 128 pad); anti-diag pitch = RS_W-1

PROFILE = False       # set by test harness to capture a trace
LAST_RESULTS = None   # BassKernelResults of the last run (for profiling)

# tuning knobs (read at build time)
CFG = {
    "diag_eng": "sync",    # engine issuing anti-diagonal staging DMAs
    "load_eng": "sync",    # engine issuing input/weight loads
    "rs_bufs": 3,
    "stg_bufs": 3,
    "pn_bufs": 3,
    "pt_bufs": 2,
    "psS_bufs": 3,
    "psR_bufs": 3,
    "psA_bufs": 2,
    "osb_bufs": 2,
    "rec64_bufs": 1,
    "r_evict": "mix",     # act | dve | mix
    "io_bufs": 3,
    "in_split": 4,        # input load DMAs per projection half
    "proj_order": "kpvq",
    "ra_dve": 0,          # rel-add chunks moved to DVE per block (0..4)
    "pad_eng": "vector",  # pad/ones memset engine: gpsimd | vector
    "xbar_lag": 2,        # jobs between block_stage and its crossbar DMA
    "av_lag": 1,          # jobs between last xbar of (h,g) and its AV
    "norm_lag": 1,        # jobs between AV and recip/bcast/normalize
    "op_lag": 1,          # jobs between norm(h=3) and outproj
}


def emit_core(nc, ins, out):
    """ins: dict name->AP (DRAM), out: AP (DRAM [2048,1024] bf16)."""
    with tile.TileContext(nc) as tc:
        deng = getattr(nc, CFG["diag_eng"])
        leng = getattr(nc, CFG["load_eng"])
        peng = getattr(nc, CFG["pad_eng"] if CFG["pad_eng"] != "gpsimd"
                       else "gpsimd")
        with (
            tc.tile_pool(name="wgt", bufs=1) as wp,
            tc.tile_pool(name="io", bufs=CFG["io_bufs"]) as iop,
            tc.tile_pool(name="per", bufs=1) as per,
            tc.tile_pool(name="work", bufs=3) as wk,
            tc.tile_pool(name="pt", bufs=CFG["pt_bufs"]) as ptp,
            tc.tile_pool(name="psS", bufs=CFG["psS_bufs"], space="PSUM") as psS,
            tc.tile_pool(name="psR", bufs=CFG["psR_bufs"], space="PSUM") as psR,
            tc.tile_pool(name="psA", bufs=CFG["psA_bufs"], space="PSUM") as psA,
        ):
            # ---------------- persistent tiles ----------------
            q1t = per.tile([128, 2 * L], BF, tag="q1t")
            q2t = per.tile([128, 2 * L], BF, tag="q2t")
            kt = per.tile([128, 2 * L], BF, tag="kt")
            peht = per.tile([128, 2 * L], BF, tag="peht")
            at = per.tile([128, 2 * L], BF, tag="at")
            vh = per.tile([128, 16 * 260], BF, tag="vh")
            b1s = per.tile([128, 2], F32, tag="b1s")
            b2s = per.tile([128, 2], F32, tag="b2s")
            bks = per.tile([128, 2], F32, tag="bks")
            bvs = per.tile([128, 2], F32, tag="bvs")

            ident = per.tile([128, 128], BF, tag="ident")
            make_identity(nc, ident[:])
            nc.sync.dma_start(b1s[:], ins["b1"])
            nc.sync.dma_start(b2s[:], ins["b2"])
            nc.sync.dma_start(bks[:], ins["bk"])
            nc.sync.dma_start(bvs[:], ins["bv"])

            # ---------------- weights (lazy: loaded just before use) ------
            _wcache = {}

            def load_w(name, cols):
                if name in _wcache:
                    return _wcache[name]
                t = wp.tile([128, cols], BF, tag=name, name=f"w_{name}")
                leng.dma_start(
                    t[:], ins[name].rearrange("(a p) e -> p a e", p=128)
                )
                _wcache[name] = t
                return t

            # ---------------- phase A: projections ----------------
            vht = per.tile([128, 2 * L], BF, tag="vht")

            def project(xname, wname, evict):
                w_s = load_w(wname, 2048)
                src = ins[xname].rearrange("(a p) n -> p a n", p=128)
                xts = []
                for half in range(2):
                    xt = iop.tile([128, 4 * L], BF, tag="inT",
                                  name=f"in_{xname}_{half}")
                    ns = CFG["in_split"]
                    blk = 4 // ns
                    for s in range(ns):
                        leng.dma_start(
                            xt[:, 2048 * blk * s: 2048 * blk * (s + 1)],
                            src[:, 4 * half + blk * s:
                                4 * half + blk * (s + 1), :],
                        )
                    xts.append(xt)
                for et in range(2):
                    for rc in range(4):
                        ps = psS.tile([128, 512], F32, tag="S",
                                      name=f"ps_{xname}_{et}_{rc}")
                        for kc in range(8):
                            xt = xts[kc // 4]
                            kcc = kc % 4
                            nc.tensor.matmul(
                                ps[:],
                                w_s[:, 256 * kc + 128 * et: 256 * kc + 128 * et + 128],
                                xt[:, 2048 * kcc + 512 * rc: 2048 * kcc + 512 * rc + 512],
                                start=(kc == 0),
                                stop=(kc == 7),
                            )
                        evict(et, rc, ps)

            def evict_q(et, rc, ps):
                cs = 2048 * et + 512 * rc
                nc.scalar.activation(q1t[:, cs:cs + 512], ps[:], IDENT,
                                     bias=b1s[:, et:et + 1], scale=0.125)
                nc.scalar.activation(q2t[:, cs:cs + 512], ps[:], IDENT,
                                     bias=b2s[:, et:et + 1], scale=0.125)

            def evict_k(et, rc, ps):
                cs = 2048 * et + 512 * rc
                nc.scalar.activation(kt[:, cs:cs + 512], ps[:], IDENT,
                                     bias=bks[:, et:et + 1], scale=1.0)

            def evict_v(et, rc, ps):
                cs = 2048 * et + 512 * rc
                nc.scalar.activation(vht[:, cs:cs + 512], ps[:], IDENT,
                                     bias=bvs[:, et:et + 1], scale=1.0)

            def evict_pe(et, rc, ps):
                cs = 2048 * et + 512 * rc
                nc.vector.tensor_copy(peht[:, cs:cs + 512], ps[:])

            projs = {"q": ("q_in", "wq", evict_q), "k": ("k_in", "wk", evict_k),
                     "v": ("v_in", "wv", evict_v), "p": ("pe_in", "rk", evict_pe)}
            for c in CFG["proj_order"]:
                project(*projs[c])
            wo_s = load_w("wo", 2048)

            # vh assembly: PE-transpose one [128,128] tile per (ct, pair)
            # (covers both heads of the pair), evict the two heads' column
            # halves into their vh blocks.
            for ct in range(16):
                for p in range(2):
                    scr = psS.tile([128, 512], BF, tag="S",
                                   name=f"scr_{ct}_{p}")
                    nc.tensor.transpose(
                        scr[:, 0:128],
                        vht[:, 2048 * p + 128 * ct: 2048 * p + 128 * ct + 128],
                        ident[:],
                    )
                    for hh in range(2):
                        h4 = 2 * p + hh
                        nc.vector.tensor_copy(
                            vh[:, 260 * ct + 65 * h4: 260 * ct + 65 * h4 + 64],
                            scr[:, 64 * hh: 64 * hh + 64],
                        )
            ones_cols = vh[:].rearrange("p (ct c) -> p ct c", c=260)
            for h in range(4):
                peng.memset(ones_cols[:, :, 65 * h + 64], 1.0)

            def outproj_rt(rt):
                osb = wk.tile([128, 1024], BF, tag="osb",
                              bufs=CFG["osb_bufs"], name=f"osb_{rt}")
                for n in range(2):
                    op_ = psS.tile([128, 512], F32, tag="S",
                                   name=f"op_{rt}_{n}")
                    for hc in range(2):
                        nc.tensor.matmul(
                            op_[:],
                            at[:, 2048 * hc + 128 * rt: 2048 * hc + 128 * rt + 128],
                            wo_s[:, 1024 * hc + 512 * n: 1024 * hc + 512 * n + 512],
                            start=(hc == 0), stop=(hc == 1),
                        )
                    if n == 0:
                        nc.scalar.copy(osb[:, 0:512], op_[:])
                    else:
                        nc.vector.tensor_copy(osb[:, 512:1024], op_[:])
                nc.sync.dma_start(out[128 * rt: 128 * rt + 128, :], osb[:])

            # ---------------- phase B: attention ----------------
            # rel-score stage for one (h, bi): matmuls -> rs evict -> pad
            # memset -> anti-diagonal staging DMA.  Returns the staged tile.
            def rel_stage(h, bi):
                et, ph = h // 2, h % 2
                r0, r1 = 64 * ph, 64 * ph + 64
                ri = 128 * bi
                Wb = ri + 128
                nch = (Wb + 511) // 512
                xmin = 1920 - ri
                rs = wk.tile([128, RS_W], BF, tag="rs",
                             bufs=CFG["rs_bufs"], name=f"rs_h{h}_b{bi}")
                for jc in range(nch):
                    w = min(512, Wb - 512 * jc)
                    rp = psR.tile([128, 512], F32, tag="R",
                                  name=f"rp_h{h}_b{bi}_{jc}")
                    nc.tensor.matmul(
                        rp[:, :w],
                        q2t[r0:r1, 2048 * et + ri: 2048 * et + ri + 128],
                        peht[r0:r1, 2048 * et + xmin + 512 * jc:
                             2048 * et + xmin + 512 * jc + w],
                        start=True, stop=True,
                    )
                    use_act = (CFG["r_evict"] == "act" or
                               (CFG["r_evict"] == "mix" and jc % 2 == 0))
                    if use_act:
                        nc.scalar.copy(rs[:, 512 * jc: 512 * jc + w],
                                       rp[:, :w])
                    else:
                        nc.vector.tensor_copy(
                            rs[:, 512 * jc: 512 * jc + w], rp[:, :w])
                peng.memset(rs[:, Wb:Wb + 128], -1e9)
                staged = wk.tile([128, 2048], BF, tag="stg",
                                 bufs=CFG["stg_bufs"], name=f"stg_h{h}_b{bi}")
                diag = bass.AP(
                    tensor=rs.tensor,
                    offset=rs.offset + 127,
                    ap=[[RS_W - 1, 128], [1, Wb]],
                )
                deng.dma_start(staged[:, :Wb], diag)
                return staged

            # content/ident/exp chunks for one (h, bi); crossbar is deferred
            def block_stage(h, bi, staged, pn, nrel_dve):
                et, ph = h // 2, h % 2
                r0, r1 = 64 * ph, 64 * ph + 64
                ri = 128 * bi
                Wb = ri + 128
                nch = (Wb + 511) // 512
                for ci_chunk in range(nch):
                    c0 = 512 * ci_chunk
                    wc = min(512, Wb - c0)
                    use_dve = ci_chunk < nrel_dve
                    sp = psS.tile([128, 512], F32, tag="S",
                                  name=f"sp_h{h}_b{bi}_{ci_chunk}")
                    nc.tensor.matmul(
                        sp[:, :wc],
                        q1t[r0:r1, 2048 * et + ri: 2048 * et + ri + 128],
                        kt[r0:r1, 2048 * et + c0: 2048 * et + c0 + wc],
                        start=True, stop=use_dve,
                    )
                    if use_dve:
                        nc.vector.scalar_tensor_tensor(
                            out=sp[:, :wc], in0=sp[:, :wc], scalar=1.0,
                            in1=staged[:, c0:c0 + wc],
                            op0=MULT, op1=ADD,
                        )
                    else:
                        nc.tensor.matmul(
                            sp[:, :wc],
                            ident[:],
                            staged[:, c0:c0 + wc],
                            start=False, stop=True,
                        )
                    nc.scalar.activation(pn[:, c0:c0 + wc], sp[:, :wc], EXP)

            def xbar_stage(bi, pn, pt):
                # one batched crossbar transpose: pn [128, Wb] -> pt strips
                Wb = 128 * bi + 128
                dst = bass.AP(
                    tensor=pt.tensor,
                    offset=pt.offset + 128 * (bi % 4),
                    ap=[[pt.tensor.shape[-1], 128], [512, Wb // 128],
                        [1, 128]],
                )
                nc.sync.dma_start_transpose(dst, pn[:, :Wb])

            def av_stage(h, g, pt, av):
                for ci in range(4 * g + 4):
                    o = max(0, 128 * ci - 512 * g)
                    nc.tensor.matmul(
                        av[:, o:512],
                        vh[:, 260 * ci + 65 * h: 260 * ci + 65 * h + 65],
                        pt[:, 512 * ci + o: 512 * ci + 512],
                        start=(ci == 0), stop=(ci == 4 * g + 3),
                    )

            def norm_stage(h, g, av):
                et, ph = h // 2, h % 2
                r0, r1 = 64 * ph, 64 * ph + 64
                rec = wk.tile([1, 512], F32, tag="rec", name=f"rec_h{h}_g{g}")
                nc.vector.reciprocal(rec[:], av[64:65, :])
                rec64 = wk.tile([64, 512], F32, tag="rec64",
                                bufs=CFG["rec64_bufs"], name=f"rec64_h{h}_g{g}")
                nc.gpsimd.partition_broadcast(rec64[:], rec[:])
                nc.vector.tensor_tensor(
                    out=at[r0:r1, 2048 * et + 512 * g:
                           2048 * et + 512 * g + 512],
                    in0=av[0:64, :],
                    in1=rec64[:],
                    op=MULT,
                )

            # software-pipelined schedule over jobs (h, g, bi): the rel
            # stage runs one job ahead; crossbar/AV/normalize/outproj are
            # deferred so their dependencies are complete by the time each
            # engine reaches them in program order.
            seq = [(h, g, bi) for g in range(4) for h in range(4)
                   for bi in range(4 * g, 4 * g + 4)]
            deferred = {}   # emit_idx -> list of callables

            def defer(idx, fn):
                deferred.setdefault(idx, []).append(fn)

            pts = {}
            avs = {}
            staged_tiles = {}
            staged_tiles[seq[0]] = rel_stage(seq[0][0], seq[0][2])
            n = len(seq)
            for idx, (h, g, bi) in enumerate(seq):
                if (h, g) not in pts:
                    pts[(h, g)] = ptp.tile(
                        [128, 512 * (4 * g + 4)], BF, tag="pt",
                        bufs=CFG["pt_bufs"], name=f"pt_h{h}_g{g}")
                if idx + 1 < n:
                    nxt = seq[idx + 1]
                    staged_tiles[nxt] = rel_stage(nxt[0], nxt[2])
                pn = wk.tile([128, 2048], BF, tag="pn",
                             bufs=CFG["pn_bufs"], name=f"pn_h{h}_b{bi}")
                block_stage(h, bi, staged_tiles.pop((h, g, bi)), pn,
                            CFG["ra_dve"])
                pt = pts[(h, g)]
                xb_idx = idx + CFG["xbar_lag"]
                defer(xb_idx, lambda bi=bi, pn=pn, pt=pt:
                      xbar_stage(bi, pn, pt))
                if bi == 4 * g + 3:
                    av_idx = xb_idx + CFG["av_lag"]

                    def do_av(h=h, g=g, pt=pt):
                        avs[(h, g)] = psA.tile([65, 512], F32, tag="A",
                                               name=f"av_h{h}_g{g}")
                        av_stage(h, g, pt, avs[(h, g)])
                        pts.pop((h, g), None)
                    defer(av_idx, do_av)
                    nm_idx = av_idx + CFG["norm_lag"]
                    defer(nm_idx, lambda h=h, g=g:
                          norm_stage(h, g, avs.pop((h, g))))
                    if h == 3:
                        def do_op(g=g):
                            for rt in range(4 * g, 4 * g + 4):
                                outproj_rt(rt)
                        defer(nm_idx + CFG["op_lag"], do_op)
                for fn in deferred.pop(idx, []):
                    fn()
            # flush remaining deferred work in index order
            for idx in sorted(deferred):
                for fn in deferred[idx]:
                    fn()
    return nc


# ---------------- host side ----------------

def _bf16(x):
    return np.ascontiguousarray(x).astype(ml_dtypes.bfloat16)


def _col2d(vec256):
    """[256] f32 -> [128, 2] with v2d[p, a] = vec[128a + p]."""
    return np.ascontiguousarray(
        np.asarray(vec256, np.float32).reshape(2, 128).T)


def core_inputs(q_b, k_b, v_b, pos_enc, Wq, bq, Wk, bk, Wv, bv, Wo,
                r_w_bias, r_r_bias, r_kernel, g):
    sl = slice(256 * g, 256 * g + 256)
    rk_cat = np.concatenate([r_kernel[4 * g + i] for i in range(4)], axis=1)
    return {
        "q_in": _bf16(q_b.T),
        "k_in": _bf16(k_b.T),
        "v_in": _bf16(v_b.T),
        "pe_in": _bf16(pos_enc[1:2049].T),
        "wq": _bf16(Wq[:, sl]),
        "wk": _bf16(Wk[:, sl]),
        "wv": _bf16(Wv[:, sl]),
        "rk": _bf16(rk_cat),
        "wo": _bf16(Wo[sl, :]),
        "b1": _col2d(0.125 * (bq[sl] + r_w_bias[4 * g:4 * g + 4].reshape(256))),
        "b2": _col2d(0.125 * (bq[sl] + r_r_bias[4 * g:4 * g + 4].reshape(256))),
        "bk": _col2d(bk[sl]),
        "bv": _col2d(bv[sl]),
    }


_SHAPES = {
    "q_in": ([1024, 2048], BF), "k_in": ([1024, 2048], BF),
    "v_in": ([1024, 2048], BF), "pe_in": ([1024, 2048], BF),
    "wq": ([1024, 256], BF), "wk": ([1024, 256], BF), "wv": ([1024, 256], BF),
    "rk": ([1024, 256], BF), "wo": ([256, 1024], BF),
    "b1": ([128, 2], F32), "b2": ([128, 2], F32),
    "bk": ([128, 2], F32), "bv": ([128, 2], F32),
}

_NC_CACHE = {}


def _build():
    key = tuple(sorted((k, str(v)) for k, v in CFG.items()))
    if key in _NC_CACHE:
        return _NC_CACHE[key]
    nc = bacc.Bacc("TRN2", target_bir_lowering=False, debug=False,
                   enable_asserts=False)
    ins = {name: nc.dram_tensor(name, shape, dt, kind="ExternalInput").ap()
           for name, (shape, dt) in _SHAPES.items()}
    out = nc.dram_tensor("out", [2048, 1024], BF, kind="ExternalOutput").ap()
    emit_core(nc, ins, out)
    nc.compile()
    nc.m = get_hw_module(nc.m)
    _NC_CACHE[key] = nc
    return nc


def kernel(**inputs):
    global LAST_RESULTS
    inp = {k: np.asarray(v) for k, v in inputs.items()}
    nc = _build()
    in_maps = []
    for c in range(8):
        b, g = c // 4, c % 4
        in_maps.append(core_inputs(
            inp["q"][b], inp["k"][b], inp["v"][b], inp["pos_enc"],
            inp["Wq"], inp["bq"], inp["Wk"], inp["bk"], inp["Wv"], inp["bv"],
            inp["Wo"], inp["r_w_bias"], inp["r_r_bias"], inp["r_kernel"], g))
    res = bass_utils.run_bass_kernel_spmd(
        nc, in_maps, core_ids=list(range(8)), trace=PROFILE)
    LAST_RESULTS = res
    out = np.zeros((2, 2048, 1024), np.float32)
    for c in range(8):
        b = c // 4
        out[b] += np.asarray(res.results[c]["out"]).astype(np.float32)
    out += np.asarray(inp["bo"], np.float32)[None, None, :]
    return out


# revision 16
# speedup vs baseline: 1.6492x; 1.6492x over previous
"""Transformer-XL relative attention (B=2, L=2048, D=1024, H=16) on 8 TRN2
NeuronCores.

Sharding: data-parallel over batch x tensor-parallel over heads.  Core
c = 4*b + g handles batch b, head group g (4 heads).  Wq/Wk/Wv are
column-sharded, Wo row-sharded; each core emits a partial [2048,1024]
output (bf16) which the host sums per batch in f32 (+bo).

Per-core layouts (bf16 in SBUF, head pair = h//2, row = 64*(h%2)+e):
  q1t/q2t/kt : [128, 2*2048]  e-tile h//2 at cols [2048*(h//2)], free = seq
  peht       : [128, 2*2048]  relative-position keys pe[1:2049] @ r_kernel
  vh         : [128, 16*260]  per key-tile: 4 heads' V (64 cols each) + a
                              ones column per head (free row-sum trick)
  at (A^T)   : [128, 2*2048]  normalized attention output, transposed

Rel-shift: for query block bi (rows ri..ri+127) R_s[rr, x] holds
Q2 . peh[xmin + x] (xmin = 1920 - ri); the score chunk at columns
[c0, c0+wc) needs staged[rr, cc] = R_s[rr, 127 - rr + c0 + cc] -- an
anti-diagonal flat access pattern (offset 127+c0, ap=[[PITCH-1,128],[1,wc]])
which only DMA engines can execute (SBUF->SBUF).  R_s columns beyond the
causal edge are padded with -1e9; the anti-diagonal read maps exactly the
strict upper triangle onto that pad, so exp() yields the causal zeros with
no separate masking pass.

P^T (pn -> pt) is done by the DMA crossbar (dma_start_transpose): one
batched InstDmaTransposeAnt per (head, query block) transposes each
128-col block of pn into its pt strip via a 3D strided dst AP, freeing
the PE of 544 transpose matmuls and the DVE of the psT evictions.

The rel pipeline is software-pipelined one query-block ahead: the rel
matmuls + eviction + pad + anti-diag staging DMA for block i+1 are
emitted before the content/ident/exp chunks of block i, hiding the DMA
completion-semaphore latency (900 ns) behind a full block of PE work.
"""

import numpy as np
import ml_dtypes
import concourse.bass as bass
import concourse.mybir as mybir
import concourse.tile as tile
from concourse import bacc
from concourse import bass_utils
from concourse.bass_interp import get_hw_module
from concourse.masks import make_identity

BF = mybir.dt.bfloat16
F32 = mybir.dt.float32
EXP = mybir.ActivationFunctionType.Exp
IDENT = mybir.ActivationFunctionType.Identity
MULT = mybir.AluOpType.mult
ADD = mybir.AluOpType.add

L = 2048          # sequence length
RS_W = 2176       # R_s tile width (2048 + 128 pad); anti-diag pitch = RS_W-1

PROFILE = False       # set by test harness to capture a trace
LAST_RESULTS = None   # BassKernelResults of the last run (for profiling)

# tuning knobs (read at build time)
CFG = {
    "diag_eng": "sync",    # engine issuing anti-diagonal staging DMAs
    "load_eng": "sync",    # engine issuing input/weight loads
    "rs_bufs": 3,
    "stg_bufs": 3,
    "pn_bufs": 3,
    "pt_bufs": 2,
    "psS_bufs": 2,
    "psR_bufs": 3,
    "psA_bufs": 1,
    "osb_bufs": 2,
    "rec64_bufs": 1,
    "r_evict": "mix",     # act | dve | mix
    "io_bufs": 3,
    "in_split": 4,        # input load DMAs per projection half
    "proj_order": "kpvq",
    "ra_dve": 0,          # rel-add chunks moved to DVE per block (0..4)
    "pad_eng": "vector",  # pad/ones memset engine: gpsimd | vector
    "pt_mode": "pe",      # pe (transpose matmuls) | xbar (batched DMA)
    "psT_bufs": 2,        # PSUM pool for PE transposes (pt_mode=pe)
    "pt_evict": "dve",    # psT -> pt eviction engine: act | dve | mix
    "tr_lag": 1,          # jobs between block chunks and its PE transposes
    "xbar_lag": 2,        # jobs between block_stage and its crossbar DMA
    "av_lag": 1,          # jobs between last xbar of (h,g) and its AV
    "norm_lag": 1,        # jobs between AV and recip/bcast/normalize
    "op_lag": 1,          # jobs between norm(h=3) and outproj
}


def emit_core(nc, ins, out):
    """ins: dict name->AP (DRAM), out: AP (DRAM [2048,1024] bf16)."""
    with tile.TileContext(nc) as tc:
        deng = getattr(nc, CFG["diag_eng"])
        leng = getattr(nc, CFG["load_eng"])
        peng = getattr(nc, CFG["pad_eng"] if CFG["pad_eng"] != "gpsimd"
                       else "gpsimd")
        with (
            tc.tile_pool(name="wgt", bufs=1) as wp,
            tc.tile_pool(name="io", bufs=CFG["io_bufs"]) as iop,
            tc.tile_pool(name="per", bufs=1) as per,
            tc.tile_pool(name="work", bufs=3) as wk,
            tc.tile_pool(name="pt", bufs=CFG["pt_bufs"]) as ptp,
            tc.tile_pool(name="psS", bufs=CFG["psS_bufs"], space="PSUM") as psS,
            tc.tile_pool(name="psR", bufs=CFG["psR_bufs"], space="PSUM") as psR,
            tc.tile_pool(name="psA", bufs=CFG["psA_bufs"], space="PSUM") as psA,
            tc.tile_pool(name="psT", bufs=CFG["psT_bufs"], space="PSUM") as psT,
        ):
            # ---------------- persistent tiles ----------------
            q1t = per.tile([128, 2 * L], BF, tag="q1t")
            q2t = per.tile([128, 2 * L], BF, tag="q2t")
            kt = per.tile([128, 2 * L], BF, tag="kt")
            peht = per.tile([128, 2 * L], BF, tag="peht")
            at = per.tile([128, 2 * L], BF, tag="at")
            vh = per.tile([128, 16 * 260], BF, tag="vh")
            b1s = per.tile([128, 2], F32, tag="b1s")
            b2s = per.tile([128, 2], F32, tag="b2s")
            bks = per.tile([128, 2], F32, tag="bks")
            bvs = per.tile([128, 2], F32, tag="bvs")

            ident = per.tile([128, 128], BF, tag="ident")
            make_identity(nc, ident[:])
            # bias loads go through the Activation HWDGE queue so they don't
            # head-of-line block the SP queue ahead of the weight/input loads
            nc.scalar.dma_start(b1s[:], ins["b1"])
            nc.scalar.dma_start(b2s[:], ins["b2"])
            nc.scalar.dma_start(bks[:], ins["bk"])
            nc.scalar.dma_start(bvs[:], ins["bv"])

            # ---------------- weights (lazy: loaded just before use) ------
            _wcache = {}

            def load_w(name, cols):
                if name in _wcache:
                    return _wcache[name]
                t = wp.tile([128, cols], BF, tag=name, name=f"w_{name}")
                leng.dma_start(
                    t[:], ins[name].rearrange("(a p) e -> p a e", p=128)
                )
                _wcache[name] = t
                return t

            # ---------------- phase A: projections ----------------
            vht = per.tile([128, 2 * L], BF, tag="vht")

            # kc-outer projection: all 8 (et, rc) accumulation groups live in
            # PSUM simultaneously (borrowing every PSUM pool), so each input
            # chunk DMA is consumed as soon as it lands.
            _proj_pools = [psS, psS, psR, psR, psR, psT, psT, psA]
            _proj_tags = ["S", "S", "R", "R", "R", "T", "T", "A"]

            def project(xname, wname, evict):
                w_s = load_w(wname, 2048)
                src = ins[xname].rearrange("(a p) n -> p a n", p=128)
                xts = []
                for half in range(2):
                    xt = iop.tile([128, 4 * L], BF, tag="inT",
                                  name=f"in_{xname}_{half}")
                    ns = CFG["in_split"]
                    blk = 4 // ns
                    for s in range(ns):
                        leng.dma_start(
                            xt[:, 2048 * blk * s: 2048 * blk * (s + 1)],
                            src[:, 4 * half + blk * s:
                                4 * half + blk * (s + 1), :],
                        )
                    xts.append(xt)
                pss = {}
                for i, (et, rc) in enumerate(
                        [(e, r) for e in range(2) for r in range(4)]):
                    pss[(et, rc)] = _proj_pools[i].tile(
                        [128, 512], F32, tag=_proj_tags[i],
                        name=f"ps_{xname}_{et}_{rc}")
                for kc in range(8):
                    xt = xts[kc // 4]
                    kcc = kc % 4
                    for et in range(2):
                        for rc in range(4):
                            nc.tensor.matmul(
                                pss[(et, rc)][:],
                                w_s[:, 256 * kc + 128 * et: 256 * kc + 128 * et + 128],
                                xt[:, 2048 * kcc + 512 * rc: 2048 * kcc + 512 * rc + 512],
                                start=(kc == 0),
                                stop=(kc == 7),
                            )
                for et in range(2):
                    for rc in range(4):
                        evict(et, rc, pss[(et, rc)])

            def evict_q(et, rc, ps):
                cs = 2048 * et + 512 * rc
                nc.scalar.activation(q1t[:, cs:cs + 512], ps[:], IDENT,
                                     bias=b1s[:, et:et + 1], scale=0.125)
                nc.scalar.activation(q2t[:, cs:cs + 512], ps[:], IDENT,
                                     bias=b2s[:, et:et + 1], scale=0.125)

            def evict_k(et, rc, ps):
                cs = 2048 * et + 512 * rc
                nc.scalar.activation(kt[:, cs:cs + 512], ps[:], IDENT,
                                     bias=bks[:, et:et + 1], scale=1.0)

            def evict_v(et, rc, ps):
                cs = 2048 * et + 512 * rc
                nc.scalar.activation(vht[:, cs:cs + 512], ps[:], IDENT,
                                     bias=bvs[:, et:et + 1], scale=1.0)

            def evict_pe(et, rc, ps):
                cs = 2048 * et + 512 * rc
                nc.vector.tensor_copy(peht[:, cs:cs + 512], ps[:])

            projs = {"q": ("q_in", "wq", evict_q), "k": ("k_in", "wk", evict_k),
                     "v": ("v_in", "wv", evict_v), "p": ("pe_in", "rk", evict_pe)}
            for c in CFG["proj_order"]:
                project(*projs[c])
            wo_s = load_w("wo", 2048)

            # vh assembly: PE-transpose one [128,128] tile per (ct, pair)
            # (covers both heads of the pair), evict the two heads' column
            # halves into their vh blocks.
            for ct in range(16):
                for p in range(2):
                    scr = psS.tile([128, 512], BF, tag="S",
                                   name=f"scr_{ct}_{p}")
                    nc.tensor.transpose(
                        scr[:, 0:128],
                        vht[:, 2048 * p + 128 * ct: 2048 * p + 128 * ct + 128],
                        ident[:],
                    )
                    for hh in range(2):
                        h4 = 2 * p + hh
                        nc.vector.tensor_copy(
                            vh[:, 260 * ct + 65 * h4: 260 * ct + 65 * h4 + 64],
                            scr[:, 64 * hh: 64 * hh + 64],
                        )
            ones_cols = vh[:].rearrange("p (ct c) -> p ct c", c=260)
            for h in range(4):
                peng.memset(ones_cols[:, :, 65 * h + 64], 1.0)

            def outproj_rt(rt):
                osb = wk.tile([128, 1024], BF, tag="osb",
                              bufs=CFG["osb_bufs"], name=f"osb_{rt}")
                for n in range(2):
                    op_ = psS.tile([128, 512], F32, tag="S",
                                   name=f"op_{rt}_{n}")
                    for hc in range(2):
                        nc.tensor.matmul(
                            op_[:],
                            at[:, 2048 * hc + 128 * rt: 2048 * hc + 128 * rt + 128],
                            wo_s[:, 1024 * hc + 512 * n: 1024 * hc + 512 * n + 512],
                            start=(hc == 0), stop=(hc == 1),
                        )
                    if n == 0:
                        nc.scalar.copy(osb[:, 0:512], op_[:])
                    else:
                        nc.vector.tensor_copy(osb[:, 512:1024], op_[:])
                nc.sync.dma_start(out[128 * rt: 128 * rt + 128, :], osb[:])

            # ---------------- phase B: attention ----------------
            # rel-score stage for one (h, bi): matmuls -> rs evict -> pad
            # memset -> anti-diagonal staging DMA.  Returns the staged tile.
            def rel_stage(h, bi):
                et, ph = h // 2, h % 2
                r0, r1 = 64 * ph, 64 * ph + 64
                ri = 128 * bi
                Wb = ri + 128
                nch = (Wb + 511) // 512
                xmin = 1920 - ri
                rs = wk.tile([128, RS_W], BF, tag="rs",
                             bufs=CFG["rs_bufs"], name=f"rs_h{h}_b{bi}")
                for jc in range(nch):
                    w = min(512, Wb - 512 * jc)
                    rp = psR.tile([128, 512], F32, tag="R",
                                  name=f"rp_h{h}_b{bi}_{jc}")
                    nc.tensor.matmul(
                        rp[:, :w],
                        q2t[r0:r1, 2048 * et + ri: 2048 * et + ri + 128],
                        peht[r0:r1, 2048 * et + xmin + 512 * jc:
                             2048 * et + xmin + 512 * jc + w],
                        start=True, stop=True,
                    )
                    use_act = (CFG["r_evict"] == "act" or
                               (CFG["r_evict"] == "mix" and jc % 2 == 0))
                    if use_act:
                        nc.scalar.copy(rs[:, 512 * jc: 512 * jc + w],
                                       rp[:, :w])
                    else:
                        nc.vector.tensor_copy(
                            rs[:, 512 * jc: 512 * jc + w], rp[:, :w])
                peng.memset(rs[:, Wb:Wb + 128], -1e9)
                staged = wk.tile([128, 2048], BF, tag="stg",
                                 bufs=CFG["stg_bufs"], name=f"stg_h{h}_b{bi}")
                diag = bass.AP(
                    tensor=rs.tensor,
                    offset=rs.offset + 127,
                    ap=[[RS_W - 1, 128], [1, Wb]],
                )
                deng.dma_start(staged[:, :Wb], diag)
                return staged

            # content/ident/exp chunks for one (h, bi); crossbar is deferred
            def block_stage(h, bi, staged, pn, pt, nrel_dve):
                et, ph = h // 2, h % 2
                r0, r1 = 64 * ph, 64 * ph + 64
                ri = 128 * bi
                Wb = ri + 128
                nch = (Wb + 511) // 512
                for ci_chunk in range(nch):
                    c0 = 512 * ci_chunk
                    wc = min(512, Wb - c0)
                    use_dve = ci_chunk < nrel_dve
                    sp = psS.tile([128, 512], F32, tag="S",
                                  name=f"sp_h{h}_b{bi}_{ci_chunk}")
                    nc.tensor.matmul(
                        sp[:, :wc],
                        q1t[r0:r1, 2048 * et + ri: 2048 * et + ri + 128],
                        kt[r0:r1, 2048 * et + c0: 2048 * et + c0 + wc],
                        start=True, stop=use_dve,
                    )
                    if use_dve:
                        nc.vector.scalar_tensor_tensor(
                            out=sp[:, :wc], in0=sp[:, :wc], scalar=1.0,
                            in1=staged[:, c0:c0 + wc],
                            op0=MULT, op1=ADD,
                        )
                    else:
                        nc.tensor.matmul(
                            sp[:, :wc],
                            ident[:],
                            staged[:, c0:c0 + wc],
                            start=False, stop=True,
                        )
                    nc.scalar.activation(pn[:, c0:c0 + wc], sp[:, :wc], EXP)

            # PE transposes for one (h, bi): emitted after all the block's
            # content/ident matmuls so the exps they wait on are complete.
            def transp_stage(h, bi, pn, pt):
                Wb = 128 * bi + 128
                nch = (Wb + 511) // 512
                for ci_chunk in range(nch):
                    c0 = 512 * ci_chunk
                    wc = min(512, Wb - c0)
                    tp_ = psT.tile([128, 512], BF, tag="T",
                                   name=f"tp_h{h}_b{bi}_{ci_chunk}")
                    for s in range(wc // 128):
                        nc.tensor.transpose(
                            tp_[:, 128 * s: 128 * s + 128],
                            pn[:, c0 + 128 * s: c0 + 128 * s + 128],
                            ident[:],
                        )
                    dst = bass.AP(
                        tensor=pt.tensor,
                        offset=pt.offset + 512 * (c0 // 128)
                        + 128 * (bi % 4),
                        ap=[[pt.tensor.shape[-1], 128],
                            [512, wc // 128], [1, 128]],
                    )
                    use_act = (CFG["pt_evict"] == "act" or
                               (CFG["pt_evict"] == "mix"
                                and (bi + ci_chunk) % 2 == 0))
                    if use_act:
                        nc.scalar.copy(dst, tp_[:, :wc])
                    else:
                        nc.vector.tensor_copy(dst, tp_[:, :wc])

            def xbar_stage(bi, pn, pt):
                # one batched crossbar transpose: pn [128, Wb] -> pt strips
                Wb = 128 * bi + 128
                dst = bass.AP(
                    tensor=pt.tensor,
                    offset=pt.offset + 128 * (bi % 4),
                    ap=[[pt.tensor.shape[-1], 128], [512, Wb // 128],
                        [1, 128]],
                )
                nc.sync.dma_start_transpose(dst, pn[:, :Wb])

            def av_stage(h, g, pt, av):
                for ci in range(4 * g + 4):
                    o = max(0, 128 * ci - 512 * g)
                    nc.tensor.matmul(
                        av[:, o:512],
                        vh[:, 260 * ci + 65 * h: 260 * ci + 65 * h + 65],
                        pt[:, 512 * ci + o: 512 * ci + 512],
                        start=(ci == 0), stop=(ci == 4 * g + 3),
                    )

            def norm_stage(h, g, av):
                et, ph = h // 2, h % 2
                r0, r1 = 64 * ph, 64 * ph + 64
                rec = wk.tile([1, 512], F32, tag="rec", name=f"rec_h{h}_g{g}")
                nc.vector.reciprocal(rec[:], av[64:65, :])
                rec64 = wk.tile([64, 512], F32, tag="rec64",
                                bufs=CFG["rec64_bufs"], name=f"rec64_h{h}_g{g}")
                nc.gpsimd.partition_broadcast(rec64[:], rec[:])
                nc.vector.tensor_tensor(
                    out=at[r0:r1, 2048 * et + 512 * g:
                           2048 * et + 512 * g + 512],
                    in0=av[0:64, :],
                    in1=rec64[:],
                    op=MULT,
                )

            # software-pipelined schedule over jobs (h, g, bi): the rel
            # stage runs one job ahead; crossbar/AV/normalize/outproj are
            # deferred so their dependencies are complete by the time each
            # engine reaches them in program order.
            seq = [(h, g, bi) for g in range(4) for h in range(4)
                   for bi in range(4 * g, 4 * g + 4)]
            deferred = {}   # emit_idx -> list of callables

            def defer(idx, fn):
                deferred.setdefault(idx, []).append(fn)

            pts = {}
            avs = {}
            staged_tiles = {}
            staged_tiles[seq[0]] = rel_stage(seq[0][0], seq[0][2])
            n = len(seq)
            for idx, (h, g, bi) in enumerate(seq):
                if (h, g) not in pts:
                    pts[(h, g)] = ptp.tile(
                        [128, 512 * (4 * g + 4)], BF, tag="pt",
                        bufs=CFG["pt_bufs"], name=f"pt_h{h}_g{g}")
                if idx + 1 < n:
                    nxt = seq[idx + 1]
                    staged_tiles[nxt] = rel_stage(nxt[0], nxt[2])
                pn = wk.tile([128, 2048], BF, tag="pn",
                             bufs=CFG["pn_bufs"], name=f"pn_h{h}_b{bi}")
                pt = pts[(h, g)]
                block_stage(h, bi, staged_tiles.pop((h, g, bi)), pn, pt,
                            CFG["ra_dve"])
                if CFG["pt_mode"] == "xbar":
                    xb_idx = idx + CFG["xbar_lag"]
                    defer(xb_idx, lambda bi=bi, pn=pn, pt=pt:
                          xbar_stage(bi, pn, pt))
                else:
                    xb_idx = idx + CFG["tr_lag"]
                    if CFG["tr_lag"]:
                        defer(xb_idx, lambda h=h, bi=bi, pn=pn, pt=pt:
                              transp_stage(h, bi, pn, pt))
                    else:
                        transp_stage(h, bi, pn, pt)
                if bi == 4 * g + 3:
                    av_idx = xb_idx + CFG["av_lag"]

                    def do_av(h=h, g=g, pt=pt):
                        avs[(h, g)] = psA.tile([65, 512], F32, tag="A",
                                               name=f"av_h{h}_g{g}")
                        av_stage(h, g, pt, avs[(h, g)])
                        pts.pop((h, g), None)
                    defer(av_idx, do_av)
                    nm_idx = av_idx + CFG["norm_lag"]
                    defer(nm_idx, lambda h=h, g=g:
                          norm_stage(h, g, avs.pop((h, g))))
                    if h == 3:
                        def do_op(g=g):
                            for rt in range(4 * g, 4 * g + 4):
                                outproj_rt(rt)
                        defer(nm_idx + CFG["op_lag"], do_op)
                for fn in deferred.pop(idx, []):
                    fn()
            # flush remaining deferred work in index order
            for idx in sorted(deferred):
                for fn in deferred[idx]:
                    fn()
    return nc


# ---------------- host side ----------------

def _bf16(x):
    return np.ascontiguousarray(x).astype(ml_dtypes.bfloat16)


def _col2d(vec256):
    """[256] f32 -> [128, 2] with v2d[p, a] = vec[128a + p]."""
    return np.ascontiguousarray(
        np.asarray(vec256, np.float32).reshape(2, 128).T)


def core_inputs(q_b, k_b, v_b, pos_enc, Wq, bq, Wk, bk, Wv, bv, Wo,
                r_w_bias, r_r_bias, r_kernel, g):
    sl = slice(256 * g, 256 * g + 256)
    rk_cat = np.concatenate([r_kernel[4 * g + i] for i in range(4)], axis=1)
    return {
        "q_in": _bf16(q_b.T),
        "k_in": _bf16(k_b.T),
        "v_in": _bf16(v_b.T),
        "pe_in": _bf16(pos_enc[1:2049].T),
        "wq": _bf16(Wq[:, sl]),
        "wk": _bf16(Wk[:, sl]),
        "wv": _bf16(Wv[:, sl]),
        "rk": _bf16(rk_cat),
        "wo": _bf16(Wo[sl, :]),
        "b1": _col2d(0.125 * (bq[sl] + r_w_bias[4 * g:4 * g + 4].reshape(256))),
        "b2": _col2d(0.125 * (bq[sl] + r_r_bias[4 * g:4 * g + 4].reshape(256))),
        "bk": _col2d(bk[sl]),
        "bv": _col2d(bv[sl]),
    }


_SHAPES = {
    "q_in": ([1024, 2048], BF), "k_in": ([1024, 2048], BF),
    "v_in": ([1024, 2048], BF), "pe_in": ([1024, 2048], BF),
    "wq": ([1024, 256], BF), "wk": ([1024, 256], BF), "wv": ([1024, 256], BF),
    "rk": ([1024, 256], BF), "wo": ([256, 1024], BF),
    "b1": ([128, 2], F32), "b2": ([128, 2], F32),
    "bk": ([128, 2], F32), "bv": ([128, 2], F32),
}

_NC_CACHE = {}


def _build():
    key = tuple(sorted((k, str(v)) for k, v in CFG.items()))
    if key in _NC_CACHE:
        return _NC_CACHE[key]
    nc = bacc.Bacc("TRN2", target_bir_lowering=False, debug=False,
                   enable_asserts=False)
    ins = {name: nc.dram_tensor(name, shape, dt, kind="ExternalInput").ap()
           for name, (shape, dt) in _SHAPES.items()}
    out = nc.dram_tensor("out", [2048, 1024], BF, kind="ExternalOutput").ap()
    emit_core(nc, ins, out)
    nc.compile()
    nc.m = get_hw_module(nc.m)
    _NC_CACHE[key] = nc
    return nc


def kernel(**inputs):
    global LAST_RESULTS
    inp = {k: np.asarray(v) for k, v in inputs.items()}
    nc = _build()
    in_maps = []
    for c in range(8):
        b, g = c // 4, c % 4
        in_maps.append(core_inputs(
            inp["q"][b], inp["k"][b], inp["v"][b], inp["pos_enc"],
            inp["Wq"], inp["bq"], inp["Wk"], inp["bk"], inp["Wv"], inp["bv"],
            inp["Wo"], inp["r_w_bias"], inp["r_r_bias"], inp["r_kernel"], g))
    res = bass_utils.run_bass_kernel_spmd(
        nc, in_maps, core_ids=list(range(8)), trace=PROFILE)
    LAST_RESULTS = res
    out = np.zeros((2, 2048, 1024), np.float32)
    for c in range(8):
        b = c // 4
        out[b] += np.asarray(res.results[c]["out"]).astype(np.float32)
    out += np.asarray(inp["bo"], np.float32)[None, None, :]
    return out


# revision 25
# speedup vs baseline: 1.6764x; 1.0165x over previous
"""Transformer-XL relative attention (B=2, L=2048, D=1024, H=16) on 8 TRN2
NeuronCores.

Sharding: data-parallel over batch x tensor-parallel over heads.  Core
c = 4*b + g handles batch b, head group g (4 heads).  Wq/Wk/Wv are
column-sharded, Wo row-sharded; each core emits a partial [2048,1024]
output (bf16) which the host sums per batch in f32 (+bo).

Per-core layouts (bf16 in SBUF, head pair = h//2, row = 64*(h%2)+e):
  q1t/q2t/kt : [128, 2*2048]  e-tile h//2 at cols [2048*(h//2)], free = seq
  peht       : [128, 2*2048]  relative-position keys pe[1:2049] @ r_kernel
  vh         : [128, 16*260]  per key-tile: 4 heads' V (64 cols each) + a
                              ones column per head (free row-sum trick)
  at (A^T)   : [128, 2*2048]  normalized attention output, transposed

Rel-shift: for query block bi (rows ri..ri+127) R_s[rr, x] holds
Q2 . peh[xmin + x] (xmin = 1920 - ri); the score chunk at columns
[c0, c0+wc) needs staged[rr, cc] = R_s[rr, 127 - rr + c0 + cc] -- an
anti-diagonal flat access pattern (offset 127+c0, ap=[[PITCH-1,128],[1,wc]])
which only DMA engines can execute (SBUF->SBUF).  R_s columns beyond the
causal edge are padded with -1e9; the anti-diagonal read maps exactly the
strict upper triangle onto that pad, so exp() yields the causal zeros with
no separate masking pass.

Projections run kc-outer with all 8 (et, rc) accumulation groups resident
in PSUM so input-chunk DMAs are consumed as they land.  SBUF pools are
phase-scoped: the input/weight pools are released after the projections,
freeing ~64 KB/partition for deep phase-B pipeline buffers.

The rel pipeline is software-pipelined one query-block ahead, and the
P^T transposes (PE matmuls, or batched DMA-crossbar in pt_mode=xbar),
AV, normalize and output projection are emitted with configurable lags
so their dependencies are complete when each in-order engine queue
reaches them.
"""

import numpy as np
import ml_dtypes
import concourse.bass as bass
import concourse.mybir as mybir
import concourse.tile as tile
from concourse import bacc
from concourse import bass_utils
from concourse.bass_interp import get_hw_module
from concourse.masks import make_identity

BF = mybir.dt.bfloat16
F32 = mybir.dt.float32
EXP = mybir.ActivationFunctionType.Exp
IDENT = mybir.ActivationFunctionType.Identity
MULT = mybir.AluOpType.mult
ADD = mybir.AluOpType.add

L = 2048          # sequence length
RS_W = 2176       # R_s tile width (2048 + 128 pad); anti-diag pitch = RS_W-1

PROFILE = False       # set by test harness to capture a trace
LAST_RESULTS = None   # BassKernelResults of the last run (for profiling)
MARKS = []            # (instruction id watermark, stage label) per build


def _mark(nc, label):
    MARKS.append((nc.next_id(), label))

# tuning knobs (read at build time)
CFG = {
    "diag_eng": "sync",    # engine issuing anti-diagonal staging DMAs
    "load_eng": "sync",    # engine issuing input/weight loads
    "rs_bufs": 4,
    "stg_bufs": 5,
    "pn_bufs": 6,
    "pt_bufs": 2,
    "psS_bufs": 2,
    "psR_bufs": 2,
    "psA_bufs": 2,
    "osb_bufs": 2,
    "rec64_bufs": 2,
    "r_evict": "mix",     # act | dve | mix
    "io_bufs": 3,
    "in_split": 4,        # input load DMAs per projection half
    "proj_order": "kpvq",
    "ra_dve": 0,          # rel-add chunks moved to DVE per block (0..4)
    "ra_pool": 0,         # rel-add chunks moved to Pool per block (0..4)
    "pad_eng": "vector",  # pad/ones memset engine: gpsimd | vector
    "pt_mode": "pe",      # pe (transpose matmuls) | xbar (batched DMA)
    "psT_bufs": 2,        # PSUM pool for PE transposes (pt_mode=pe)
    "pt_evict": "dve",    # psT -> pt eviction engine: act | dve | mix
    "tr_lag": 1,          # jobs between block chunks and its PE transposes
    "xbar_lag": 2,        # jobs between block_stage and its crossbar DMA
    "av_split": 1,        # 1: emit AV strips right after each block's tr
    "rel_lead": 2,        # how many jobs ahead the rel stage runs
    "av_lag": 1,          # jobs between last xbar of (h,g) and its AV
    "norm_lag": 1,        # jobs between AV and recip/bcast/normalize
    "op_lag": 1,          # jobs between norm(h=3) and outproj
    "bcast": "gpsimd",    # denominator broadcast: gpsimd | pe
    "store_eng": "sync",  # output store queue: sync | scalar
}


def emit_core(nc, ins, out):
    """ins: dict name->AP (DRAM), out: AP (DRAM [2048,1024] bf16)."""
    with tile.TileContext(nc) as tc:
        deng = getattr(nc, CFG["diag_eng"])
        leng = getattr(nc, CFG["load_eng"])
        peng = getattr(nc, CFG["pad_eng"] if CFG["pad_eng"] != "gpsimd"
                       else "gpsimd")
        with (
            tc.tile_pool(name="per", bufs=1) as per,
            tc.tile_pool(name="psS", bufs=CFG["psS_bufs"], space="PSUM") as psS,
            tc.tile_pool(name="psR", bufs=CFG["psR_bufs"], space="PSUM") as psR,
            tc.tile_pool(name="psA", bufs=CFG["psA_bufs"], space="PSUM") as psA,
            tc.tile_pool(name="psT", bufs=CFG["psT_bufs"], space="PSUM") as psT,
        ):
            # ---------------- persistent tiles ----------------
            q1t = per.tile([128, 2 * L], BF, tag="q1t")
            q2t = per.tile([128, 2 * L], BF, tag="q2t")
            kt = per.tile([128, 2 * L], BF, tag="kt")
            peht = per.tile([128, 2 * L], BF, tag="peht")
            at = per.tile([128, 2 * L], BF, tag="at")
            vh = per.tile([128, 16 * 260], BF, tag="vh")
            wo_s = per.tile([128, 2048], BF, tag="wo")
            b1s = per.tile([128, 2], F32, tag="b1s")
            b2s = per.tile([128, 2], F32, tag="b2s")
            bks = per.tile([128, 2], F32, tag="bks")
            bvs = per.tile([128, 2], F32, tag="bvs")

            ident = per.tile([128, 128], BF, tag="ident")
            make_identity(nc, ident[:])
            ones64 = per.tile([1, 64], BF, tag="ones64")
            nc.vector.memset(ones64[:], 1.0)
            # bias loads go through the Activation HWDGE queue so they don't
            # head-of-line block the SP queue ahead of the weight/input loads
            nc.scalar.dma_start(b1s[:], ins["b1"])
            nc.scalar.dma_start(b2s[:], ins["b2"])
            nc.scalar.dma_start(bks[:], ins["bk"])
            nc.scalar.dma_start(bvs[:], ins["bv"])

            def evict_q(et, rc, ps):
                cs = 2048 * et + 512 * rc
                nc.scalar.activation(q1t[:, cs:cs + 512], ps[:], IDENT,
                                     bias=b1s[:, et:et + 1], scale=0.125)
                nc.scalar.activation(q2t[:, cs:cs + 512], ps[:], IDENT,
                                     bias=b2s[:, et:et + 1], scale=0.125)

            def evict_k(et, rc, ps):
                cs = 2048 * et + 512 * rc
                nc.scalar.activation(kt[:, cs:cs + 512], ps[:], IDENT,
                                     bias=bks[:, et:et + 1], scale=1.0)

            def evict_pe(et, rc, ps):
                cs = 2048 * et + 512 * rc
                nc.vector.tensor_copy(peht[:, cs:cs + 512], ps[:])

            # ---------------- phase A: projections (scoped pools) ---------
            # kc-outer: all 8 (et, rc) accumulation groups live in PSUM
            # simultaneously (borrowing every PSUM pool), so each input
            # chunk DMA is consumed as soon as it lands.
            _proj_pools = [psS, psS, psR, psR, psR, psT, psT, psA]
            _proj_tags = ["S", "S", "R", "R", "R", "T", "T", "A"]

            with (
                tc.tile_pool(name="wgt", bufs=1) as wp,
                tc.tile_pool(name="io", bufs=CFG["io_bufs"]) as iop,
            ):
                vht = iop.tile([128, 2 * L], BF, tag="vht", bufs=1)

                def evict_v(et, rc, ps):
                    cs = 2048 * et + 512 * rc
                    nc.scalar.activation(vht[:, cs:cs + 512], ps[:], IDENT,
                                         bias=bvs[:, et:et + 1], scale=1.0)

                def load_w(name):
                    t = wp.tile([128, 2048], BF, tag=name, name=f"w_{name}")
                    leng.dma_start(
                        t[:], ins[name].rearrange("(a p) e -> p a e", p=128)
                    )
                    return t

                def project(xname, wname, evict):
                    w_s = load_w(wname)
                    src = ins[xname].rearrange("(a p) n -> p a n", p=128)
                    xts = []
                    for half in range(2):
                        xt = iop.tile([128, 4 * L], BF, tag="inT",
                                      name=f"in_{xname}_{half}")
                        ns = CFG["in_split"]
                        blk = 4 // ns
                        for s in range(ns):
                            leng.dma_start(
                                xt[:, 2048 * blk * s: 2048 * blk * (s + 1)],
                                src[:, 4 * half + blk * s:
                                    4 * half + blk * (s + 1), :],
                            )
                        xts.append(xt)
                    pss = {}
                    for i, (et, rc) in enumerate(
                            [(e, r) for e in range(2) for r in range(4)]):
                        pss[(et, rc)] = _proj_pools[i].tile(
                            [128, 512], F32, tag=_proj_tags[i],
                            name=f"ps_{xname}_{et}_{rc}")
                    for kc in range(8):
                        xt = xts[kc // 4]
                        kcc = kc % 4
                        for et in range(2):
                            for rc in range(4):
                                nc.tensor.matmul(
                                    pss[(et, rc)][:],
                                    w_s[:, 256 * kc + 128 * et:
                                        256 * kc + 128 * et + 128],
                                    xt[:, 2048 * kcc + 512 * rc:
                                       2048 * kcc + 512 * rc + 512],
                                    start=(kc == 0),
                                    stop=(kc == 7),
                                )
                    for et in range(2):
                        for rc in range(4):
                            evict(et, rc, pss[(et, rc)])

                projs = {"q": ("q_in", "wq", evict_q),
                         "k": ("k_in", "wk", evict_k),
                         "v": ("v_in", "wv", evict_v),
                         "p": ("pe_in", "rk", evict_pe)}
                for c in CFG["proj_order"]:
                    _mark(nc, f"proj_{c}")
                    project(*projs[c])
                leng.dma_start(
                    wo_s[:], ins["wo"].rearrange("(a p) e -> p a e", p=128))

                _mark(nc, "vh_asm")
                # vh assembly: PE-transpose one [128,128] tile per (ct,
                # pair), evict the two heads' column halves into vh blocks.
                for ct in range(16):
                    for p in range(2):
                        scr = psT.tile([128, 512], BF, tag="T",
                                       name=f"scr_{ct}_{p}")
                        nc.tensor.transpose(
                            scr[:, 0:128],
                            vht[:, 2048 * p + 128 * ct:
                                2048 * p + 128 * ct + 128],
                            ident[:],
                        )
                        for hh in range(2):
                            h4 = 2 * p + hh
                            nc.vector.tensor_copy(
                                vh[:, 260 * ct + 65 * h4:
                                   260 * ct + 65 * h4 + 64],
                                scr[:, 64 * hh: 64 * hh + 64],
                            )
                ones_cols = vh[:].rearrange("p (ct c) -> p ct c", c=260)
                for h in range(4):
                    peng.memset(ones_cols[:, :, 65 * h + 64], 1.0)

            # ---------------- phase B: attention (scoped pools) -----------
            with (
                tc.tile_pool(name="work", bufs=3) as wk,
                tc.tile_pool(name="pt", bufs=CFG["pt_bufs"]) as ptp,
            ):
                def outproj_rt(rt):
                    osb = wk.tile([128, 1024], BF, tag="osb",
                                  bufs=CFG["osb_bufs"], name=f"osb_{rt}")
                    for n in range(2):
                        op_ = psS.tile([128, 512], F32, tag="S",
                                       name=f"op_{rt}_{n}")
                        for hc in range(2):
                            nc.tensor.matmul(
                                op_[:],
                                at[:, 2048 * hc + 128 * rt:
                                   2048 * hc + 128 * rt + 128],
                                wo_s[:, 1024 * hc + 512 * n:
                                     1024 * hc + 512 * n + 512],
                                start=(hc == 0), stop=(hc == 1),
                            )
                        if n == 0:
                            nc.scalar.copy(osb[:, 0:512], op_[:])
                        else:
                            nc.vector.tensor_copy(osb[:, 512:1024], op_[:])
                    getattr(nc, CFG["store_eng"]).dma_start(
                        out[128 * rt: 128 * rt + 128, :], osb[:])

                # rel-score stage for one (h, bi): matmuls -> rs evict ->
                # pad memset -> anti-diagonal staging DMA.
                def rel_stage(h, bi):
                    et, ph = h // 2, h % 2
                    r0, r1 = 64 * ph, 64 * ph + 64
                    ri = 128 * bi
                    Wb = ri + 128
                    nch = (Wb + 511) // 512
                    xmin = 1920 - ri
                    rs = wk.tile([128, RS_W], BF, tag="rs",
                                 bufs=CFG["rs_bufs"], name=f"rs_h{h}_b{bi}")
                    for jc in range(nch):
                        w = min(512, Wb - 512 * jc)
                        rp = psR.tile([128, 512], F32, tag="R",
                                      name=f"rp_h{h}_b{bi}_{jc}")
                        nc.tensor.matmul(
                            rp[:, :w],
                            q2t[r0:r1, 2048 * et + ri: 2048 * et + ri + 128],
                            peht[r0:r1, 2048 * et + xmin + 512 * jc:
                                 2048 * et + xmin + 512 * jc + w],
                            start=True, stop=True,
                        )
                        use_act = (CFG["r_evict"] == "act" or
                                   (CFG["r_evict"] == "mix" and jc % 2 == 0))
                        if use_act:
                            nc.scalar.copy(rs[:, 512 * jc: 512 * jc + w],
                                           rp[:, :w])
                        else:
                            nc.vector.tensor_copy(
                                rs[:, 512 * jc: 512 * jc + w], rp[:, :w])
                    peng.memset(rs[:, Wb:Wb + 128], -1e9)
                    staged = wk.tile([128, 2048], BF, tag="stg",
                                     bufs=CFG["stg_bufs"],
                                     name=f"stg_h{h}_b{bi}")
                    diag = bass.AP(
                        tensor=rs.tensor,
                        offset=rs.offset + 127,
                        ap=[[RS_W - 1, 128], [1, Wb]],
                    )
                    deng.dma_start(staged[:, :Wb], diag)
                    return staged

                # content/ident/exp chunks for one (h, bi)
                def block_stage(h, bi, staged, pn, nrel_dve):
                    et, ph = h // 2, h % 2
                    r0, r1 = 64 * ph, 64 * ph + 64
                    ri = 128 * bi
                    Wb = ri + 128
                    nch = (Wb + 511) // 512
                    for ci_chunk in range(nch):
                        c0 = 512 * ci_chunk
                        wc = min(512, Wb - c0)
                        use_dve = ci_chunk < nrel_dve
                        use_pool = (not use_dve and
                                    ci_chunk < nrel_dve + CFG["ra_pool"])
                        sp = psS.tile([128, 512], F32, tag="S",
                                      name=f"sp_h{h}_b{bi}_{ci_chunk}")
                        nc.tensor.matmul(
                            sp[:, :wc],
                            q1t[r0:r1, 2048 * et + ri: 2048 * et + ri + 128],
                            kt[r0:r1, 2048 * et + c0: 2048 * et + c0 + wc],
                            start=True, stop=use_dve or use_pool,
                        )
                        if use_dve or use_pool:
                            eng = nc.vector if use_dve else nc.gpsimd
                            eng.scalar_tensor_tensor(
                                out=sp[:, :wc], in0=sp[:, :wc], scalar=1.0,
                                in1=staged[:, c0:c0 + wc],
                                op0=MULT, op1=ADD,
                            )
                        else:
                            nc.tensor.matmul(
                                sp[:, :wc],
                                ident[:],
                                staged[:, c0:c0 + wc],
                                start=False, stop=True,
                            )
                        nc.scalar.activation(pn[:, c0:c0 + wc],
                                             sp[:, :wc], EXP)

                # PE transposes for one (h, bi): emitted with a lag so the
                # exps they wait on are complete.
                def transp_stage(h, bi, pn, pt):
                    Wb = 128 * bi + 128
                    nch = (Wb + 511) // 512
                    for ci_chunk in range(nch):
                        c0 = 512 * ci_chunk
                        wc = min(512, Wb - c0)
                        tp_ = psT.tile([128, 512], BF, tag="T",
                                       name=f"tp_h{h}_b{bi}_{ci_chunk}")
                        for s in range(wc // 128):
                            nc.tensor.transpose(
                                tp_[:, 128 * s: 128 * s + 128],
                                pn[:, c0 + 128 * s: c0 + 128 * s + 128],
                                ident[:],
                            )
                        dst = bass.AP(
                            tensor=pt.tensor,
                            offset=pt.offset + 512 * (c0 // 128)
                            + 128 * (bi % 4),
                            ap=[[pt.tensor.shape[-1], 128],
                                [512, wc // 128], [1, 128]],
                        )
                        use_act = (CFG["pt_evict"] == "act" or
                                   (CFG["pt_evict"] == "mix"
                                    and (bi + ci_chunk) % 2 == 0))
                        if use_act:
                            nc.scalar.copy(dst, tp_[:, :wc])
                        else:
                            nc.vector.tensor_copy(dst, tp_[:, :wc])

                def xbar_stage(bi, pn, pt):
                    # batched crossbar transpose: pn [128, Wb] -> pt strips
                    Wb = 128 * bi + 128
                    dst = bass.AP(
                        tensor=pt.tensor,
                        offset=pt.offset + 128 * (bi % 4),
                        ap=[[pt.tensor.shape[-1], 128], [512, Wb // 128],
                            [1, 128]],
                    )
                    nc.sync.dma_start_transpose(dst, pn[:, :Wb])

                def av_stage(h, g, pt, av, ci_lo=0, ci_hi=None):
                    if ci_hi is None:
                        ci_hi = 4 * g + 4
                    for ci in range(ci_lo, ci_hi):
                        o = max(0, 128 * ci - 512 * g)
                        nc.tensor.matmul(
                            av[:, o:512],
                            vh[:, 260 * ci + 65 * h: 260 * ci + 65 * h + 65],
                            pt[:, 512 * ci + o: 512 * ci + 512],
                            start=(ci == 0), stop=(ci == 4 * g + 3),
                        )

                def av_col_stage(h, g, pt, av, qc):
                    # accumulate strips 0..4g+qc into psA columns
                    # [128qc, 128qc+128) -- ready right after block 4g+qc's
                    # transposes (the last writer of those pt columns)
                    nci = 4 * g + qc + 1
                    for ci in range(nci):
                        nc.tensor.matmul(
                            av[:, 128 * qc: 128 * qc + 128],
                            vh[:, 260 * ci + 65 * h: 260 * ci + 65 * h + 65],
                            pt[:, 512 * ci + 128 * qc: 512 * ci + 128 * qc + 128],
                            start=(ci == 0), stop=(ci == nci - 1),
                        )

                def norm_stage(h, g, av):
                    et, ph = h // 2, h % 2
                    r0, r1 = 64 * ph, 64 * ph + 64
                    rec = wk.tile([1, 512], F32, tag="rec",
                                  name=f"rec_h{h}_g{g}")
                    nc.vector.reciprocal(rec[:], av[64:65, :])
                    if CFG["bcast"] == "pe":
                        rec_bf = wk.tile([1, 512], BF, tag="recb",
                                         name=f"recb_h{h}_g{g}")
                        nc.vector.tensor_copy(rec_bf[:], rec[:])
                        rec64 = psT.tile([64, 512], F32, tag="T",
                                         name=f"rec64_h{h}_g{g}")
                        nc.tensor.matmul(rec64[:], ones64[:], rec_bf[:],
                                         start=True, stop=True)
                        nc.vector.tensor_tensor(
                            out=at[r0:r1, 2048 * et + 512 * g:
                                   2048 * et + 512 * g + 512],
                            in0=av[0:64, :],
                            in1=rec64[:],
                            op=MULT,
                        )
                    else:
                        rec64 = wk.tile([64, 512], F32, tag="rec64",
                                        bufs=CFG["rec64_bufs"],
                                        name=f"rec64_h{h}_g{g}")
                        nc.gpsimd.partition_broadcast(rec64[:], rec[:])
                        nc.vector.tensor_tensor(
                            out=at[r0:r1, 2048 * et + 512 * g:
                                   2048 * et + 512 * g + 512],
                            in0=av[0:64, :],
                            in1=rec64[:],
                            op=MULT,
                        )

                # software-pipelined schedule over jobs (h, g, bi)
                seq = [(h, g, bi) for g in range(4) for h in range(4)
                       for bi in range(4 * g, 4 * g + 4)]
                deferred = {}   # emit_idx -> list of callables

                def defer(idx, fn):
                    deferred.setdefault(idx, []).append(fn)

                pts = {}
                avs = {}
                staged_tiles = {}
                n = len(seq)
                lead = CFG["rel_lead"]
                for j in range(min(lead, n)):
                    _mark(nc, f"rel_{j}")
                    staged_tiles[seq[j]] = rel_stage(seq[j][0], seq[j][2])
                for idx, (h, g, bi) in enumerate(seq):
                    if (h, g) not in pts:
                        pts[(h, g)] = ptp.tile(
                            [128, 512 * (4 * g + 4)], BF, tag="pt",
                            bufs=CFG["pt_bufs"], name=f"pt_h{h}_g{g}")
                    if idx + lead < n:
                        nxt = seq[idx + lead]
                        _mark(nc, f"rel_{idx+lead}")
                        staged_tiles[nxt] = rel_stage(nxt[0], nxt[2])
                    pn = wk.tile([128, 2048], BF, tag="pn",
                                 bufs=CFG["pn_bufs"], name=f"pn_h{h}_b{bi}")
                    pt = pts[(h, g)]
                    _mark(nc, f"blk_{idx}")
                    block_stage(h, bi, staged_tiles.pop((h, g, bi)), pn,
                                CFG["ra_dve"])
                    if CFG["pt_mode"] == "xbar":
                        xb_idx = idx + CFG["xbar_lag"]
                        defer(xb_idx, lambda bi=bi, pn=pn, pt=pt, _i=idx:
                              (_mark(nc, f"xbar_{_i}"),
                               xbar_stage(bi, pn, pt)))
                    else:
                        xb_idx = idx + CFG["tr_lag"]
                        if CFG["tr_lag"]:
                            defer(xb_idx, lambda h=h, bi=bi, pn=pn, pt=pt,
                                  _i=idx:
                                  (_mark(nc, f"tr_{_i}"),
                                   transp_stage(h, bi, pn, pt)))
                        else:
                            transp_stage(h, bi, pn, pt)
                    if CFG["av_split"]:
                        def do_av_part(h=h, g=g, pt=pt, bi=bi):
                            _mark(nc, f"av_{h}_{g}")
                            if (h, g) not in avs:
                                avs[(h, g)] = psA.tile(
                                    [65, 512], F32, tag="A",
                                    name=f"av_h{h}_g{g}")
                            av_col_stage(h, g, pt, avs[(h, g)], bi % 4)
                            if bi % 4 == 3:
                                pts.pop((h, g), None)
                        defer(xb_idx + CFG["av_lag"], do_av_part)
                    if bi == 4 * g + 3:
                        av_idx = xb_idx + CFG["av_lag"]
                        if not CFG["av_split"]:
                            def do_av(h=h, g=g, pt=pt):
                                _mark(nc, f"av_{h}_{g}")
                                avs[(h, g)] = psA.tile(
                                    [65, 512], F32, tag="A",
                                    name=f"av_h{h}_g{g}")
                                av_stage(h, g, pt, avs[(h, g)])
                                pts.pop((h, g), None)
                            defer(av_idx, do_av)
                        nm_idx = av_idx + CFG["norm_lag"]
                        defer(nm_idx, lambda h=h, g=g:
                              (_mark(nc, f"nm_{h}_{g}"),
                               norm_stage(h, g, avs.pop((h, g)))))
                        if h == 3:
                            def do_op(g=g):
                                _mark(nc, f"op_{g}")
                                for rt in range(4 * g, 4 * g + 4):
                                    outproj_rt(rt)
                            defer(nm_idx + CFG["op_lag"], do_op)
                    for fn in deferred.pop(idx, []):
                        fn()
                # flush remaining deferred work in index order
                for idx in sorted(deferred):
                    for fn in deferred[idx]:
                        fn()
    return nc


# ---------------- host side ----------------

def _bf16(x):
    return np.ascontiguousarray(x).astype(ml_dtypes.bfloat16)


def _col2d(vec256):
    """[256] f32 -> [128, 2] with v2d[p, a] = vec[128a + p]."""
    return np.ascontiguousarray(
        np.asarray(vec256, np.float32).reshape(2, 128).T)


def core_inputs(q_b, k_b, v_b, pos_enc, Wq, bq, Wk, bk, Wv, bv, Wo,
                r_w_bias, r_r_bias, r_kernel, g):
    sl = slice(256 * g, 256 * g + 256)
    rk_cat = np.concatenate([r_kernel[4 * g + i] for i in range(4)], axis=1)
    return {
        "q_in": _bf16(q_b.T),
        "k_in": _bf16(k_b.T),
        "v_in": _bf16(v_b.T),
        "pe_in": _bf16(pos_enc[1:2049].T),
        "wq": _bf16(Wq[:, sl]),
        "wk": _bf16(Wk[:, sl]),
        "wv": _bf16(Wv[:, sl]),
        "rk": _bf16(rk_cat),
        "wo": _bf16(Wo[sl, :]),
        "b1": _col2d(0.125 * (bq[sl] + r_w_bias[4 * g:4 * g + 4].reshape(256))),
        "b2": _col2d(0.125 * (bq[sl] + r_r_bias[4 * g:4 * g + 4].reshape(256))),
        "bk": _col2d(bk[sl]),
        "bv": _col2d(bv[sl]),
    }


_SHAPES = {
    "q_in": ([1024, 2048], BF), "k_in": ([1024, 2048], BF),
    "v_in": ([1024, 2048], BF), "pe_in": ([1024, 2048], BF),
    "wq": ([1024, 256], BF), "wk": ([1024, 256], BF), "wv": ([1024, 256], BF),
    "rk": ([1024, 256], BF), "wo": ([256, 1024], BF),
    "b1": ([128, 2], F32), "b2": ([128, 2], F32),
    "bk": ([128, 2], F32), "bv": ([128, 2], F32),
}

_NC_CACHE = {}


def _build():
    key = tuple(sorted((k, str(v)) for k, v in CFG.items()))
    if key in _NC_CACHE:
        return _NC_CACHE[key]
    MARKS.clear()
    nc = bacc.Bacc("TRN2", target_bir_lowering=False, debug=False,
                   enable_asserts=False)
    ins = {name: nc.dram_tensor(name, shape, dt, kind="ExternalInput").ap()
           for name, (shape, dt) in _SHAPES.items()}
    out = nc.dram_tensor("out", [2048, 1024], BF, kind="ExternalOutput").ap()
    emit_core(nc, ins, out)
    nc.compile()
    nc.m = get_hw_module(nc.m)
    _NC_CACHE[key] = nc
    return nc


def kernel(**inputs):
    global LAST_RESULTS
    inp = {k: np.asarray(v) for k, v in inputs.items()}
    nc = _build()
    in_maps = []
    for c in range(8):
        b, g = c // 4, c % 4
        in_maps.append(core_inputs(
            inp["q"][b], inp["k"][b], inp["v"][b], inp["pos_enc"],
            inp["Wq"], inp["bq"], inp["Wk"], inp["bk"], inp["Wv"], inp["bv"],
            inp["Wo"], inp["r_w_bias"], inp["r_r_bias"], inp["r_kernel"], g))
    res = bass_utils.run_bass_kernel_spmd(
        nc, in_maps, core_ids=list(range(8)), trace=PROFILE)
    LAST_RESULTS = res
    out = np.zeros((2, 2048, 1024), np.float32)
    for c in range(8):
        b = c // 4
        out[b] += np.asarray(res.results[c]["out"]).astype(np.float32)
    out += np.asarray(inp["bo"], np.float32)[None, None, :]
    return out


# revision 26
# speedup vs baseline: 1.6953x; 1.0113x over previous
"""Transformer-XL relative attention (B=2, L=2048, D=1024, H=16) on 8 TRN2
NeuronCores.

Sharding: data-parallel over batch x tensor-parallel over heads.  Core
c = 4*b + g handles batch b, head group g (4 heads).  Wq/Wk/Wv are
column-sharded, Wo row-sharded; each core emits a partial [2048,1024]
output (bf16) which the host sums per batch in f32 (+bo).

Per-core layouts (bf16 in SBUF, head pair = h//2, row = 64*(h%2)+e):
  q1t/q2t/kt : [128, 2*2048]  e-tile h//2 at cols [2048*(h//2)], free = seq
  peht       : [128, 2*2048]  relative-position keys pe[1:2049] @ r_kernel
  vh         : [128, 16*260]  per key-tile: 4 heads' V (64 cols each) + a
                              ones column per head (free row-sum trick)
  at (A^T)   : [128, 2*2048]  normalized attention output, transposed

Rel-shift: for query block bi (rows ri..ri+127) R_s[rr, x] holds
Q2 . peh[xmin + x] (xmin = 1920 - ri); the score chunk at columns
[c0, c0+wc) needs staged[rr, cc] = R_s[rr, 127 - rr + c0 + cc] -- an
anti-diagonal flat access pattern (offset 127+c0, ap=[[PITCH-1,128],[1,wc]])
which only DMA engines can execute (SBUF->SBUF).  R_s columns beyond the
causal edge are padded with -1e9; the anti-diagonal read maps exactly the
strict upper triangle onto that pad, so exp() yields the causal zeros with
no separate masking pass.

Projections run kc-outer with all 8 (et, rc) accumulation groups resident
in PSUM so input-chunk DMAs are consumed as they land.  SBUF pools are
phase-scoped: the input/weight pools are released after the projections,
freeing ~64 KB/partition for deep phase-B pipeline buffers.

The rel pipeline is software-pipelined one query-block ahead, and the
P^T transposes (PE matmuls, or batched DMA-crossbar in pt_mode=xbar),
AV, normalize and output projection are emitted with configurable lags
so their dependencies are complete when each in-order engine queue
reaches them.
"""

import numpy as np
import ml_dtypes
import concourse.bass as bass
import concourse.mybir as mybir
import concourse.tile as tile
from concourse import bacc
from concourse import bass_utils
from concourse.bass_interp import get_hw_module
from concourse.masks import make_identity

BF = mybir.dt.bfloat16
F32 = mybir.dt.float32
EXP = mybir.ActivationFunctionType.Exp
IDENT = mybir.ActivationFunctionType.Identity
MULT = mybir.AluOpType.mult
ADD = mybir.AluOpType.add

L = 2048          # sequence length
RS_W = 2176       # R_s tile width (2048 + 128 pad); anti-diag pitch = RS_W-1

PROFILE = False       # set by test harness to capture a trace
LAST_RESULTS = None   # BassKernelResults of the last run (for profiling)
MARKS = []            # (instruction id watermark, stage label) per build


def _mark(nc, label):
    MARKS.append((nc.next_id(), label))

# tuning knobs (read at build time)
CFG = {
    "diag_eng": "sync",    # engine issuing anti-diagonal staging DMAs
    "load_eng": "sync",    # engine issuing input/weight loads
    "rs_bufs": 5,
    "stg_bufs": 6,
    "pn_bufs": 6,
    "pt_bufs": 2,
    "psS_bufs": 2,
    "psR_bufs": 2,
    "psA_bufs": 2,
    "osb_bufs": 2,
    "rec64_bufs": 2,
    "r_evict": "mix",     # act | dve | mix
    "io_bufs": 3,
    "in_split": 4,        # input load DMAs per projection half
    "proj_order": "kpvq",
    "ra_dve": 0,          # rel-add chunks moved to DVE per block (0..4)
    "ra_pool": 0,         # rel-add chunks moved to Pool per block (0..4)
    "pad_eng": "vector",  # pad/ones memset engine: gpsimd | vector
    "pt_mode": "pe",      # pe (transpose matmuls) | xbar (batched DMA)
    "psT_bufs": 2,        # PSUM pool for PE transposes (pt_mode=pe)
    "pt_evict": "dve",    # psT -> pt eviction engine: act | dve | mix
    "tr_lag": 1,          # jobs between block chunks and its PE transposes
    "xbar_lag": 2,        # jobs between block_stage and its crossbar DMA
    "av_split": 1,        # 1: emit AV strips right after each block's tr
    "rel_lead": 2,        # how many jobs ahead the rel stage runs
    "av_lag": 2,          # jobs between last xbar of (h,g) and its AV
    "norm_lag": 1,        # jobs between AV and recip/bcast/normalize
    "op_lag": 1,          # jobs between norm(h=3) and outproj
    "bcast": "gpsimd",    # denominator broadcast: gpsimd | pe
    "store_eng": "sync",  # output store queue: sync | scalar
}


def emit_core(nc, ins, out):
    """ins: dict name->AP (DRAM), out: AP (DRAM [2048,1024] bf16)."""
    with tile.TileContext(nc) as tc:
        deng = getattr(nc, CFG["diag_eng"])
        leng = getattr(nc, CFG["load_eng"])
        peng = getattr(nc, CFG["pad_eng"] if CFG["pad_eng"] != "gpsimd"
                       else "gpsimd")
        with (
            tc.tile_pool(name="per", bufs=1) as per,
            tc.tile_pool(name="psS", bufs=CFG["psS_bufs"], space="PSUM") as psS,
            tc.tile_pool(name="psR", bufs=CFG["psR_bufs"], space="PSUM") as psR,
            tc.tile_pool(name="psA", bufs=CFG["psA_bufs"], space="PSUM") as psA,
            tc.tile_pool(name="psT", bufs=CFG["psT_bufs"], space="PSUM") as psT,
        ):
            # ---------------- persistent tiles ----------------
            q1t = per.tile([128, 2 * L], BF, tag="q1t")
            q2t = per.tile([128, 2 * L], BF, tag="q2t")
            kt = per.tile([128, 2 * L], BF, tag="kt")
            peht = per.tile([128, 2 * L], BF, tag="peht")
            at = per.tile([128, 2 * L], BF, tag="at")
            vh = per.tile([128, 16 * 260], BF, tag="vh")
            wo_s = per.tile([128, 2048], BF, tag="wo")
            b1s = per.tile([128, 2], F32, tag="b1s")
            b2s = per.tile([128, 2], F32, tag="b2s")
            bks = per.tile([128, 2], F32, tag="bks")
            bvs = per.tile([128, 2], F32, tag="bvs")

            ident = per.tile([128, 128], BF, tag="ident")
            make_identity(nc, ident[:])
            ones64 = per.tile([1, 64], BF, tag="ones64")
            nc.vector.memset(ones64[:], 1.0)
            # bias loads go through the Activation HWDGE queue so they don't
            # head-of-line block the SP queue ahead of the weight/input loads
            nc.scalar.dma_start(b1s[:], ins["b1"])
            nc.scalar.dma_start(b2s[:], ins["b2"])
            nc.scalar.dma_start(bks[:], ins["bk"])
            nc.scalar.dma_start(bvs[:], ins["bv"])

            def evict_q(et, rc, ps):
                cs = 2048 * et + 512 * rc
                nc.scalar.activation(q1t[:, cs:cs + 512], ps[:], IDENT,
                                     bias=b1s[:, et:et + 1], scale=0.125)
                nc.scalar.activation(q2t[:, cs:cs + 512], ps[:], IDENT,
                                     bias=b2s[:, et:et + 1], scale=0.125)

            def evict_k(et, rc, ps):
                cs = 2048 * et + 512 * rc
                nc.scalar.activation(kt[:, cs:cs + 512], ps[:], IDENT,
                                     bias=bks[:, et:et + 1], scale=1.0)

            def evict_pe(et, rc, ps):
                cs = 2048 * et + 512 * rc
                nc.vector.tensor_copy(peht[:, cs:cs + 512], ps[:])

            # ---------------- phase A: projections (scoped pools) ---------
            # kc-outer: all 8 (et, rc) accumulation groups live in PSUM
            # simultaneously (borrowing every PSUM pool), so each input
            # chunk DMA is consumed as soon as it lands.
            _proj_pools = [psS, psS, psR, psR, psR, psT, psT, psA]
            _proj_tags = ["S", "S", "R", "R", "R", "T", "T", "A"]

            with (
                tc.tile_pool(name="wgt", bufs=1) as wp,
                tc.tile_pool(name="io", bufs=CFG["io_bufs"]) as iop,
            ):
                vht = iop.tile([128, 2 * L], BF, tag="vht", bufs=1)

                def evict_v(et, rc, ps):
                    cs = 2048 * et + 512 * rc
                    nc.scalar.activation(vht[:, cs:cs + 512], ps[:], IDENT,
                                         bias=bvs[:, et:et + 1], scale=1.0)

                def load_w(name):
                    t = wp.tile([128, 2048], BF, tag=name, name=f"w_{name}")
                    leng.dma_start(
                        t[:], ins[name].rearrange("(a p) e -> p a e", p=128)
                    )
                    return t

                def project(xname, wname, evict):
                    w_s = load_w(wname)
                    src = ins[xname].rearrange("(a p) n -> p a n", p=128)
                    xts = []
                    for half in range(2):
                        xt = iop.tile([128, 4 * L], BF, tag="inT",
                                      name=f"in_{xname}_{half}")
                        ns = CFG["in_split"]
                        blk = 4 // ns
                        for s in range(ns):
                            leng.dma_start(
                                xt[:, 2048 * blk * s: 2048 * blk * (s + 1)],
                                src[:, 4 * half + blk * s:
                                    4 * half + blk * (s + 1), :],
                            )
                        xts.append(xt)
                    pss = {}
                    for i, (et, rc) in enumerate(
                            [(e, r) for e in range(2) for r in range(4)]):
                        pss[(et, rc)] = _proj_pools[i].tile(
                            [128, 512], F32, tag=_proj_tags[i],
                            name=f"ps_{xname}_{et}_{rc}")
                    for kc in range(8):
                        xt = xts[kc // 4]
                        kcc = kc % 4
                        for et in range(2):
                            for rc in range(4):
                                nc.tensor.matmul(
                                    pss[(et, rc)][:],
                                    w_s[:, 256 * kc + 128 * et:
                                        256 * kc + 128 * et + 128],
                                    xt[:, 2048 * kcc + 512 * rc:
                                       2048 * kcc + 512 * rc + 512],
                                    start=(kc == 0),
                                    stop=(kc == 7),
                                )
                    for et in range(2):
                        for rc in range(4):
                            evict(et, rc, pss[(et, rc)])

                projs = {"q": ("q_in", "wq", evict_q),
                         "k": ("k_in", "wk", evict_k),
                         "v": ("v_in", "wv", evict_v),
                         "p": ("pe_in", "rk", evict_pe)}
                for c in CFG["proj_order"]:
                    _mark(nc, f"proj_{c}")
                    project(*projs[c])
                leng.dma_start(
                    wo_s[:], ins["wo"].rearrange("(a p) e -> p a e", p=128))

                _mark(nc, "vh_asm")
                # vh assembly: PE-transpose one [128,128] tile per (ct,
                # pair), evict the two heads' column halves into vh blocks.
                for ct in range(16):
                    for p in range(2):
                        scr = psT.tile([128, 512], BF, tag="T",
                                       name=f"scr_{ct}_{p}")
                        nc.tensor.transpose(
                            scr[:, 0:128],
                            vht[:, 2048 * p + 128 * ct:
                                2048 * p + 128 * ct + 128],
                            ident[:],
                        )
                        for hh in range(2):
                            h4 = 2 * p + hh
                            nc.vector.tensor_copy(
                                vh[:, 260 * ct + 65 * h4:
                                   260 * ct + 65 * h4 + 64],
                                scr[:, 64 * hh: 64 * hh + 64],
                            )
                ones_cols = vh[:].rearrange("p (ct c) -> p ct c", c=260)
                for h in range(4):
                    peng.memset(ones_cols[:, :, 65 * h + 64], 1.0)

            # ---------------- phase B: attention (scoped pools) -----------
            with (
                tc.tile_pool(name="work", bufs=3) as wk,
                tc.tile_pool(name="pt", bufs=CFG["pt_bufs"]) as ptp,
            ):
                def outproj_rt(rt):
                    osb = wk.tile([128, 1024], BF, tag="osb",
                                  bufs=CFG["osb_bufs"], name=f"osb_{rt}")
                    for n in range(2):
                        op_ = psS.tile([128, 512], F32, tag="S",
                                       name=f"op_{rt}_{n}")
                        for hc in range(2):
                            nc.tensor.matmul(
                                op_[:],
                                at[:, 2048 * hc + 128 * rt:
                                   2048 * hc + 128 * rt + 128],
                                wo_s[:, 1024 * hc + 512 * n:
                                     1024 * hc + 512 * n + 512],
                                start=(hc == 0), stop=(hc == 1),
                            )
                        if n == 0:
                            nc.scalar.copy(osb[:, 0:512], op_[:])
                        else:
                            nc.vector.tensor_copy(osb[:, 512:1024], op_[:])
                    getattr(nc, CFG["store_eng"]).dma_start(
                        out[128 * rt: 128 * rt + 128, :], osb[:])

                # rel-score stage for one (h, bi): matmuls -> rs evict ->
                # pad memset -> anti-diagonal staging DMA.
                def rel_stage(h, bi):
                    et, ph = h // 2, h % 2
                    r0, r1 = 64 * ph, 64 * ph + 64
                    ri = 128 * bi
                    Wb = ri + 128
                    nch = (Wb + 511) // 512
                    xmin = 1920 - ri
                    rs = wk.tile([128, RS_W], BF, tag="rs",
                                 bufs=CFG["rs_bufs"], name=f"rs_h{h}_b{bi}")
                    for jc in range(nch):
                        w = min(512, Wb - 512 * jc)
                        rp = psR.tile([128, 512], F32, tag="R",
                                      name=f"rp_h{h}_b{bi}_{jc}")
                        nc.tensor.matmul(
                            rp[:, :w],
                            q2t[r0:r1, 2048 * et + ri: 2048 * et + ri + 128],
                            peht[r0:r1, 2048 * et + xmin + 512 * jc:
                                 2048 * et + xmin + 512 * jc + w],
                            start=True, stop=True,
                        )
                        use_act = (CFG["r_evict"] == "act" or
                                   (CFG["r_evict"] == "mix" and jc % 2 == 0))
                        if use_act:
                            nc.scalar.copy(rs[:, 512 * jc: 512 * jc + w],
                                           rp[:, :w])
                        else:
                            nc.vector.tensor_copy(
                                rs[:, 512 * jc: 512 * jc + w], rp[:, :w])
                    peng.memset(rs[:, Wb:Wb + 128], -1e9)
                    staged = wk.tile([128, 2048], BF, tag="stg",
                                     bufs=CFG["stg_bufs"],
                                     name=f"stg_h{h}_b{bi}")
                    diag = bass.AP(
                        tensor=rs.tensor,
                        offset=rs.offset + 127,
                        ap=[[RS_W - 1, 128], [1, Wb]],
                    )
                    deng.dma_start(staged[:, :Wb], diag)
                    return staged

                # content/ident/exp chunks for one (h, bi)
                def block_stage(h, bi, staged, pn, nrel_dve):
                    et, ph = h // 2, h % 2
                    r0, r1 = 64 * ph, 64 * ph + 64
                    ri = 128 * bi
                    Wb = ri + 128
                    nch = (Wb + 511) // 512
                    for ci_chunk in range(nch):
                        c0 = 512 * ci_chunk
                        wc = min(512, Wb - c0)
                        use_dve = ci_chunk < nrel_dve
                        use_pool = (not use_dve and
                                    ci_chunk < nrel_dve + CFG["ra_pool"])
                        sp = psS.tile([128, 512], F32, tag="S",
                                      name=f"sp_h{h}_b{bi}_{ci_chunk}")
                        nc.tensor.matmul(
                            sp[:, :wc],
                            q1t[r0:r1, 2048 * et + ri: 2048 * et + ri + 128],
                            kt[r0:r1, 2048 * et + c0: 2048 * et + c0 + wc],
                            start=True, stop=use_dve or use_pool,
                        )
                        if use_dve or use_pool:
                            eng = nc.vector if use_dve else nc.gpsimd
                            eng.scalar_tensor_tensor(
                                out=sp[:, :wc], in0=sp[:, :wc], scalar=1.0,
                                in1=staged[:, c0:c0 + wc],
                                op0=MULT, op1=ADD,
                            )
                        else:
                            nc.tensor.matmul(
                                sp[:, :wc],
                                ident[:],
                                staged[:, c0:c0 + wc],
                                start=False, stop=True,
                            )
                        nc.scalar.activation(pn[:, c0:c0 + wc],
                                             sp[:, :wc], EXP)

                # PE transposes for one (h, bi): emitted with a lag so the
                # exps they wait on are complete.
                def transp_stage(h, bi, pn, pt):
                    Wb = 128 * bi + 128
                    nch = (Wb + 511) // 512
                    for ci_chunk in range(nch):
                        c0 = 512 * ci_chunk
                        wc = min(512, Wb - c0)
                        tp_ = psT.tile([128, 512], BF, tag="T",
                                       name=f"tp_h{h}_b{bi}_{ci_chunk}")
                        for s in range(wc // 128):
                            nc.tensor.transpose(
                                tp_[:, 128 * s: 128 * s + 128],
                                pn[:, c0 + 128 * s: c0 + 128 * s + 128],
                                ident[:],
                            )
                        dst = bass.AP(
                            tensor=pt.tensor,
                            offset=pt.offset + 512 * (c0 // 128)
                            + 128 * (bi % 4),
                            ap=[[pt.tensor.shape[-1], 128],
                                [512, wc // 128], [1, 128]],
                        )
                        use_act = (CFG["pt_evict"] == "act" or
                                   (CFG["pt_evict"] == "mix"
                                    and (bi + ci_chunk) % 2 == 0))
                        if use_act:
                            nc.scalar.copy(dst, tp_[:, :wc])
                        else:
                            nc.vector.tensor_copy(dst, tp_[:, :wc])

                def xbar_stage(bi, pn, pt):
                    # batched crossbar transpose: pn [128, Wb] -> pt strips
                    Wb = 128 * bi + 128
                    dst = bass.AP(
                        tensor=pt.tensor,
                        offset=pt.offset + 128 * (bi % 4),
                        ap=[[pt.tensor.shape[-1], 128], [512, Wb // 128],
                            [1, 128]],
                    )
                    nc.sync.dma_start_transpose(dst, pn[:, :Wb])

                def av_stage(h, g, pt, av, ci_lo=0, ci_hi=None):
                    if ci_hi is None:
                        ci_hi = 4 * g + 4
                    for ci in range(ci_lo, ci_hi):
                        o = max(0, 128 * ci - 512 * g)
                        nc.tensor.matmul(
                            av[:, o:512],
                            vh[:, 260 * ci + 65 * h: 260 * ci + 65 * h + 65],
                            pt[:, 512 * ci + o: 512 * ci + 512],
                            start=(ci == 0), stop=(ci == 4 * g + 3),
                        )

                def av_col_stage(h, g, pt, av, qc):
                    # accumulate strips 0..4g+qc into psA columns
                    # [128qc, 128qc+128) -- ready right after block 4g+qc's
                    # transposes (the last writer of those pt columns)
                    nci = 4 * g + qc + 1
                    for ci in range(nci):
                        nc.tensor.matmul(
                            av[:, 128 * qc: 128 * qc + 128],
                            vh[:, 260 * ci + 65 * h: 260 * ci + 65 * h + 65],
                            pt[:, 512 * ci + 128 * qc: 512 * ci + 128 * qc + 128],
                            start=(ci == 0), stop=(ci == nci - 1),
                        )

                def norm_stage(h, g, av):
                    et, ph = h // 2, h % 2
                    r0, r1 = 64 * ph, 64 * ph + 64
                    rec = wk.tile([1, 512], F32, tag="rec",
                                  name=f"rec_h{h}_g{g}")
                    nc.vector.reciprocal(rec[:], av[64:65, :])
                    if CFG["bcast"] == "pe":
                        rec_bf = wk.tile([1, 512], BF, tag="recb",
                                         name=f"recb_h{h}_g{g}")
                        nc.vector.tensor_copy(rec_bf[:], rec[:])
                        rec64 = psT.tile([64, 512], F32, tag="T",
                                         name=f"rec64_h{h}_g{g}")
                        nc.tensor.matmul(rec64[:], ones64[:], rec_bf[:],
                                         start=True, stop=True)
                        nc.vector.tensor_tensor(
                            out=at[r0:r1, 2048 * et + 512 * g:
                                   2048 * et + 512 * g + 512],
                            in0=av[0:64, :],
                            in1=rec64[:],
                            op=MULT,
                        )
                    else:
                        rec64 = wk.tile([64, 512], F32, tag="rec64",
                                        bufs=CFG["rec64_bufs"],
                                        name=f"rec64_h{h}_g{g}")
                        nc.gpsimd.partition_broadcast(rec64[:], rec[:])
                        nc.vector.tensor_tensor(
                            out=at[r0:r1, 2048 * et + 512 * g:
                                   2048 * et + 512 * g + 512],
                            in0=av[0:64, :],
                            in1=rec64[:],
                            op=MULT,
                        )

                # software-pipelined schedule over jobs (h, g, bi)
                seq = [(h, g, bi) for g in range(4) for h in range(4)
                       for bi in range(4 * g, 4 * g + 4)]
                deferred = {}   # emit_idx -> list of callables

                def defer(idx, fn):
                    deferred.setdefault(idx, []).append(fn)

                pts = {}
                avs = {}
                staged_tiles = {}
                n = len(seq)
                lead = CFG["rel_lead"]
                for j in range(min(lead, n)):
                    _mark(nc, f"rel_{j}")
                    staged_tiles[seq[j]] = rel_stage(seq[j][0], seq[j][2])
                for idx, (h, g, bi) in enumerate(seq):
                    if (h, g) not in pts:
                        pts[(h, g)] = ptp.tile(
                            [128, 512 * (4 * g + 4)], BF, tag="pt",
                            bufs=CFG["pt_bufs"], name=f"pt_h{h}_g{g}")
                    if idx + lead < n:
                        nxt = seq[idx + lead]
                        _mark(nc, f"rel_{idx+lead}")
                        staged_tiles[nxt] = rel_stage(nxt[0], nxt[2])
                    pn = wk.tile([128, 2048], BF, tag="pn",
                                 bufs=CFG["pn_bufs"], name=f"pn_h{h}_b{bi}")
                    pt = pts[(h, g)]
                    _mark(nc, f"blk_{idx}")
                    block_stage(h, bi, staged_tiles.pop((h, g, bi)), pn,
                                CFG["ra_dve"])
                    if CFG["pt_mode"] == "xbar":
                        xb_idx = idx + CFG["xbar_lag"]
                        defer(xb_idx, lambda bi=bi, pn=pn, pt=pt, _i=idx:
                              (_mark(nc, f"xbar_{_i}"),
                               xbar_stage(bi, pn, pt)))
                    else:
                        xb_idx = idx + CFG["tr_lag"]
                        if CFG["tr_lag"]:
                            defer(xb_idx, lambda h=h, bi=bi, pn=pn, pt=pt,
                                  _i=idx:
                                  (_mark(nc, f"tr_{_i}"),
                                   transp_stage(h, bi, pn, pt)))
                        else:
                            transp_stage(h, bi, pn, pt)
                    if CFG["av_split"]:
                        def do_av_part(h=h, g=g, pt=pt, bi=bi):
                            _mark(nc, f"av_{h}_{g}")
                            if (h, g) not in avs:
                                avs[(h, g)] = psA.tile(
                                    [65, 512], F32, tag="A",
                                    name=f"av_h{h}_g{g}")
                            av_col_stage(h, g, pt, avs[(h, g)], bi % 4)
                            if bi % 4 == 3:
                                pts.pop((h, g), None)
                        defer(xb_idx + CFG["av_lag"], do_av_part)
                    if bi == 4 * g + 3:
                        av_idx = xb_idx + CFG["av_lag"]
                        if not CFG["av_split"]:
                            def do_av(h=h, g=g, pt=pt):
                                _mark(nc, f"av_{h}_{g}")
                                avs[(h, g)] = psA.tile(
                                    [65, 512], F32, tag="A",
                                    name=f"av_h{h}_g{g}")
                                av_stage(h, g, pt, avs[(h, g)])
                                pts.pop((h, g), None)
                            defer(av_idx, do_av)
                        nm_idx = av_idx + CFG["norm_lag"]
                        defer(nm_idx, lambda h=h, g=g:
                              (_mark(nc, f"nm_{h}_{g}"),
                               norm_stage(h, g, avs.pop((h, g)))))
                        if h == 3:
                            def do_op(g=g):
                                _mark(nc, f"op_{g}")
                                for rt in range(4 * g, 4 * g + 4):
                                    outproj_rt(rt)
                            defer(nm_idx + CFG["op_lag"], do_op)
                    for fn in deferred.pop(idx, []):
                        fn()
                # flush remaining deferred work in index order
                for idx in sorted(deferred):
                    for fn in deferred[idx]:
                        fn()
    return nc


# ---------------- host side ----------------

def _bf16(x):
    return np.ascontiguousarray(x).astype(ml_dtypes.bfloat16)


def _col2d(vec256):
    """[256] f32 -> [128, 2] with v2d[p, a] = vec[128a + p]."""
    return np.ascontiguousarray(
        np.asarray(vec256, np.float32).reshape(2, 128).T)


def core_inputs(q_b, k_b, v_b, pos_enc, Wq, bq, Wk, bk, Wv, bv, Wo,
                r_w_bias, r_r_bias, r_kernel, g):
    sl = slice(256 * g, 256 * g + 256)
    rk_cat = np.concatenate([r_kernel[4 * g + i] for i in range(4)], axis=1)
    return {
        "q_in": _bf16(q_b.T),
        "k_in": _bf16(k_b.T),
        "v_in": _bf16(v_b.T),
        "pe_in": _bf16(pos_enc[1:2049].T),
        "wq": _bf16(Wq[:, sl]),
        "wk": _bf16(Wk[:, sl]),
        "wv": _bf16(Wv[:, sl]),
        "rk": _bf16(rk_cat),
        "wo": _bf16(Wo[sl, :]),
        "b1": _col2d(0.125 * (bq[sl] + r_w_bias[4 * g:4 * g + 4].reshape(256))),
        "b2": _col2d(0.125 * (bq[sl] + r_r_bias[4 * g:4 * g + 4].reshape(256))),
        "bk": _col2d(bk[sl]),
        "bv": _col2d(bv[sl]),
    }


_SHAPES = {
    "q_in": ([1024, 2048], BF), "k_in": ([1024, 2048], BF),
    "v_in": ([1024, 2048], BF), "pe_in": ([1024, 2048], BF),
    "wq": ([1024, 256], BF), "wk": ([1024, 256], BF), "wv": ([1024, 256], BF),
    "rk": ([1024, 256], BF), "wo": ([256, 1024], BF),
    "b1": ([128, 2], F32), "b2": ([128, 2], F32),
    "bk": ([128, 2], F32), "bv": ([128, 2], F32),
}

_NC_CACHE = {}


def _build():
    key = tuple(sorted((k, str(v)) for k, v in CFG.items()))
    if key in _NC_CACHE:
        return _NC_CACHE[key]
    MARKS.clear()
    nc = bacc.Bacc("TRN2", target_bir_lowering=False, debug=False,
                   enable_asserts=False)
    ins = {name: nc.dram_tensor(name, shape, dt, kind="ExternalInput").ap()
           for name, (shape, dt) in _SHAPES.items()}
    out = nc.dram_tensor("out", [2048, 1024], BF, kind="ExternalOutput").ap()
    emit_core(nc, ins, out)
    nc.compile()
    nc.m = get_hw_module(nc.m)
    _NC_CACHE[key] = nc
    return nc


def kernel(**inputs):
    global LAST_RESULTS
    inp = {k: np.asarray(v) for k, v in inputs.items()}
    nc = _build()
    in_maps = []
    for c in range(8):
        b, g = c // 4, c % 4
        in_maps.append(core_inputs(
            inp["q"][b], inp["k"][b], inp["v"][b], inp["pos_enc"],
            inp["Wq"], inp["bq"], inp["Wk"], inp["bk"], inp["Wv"], inp["bv"],
            inp["Wo"], inp["r_w_bias"], inp["r_r_bias"], inp["r_kernel"], g))
    res = bass_utils.run_bass_kernel_spmd(
        nc, in_maps, core_ids=list(range(8)), trace=PROFILE)
    LAST_RESULTS = res
    out = np.zeros((2, 2048, 1024), np.float32)
    for c in range(8):
        b = c // 4
        out[b] += np.asarray(res.results[c]["out"]).astype(np.float32)
    out += np.asarray(inp["bo"], np.float32)[None, None, :]
    return out


# revision 27
# speedup vs baseline: 1.7941x; 1.0583x over previous
"""Transformer-XL relative attention (B=2, L=2048, D=1024, H=16) on 8 TRN2
NeuronCores.

Sharding: data-parallel over batch x tensor-parallel over heads.  Core
c = 4*b + g handles batch b, head group g (4 heads).  Wq/Wk/Wv are
column-sharded, Wo row-sharded; each core emits a partial [2048,1024]
output (bf16) which the host sums per batch in f32 (+bo).

Per-core layouts (bf16 in SBUF, head pair = h//2, row = 64*(h%2)+e):
  q1t/q2t/kt : [128, 2*2048]  e-tile h//2 at cols [2048*(h//2)], free = seq
  peht       : [128, 2*2048]  relative-position keys pe[1:2049] @ r_kernel
  vh         : [128, 16*260]  per key-tile: 4 heads' V (64 cols each) + a
                              ones column per head (free row-sum trick)
  at (A^T)   : [128, 2*2048]  normalized attention output, transposed

Rel-shift: for query block bi (rows ri..ri+127) R_s[rr, x] holds
Q2 . peh[xmin + x] (xmin = 1920 - ri); the score chunk at columns
[c0, c0+wc) needs staged[rr, cc] = R_s[rr, 127 - rr + c0 + cc] -- an
anti-diagonal flat access pattern (offset 127+c0, ap=[[PITCH-1,128],[1,wc]])
which only DMA engines can execute (SBUF->SBUF).  R_s columns beyond the
causal edge are padded with -1e9; the anti-diagonal read maps exactly the
strict upper triangle onto that pad, so exp() yields the causal zeros with
no separate masking pass.

Projections run kc-outer with all 8 (et, rc) accumulation groups resident
in PSUM so input-chunk DMAs are consumed as they land.  SBUF pools are
phase-scoped: the input/weight pools are released after the projections,
freeing ~64 KB/partition for deep phase-B pipeline buffers.

The rel pipeline is software-pipelined one query-block ahead, and the
P^T transposes (PE matmuls, or batched DMA-crossbar in pt_mode=xbar),
AV, normalize and output projection are emitted with configurable lags
so their dependencies are complete when each in-order engine queue
reaches them.
"""

import numpy as np
import ml_dtypes
import concourse.bass as bass
import concourse.mybir as mybir
import concourse.tile as tile
from concourse import bacc
from concourse import bass_utils
from concourse.bass_interp import get_hw_module
from concourse.masks import make_identity

BF = mybir.dt.bfloat16
F32 = mybir.dt.float32
EXP = mybir.ActivationFunctionType.Exp
IDENT = mybir.ActivationFunctionType.Identity
MULT = mybir.AluOpType.mult
ADD = mybir.AluOpType.add

L = 2048          # sequence length
RS_W = 2176       # R_s tile width (2048 + 128 pad); anti-diag pitch = RS_W-1

PROFILE = False       # set by test harness to capture a trace
LAST_RESULTS = None   # BassKernelResults of the last run (for profiling)
MARKS = []            # (instruction id watermark, stage label) per build


def _mark(nc, label):
    MARKS.append((nc.next_id(), label))

# tuning knobs (read at build time)
CFG = {
    "diag_eng": "sync",    # engine issuing anti-diagonal staging DMAs
    "load_eng": "sync",    # engine issuing input/weight loads
    "rs_bufs": 5,
    "stg_bufs": 6,
    "pn_bufs": 6,
    "pt_bufs": 2,
    "psS_bufs": 2,
    "psR_bufs": 2,
    "psA_bufs": 2,
    "osb_bufs": 2,
    "rec64_bufs": 2,
    "r_evict": "mix",     # act | dve | mix
    "io_bufs": 3,
    "in_split": 4,        # input load DMAs per projection half
    "proj_order": "kpvq",
    "ra_dve": 0,          # rel-add chunks moved to DVE per block (0..4)
    "ra_pool": 0,         # rel-add chunks moved to Pool per block (0..4)
    "pad_eng": "vector",  # pad/ones memset engine: gpsimd | vector
    "pt_mode": "pe",      # pe (transpose matmuls) | xbar (batched DMA)
    "psT_bufs": 2,        # PSUM pool for PE transposes (pt_mode=pe)
    "pt_evict": "dve",    # psT -> pt eviction engine: act | dve | mix
    "tr_lag": 1,          # jobs between block chunks and its PE transposes
    "xbar_lag": 2,        # jobs between block_stage and its crossbar DMA
    "av_mode": "reor",    # reor (query-partition AV) | col (at-orientation)
    "av_split": 1,        # 1: emit AV strips right after each block's tr
    "rel_lead": 2,        # how many jobs ahead the rel stage runs
    "av_lag": 2,          # jobs between last xbar of (h,g) and its AV
    "norm_lag": 1,        # jobs between AV and recip/bcast/normalize
    "op_lag": 1,          # jobs between norm(h=3) and outproj
    "bcast": "gpsimd",    # denominator broadcast: gpsimd | pe
    "store_eng": "sync",  # output store queue: sync | scalar
}


def emit_core(nc, ins, out):
    """ins: dict name->AP (DRAM), out: AP (DRAM [2048,1024] bf16)."""
    with tile.TileContext(nc) as tc:
        deng = getattr(nc, CFG["diag_eng"])
        leng = getattr(nc, CFG["load_eng"])
        peng = getattr(nc, CFG["pad_eng"] if CFG["pad_eng"] != "gpsimd"
                       else "gpsimd")
        with (
            tc.tile_pool(name="per", bufs=1) as per,
            tc.tile_pool(name="psS", bufs=CFG["psS_bufs"], space="PSUM") as psS,
            tc.tile_pool(name="psR", bufs=CFG["psR_bufs"], space="PSUM") as psR,
            tc.tile_pool(name="psA", bufs=CFG["psA_bufs"], space="PSUM") as psA,
            tc.tile_pool(name="psT", bufs=CFG["psT_bufs"], space="PSUM") as psT,
        ):
            # ---------------- persistent tiles ----------------
            q1t = per.tile([128, 2 * L], BF, tag="q1t")
            q2t = per.tile([128, 2 * L], BF, tag="q2t")
            kt = per.tile([128, 2 * L], BF, tag="kt")
            peht = per.tile([128, 2 * L], BF, tag="peht")
            at = per.tile([128, 2 * L], BF, tag="at")
            vh = per.tile([128, 16 * 260], BF, tag="vh")
            wo_s = per.tile([128, 2048], BF, tag="wo")
            b1s = per.tile([128, 2], F32, tag="b1s")
            b2s = per.tile([128, 2], F32, tag="b2s")
            bks = per.tile([128, 2], F32, tag="bks")
            bvs = per.tile([128, 2], F32, tag="bvs")

            ident = per.tile([128, 128], BF, tag="ident")
            make_identity(nc, ident[:])
            ones64 = per.tile([1, 64], BF, tag="ones64")
            nc.vector.memset(ones64[:], 1.0)
            # bias loads go through the Activation HWDGE queue so they don't
            # head-of-line block the SP queue ahead of the weight/input loads
            nc.scalar.dma_start(b1s[:], ins["b1"])
            nc.scalar.dma_start(b2s[:], ins["b2"])
            nc.scalar.dma_start(bks[:], ins["bk"])
            nc.scalar.dma_start(bvs[:], ins["bv"])

            def evict_q(et, rc, ps):
                cs = 2048 * et + 512 * rc
                nc.scalar.activation(q1t[:, cs:cs + 512], ps[:], IDENT,
                                     bias=b1s[:, et:et + 1], scale=0.125)
                nc.scalar.activation(q2t[:, cs:cs + 512], ps[:], IDENT,
                                     bias=b2s[:, et:et + 1], scale=0.125)

            def evict_k(et, rc, ps):
                cs = 2048 * et + 512 * rc
                nc.scalar.activation(kt[:, cs:cs + 512], ps[:], IDENT,
                                     bias=bks[:, et:et + 1], scale=1.0)

            def evict_pe(et, rc, ps):
                cs = 2048 * et + 512 * rc
                nc.vector.tensor_copy(peht[:, cs:cs + 512], ps[:])

            # ---------------- phase A: projections (scoped pools) ---------
            # kc-outer: all 8 (et, rc) accumulation groups live in PSUM
            # simultaneously (borrowing every PSUM pool), so each input
            # chunk DMA is consumed as soon as it lands.
            _proj_pools = [psS, psS, psR, psR, psR, psT, psT, psA]
            _proj_tags = ["S", "S", "R", "R", "R", "T", "T", "A"]

            with (
                tc.tile_pool(name="wgt", bufs=1) as wp,
                tc.tile_pool(name="io", bufs=CFG["io_bufs"]) as iop,
            ):
                vht = iop.tile([128, 2 * L], BF, tag="vht", bufs=1)

                def evict_v(et, rc, ps):
                    cs = 2048 * et + 512 * rc
                    nc.scalar.activation(vht[:, cs:cs + 512], ps[:], IDENT,
                                         bias=bvs[:, et:et + 1], scale=1.0)

                def load_w(name):
                    t = wp.tile([128, 2048], BF, tag=name, name=f"w_{name}")
                    leng.dma_start(
                        t[:], ins[name].rearrange("(a p) e -> p a e", p=128)
                    )
                    return t

                def project(xname, wname, evict):
                    w_s = load_w(wname)
                    src = ins[xname].rearrange("(a p) n -> p a n", p=128)
                    xts = []
                    for half in range(2):
                        xt = iop.tile([128, 4 * L], BF, tag="inT",
                                      name=f"in_{xname}_{half}")
                        ns = CFG["in_split"]
                        blk = 4 // ns
                        for s in range(ns):
                            leng.dma_start(
                                xt[:, 2048 * blk * s: 2048 * blk * (s + 1)],
                                src[:, 4 * half + blk * s:
                                    4 * half + blk * (s + 1), :],
                            )
                        xts.append(xt)
                    pss = {}
                    for i, (et, rc) in enumerate(
                            [(e, r) for e in range(2) for r in range(4)]):
                        pss[(et, rc)] = _proj_pools[i].tile(
                            [128, 512], F32, tag=_proj_tags[i],
                            name=f"ps_{xname}_{et}_{rc}")
                    for kc in range(8):
                        xt = xts[kc // 4]
                        kcc = kc % 4
                        for et in range(2):
                            for rc in range(4):
                                nc.tensor.matmul(
                                    pss[(et, rc)][:],
                                    w_s[:, 256 * kc + 128 * et:
                                        256 * kc + 128 * et + 128],
                                    xt[:, 2048 * kcc + 512 * rc:
                                       2048 * kcc + 512 * rc + 512],
                                    start=(kc == 0),
                                    stop=(kc == 7),
                                )
                    for et in range(2):
                        for rc in range(4):
                            evict(et, rc, pss[(et, rc)])

                projs = {"q": ("q_in", "wq", evict_q),
                         "k": ("k_in", "wk", evict_k),
                         "v": ("v_in", "wv", evict_v),
                         "p": ("pe_in", "rk", evict_pe)}
                for c in CFG["proj_order"]:
                    _mark(nc, f"proj_{c}")
                    project(*projs[c])
                leng.dma_start(
                    wo_s[:], ins["wo"].rearrange("(a p) e -> p a e", p=128))

                _mark(nc, "vh_asm")
                # vh assembly: PE-transpose one [128,128] tile per (ct,
                # pair), evict the two heads' column halves into vh blocks.
                for ct in range(16):
                    for p in range(2):
                        scr = psT.tile([128, 512], BF, tag="T",
                                       name=f"scr_{ct}_{p}")
                        nc.tensor.transpose(
                            scr[:, 0:128],
                            vht[:, 2048 * p + 128 * ct:
                                2048 * p + 128 * ct + 128],
                            ident[:],
                        )
                        for hh in range(2):
                            h4 = 2 * p + hh
                            nc.vector.tensor_copy(
                                vh[:, 260 * ct + 65 * h4:
                                   260 * ct + 65 * h4 + 64],
                                scr[:, 64 * hh: 64 * hh + 64],
                            )
                ones_cols = vh[:].rearrange("p (ct c) -> p ct c", c=260)
                for h in range(4):
                    peng.memset(ones_cols[:, :, 65 * h + 64], 1.0)

            # ---------------- phase B: attention (scoped pools) -----------
            with (
                tc.tile_pool(name="work", bufs=3) as wk,
                tc.tile_pool(name="pt", bufs=CFG["pt_bufs"]) as ptp,
            ):
                def outproj_rt(rt):
                    osb = wk.tile([128, 1024], BF, tag="osb",
                                  bufs=CFG["osb_bufs"], name=f"osb_{rt}")
                    for n in range(2):
                        op_ = psS.tile([128, 512], F32, tag="S",
                                       name=f"op_{rt}_{n}")
                        for hc in range(2):
                            nc.tensor.matmul(
                                op_[:],
                                at[:, 2048 * hc + 128 * rt:
                                   2048 * hc + 128 * rt + 128],
                                wo_s[:, 1024 * hc + 512 * n:
                                     1024 * hc + 512 * n + 512],
                                start=(hc == 0), stop=(hc == 1),
                            )
                        if n == 0:
                            nc.scalar.copy(osb[:, 0:512], op_[:])
                        else:
                            nc.vector.tensor_copy(osb[:, 512:1024], op_[:])
                    getattr(nc, CFG["store_eng"]).dma_start(
                        out[128 * rt: 128 * rt + 128, :], osb[:])

                # rel-score stage for one (h, bi): matmuls -> rs evict ->
                # pad memset -> anti-diagonal staging DMA.
                def rel_stage(h, bi):
                    et, ph = h // 2, h % 2
                    r0, r1 = 64 * ph, 64 * ph + 64
                    ri = 128 * bi
                    Wb = ri + 128
                    nch = (Wb + 511) // 512
                    xmin = 1920 - ri
                    rs = wk.tile([128, RS_W], BF, tag="rs",
                                 bufs=CFG["rs_bufs"], name=f"rs_h{h}_b{bi}")
                    for jc in range(nch):
                        w = min(512, Wb - 512 * jc)
                        rp = psR.tile([128, 512], F32, tag="R",
                                      name=f"rp_h{h}_b{bi}_{jc}")
                        nc.tensor.matmul(
                            rp[:, :w],
                            q2t[r0:r1, 2048 * et + ri: 2048 * et + ri + 128],
                            peht[r0:r1, 2048 * et + xmin + 512 * jc:
                                 2048 * et + xmin + 512 * jc + w],
                            start=True, stop=True,
                        )
                        use_act = (CFG["r_evict"] == "act" or
                                   (CFG["r_evict"] == "mix" and jc % 2 == 0))
                        if use_act:
                            nc.scalar.copy(rs[:, 512 * jc: 512 * jc + w],
                                           rp[:, :w])
                        else:
                            nc.vector.tensor_copy(
                                rs[:, 512 * jc: 512 * jc + w], rp[:, :w])
                    peng.memset(rs[:, Wb:Wb + 128], -1e9)
                    staged = wk.tile([128, 2048], BF, tag="stg",
                                     bufs=CFG["stg_bufs"],
                                     name=f"stg_h{h}_b{bi}")
                    diag = bass.AP(
                        tensor=rs.tensor,
                        offset=rs.offset + 127,
                        ap=[[RS_W - 1, 128], [1, Wb]],
                    )
                    deng.dma_start(staged[:, :Wb], diag)
                    return staged

                # content/ident/exp chunks for one (h, bi)
                def block_stage(h, bi, staged, pn, nrel_dve):
                    et, ph = h // 2, h % 2
                    r0, r1 = 64 * ph, 64 * ph + 64
                    ri = 128 * bi
                    Wb = ri + 128
                    nch = (Wb + 511) // 512
                    for ci_chunk in range(nch):
                        c0 = 512 * ci_chunk
                        wc = min(512, Wb - c0)
                        use_dve = ci_chunk < nrel_dve
                        use_pool = (not use_dve and
                                    ci_chunk < nrel_dve + CFG["ra_pool"])
                        sp = psS.tile([128, 512], F32, tag="S",
                                      name=f"sp_h{h}_b{bi}_{ci_chunk}")
                        nc.tensor.matmul(
                            sp[:, :wc],
                            q1t[r0:r1, 2048 * et + ri: 2048 * et + ri + 128],
                            kt[r0:r1, 2048 * et + c0: 2048 * et + c0 + wc],
                            start=True, stop=use_dve or use_pool,
                        )
                        if use_dve or use_pool:
                            eng = nc.vector if use_dve else nc.gpsimd
                            eng.scalar_tensor_tensor(
                                out=sp[:, :wc], in0=sp[:, :wc], scalar=1.0,
                                in1=staged[:, c0:c0 + wc],
                                op0=MULT, op1=ADD,
                            )
                        else:
                            nc.tensor.matmul(
                                sp[:, :wc],
                                ident[:],
                                staged[:, c0:c0 + wc],
                                start=False, stop=True,
                            )
                        nc.scalar.activation(pn[:, c0:c0 + wc],
                                             sp[:, :wc], EXP)

                # PE transposes for one (h, bi): emitted with a lag so the
                # exps they wait on are complete.
                def transp_stage(h, bi, pn, pt):
                    Wb = 128 * bi + 128
                    nch = (Wb + 511) // 512
                    for ci_chunk in range(nch):
                        c0 = 512 * ci_chunk
                        wc = min(512, Wb - c0)
                        tp_ = psT.tile([128, 512], BF, tag="T",
                                       name=f"tp_h{h}_b{bi}_{ci_chunk}")
                        for s in range(wc // 128):
                            nc.tensor.transpose(
                                tp_[:, 128 * s: 128 * s + 128],
                                pn[:, c0 + 128 * s: c0 + 128 * s + 128],
                                ident[:],
                            )
                        dst = bass.AP(
                            tensor=pt.tensor,
                            offset=pt.offset + 512 * (c0 // 128)
                            + 128 * (bi % 4),
                            ap=[[pt.tensor.shape[-1], 128],
                                [512, wc // 128], [1, 128]],
                        )
                        use_act = (CFG["pt_evict"] == "act" or
                                   (CFG["pt_evict"] == "mix"
                                    and (bi + ci_chunk) % 2 == 0))
                        if use_act:
                            nc.scalar.copy(dst, tp_[:, :wc])
                        else:
                            nc.vector.tensor_copy(dst, tp_[:, :wc])

                def xbar_stage(bi, pn, pt):
                    # batched crossbar transpose: pn [128, Wb] -> pt strips
                    Wb = 128 * bi + 128
                    dst = bass.AP(
                        tensor=pt.tensor,
                        offset=pt.offset + 128 * (bi % 4),
                        ap=[[pt.tensor.shape[-1], 128], [512, Wb // 128],
                            [1, 128]],
                    )
                    nc.sync.dma_start_transpose(dst, pn[:, :Wb])

                def av_stage(h, g, pt, av, ci_lo=0, ci_hi=None):
                    if ci_hi is None:
                        ci_hi = 4 * g + 4
                    for ci in range(ci_lo, ci_hi):
                        o = max(0, 128 * ci - 512 * g)
                        nc.tensor.matmul(
                            av[:, o:512],
                            vh[:, 260 * ci + 65 * h: 260 * ci + 65 * h + 65],
                            pt[:, 512 * ci + o: 512 * ci + 512],
                            start=(ci == 0), stop=(ci == 4 * g + 3),
                        )

                def av_col_stage(h, g, pt, av, qc):
                    # accumulate strips 0..4g+qc into psA columns
                    # [128qc, 128qc+128) -- ready right after block 4g+qc's
                    # transposes (the last writer of those pt columns)
                    nci = 4 * g + qc + 1
                    for ci in range(nci):
                        nc.tensor.matmul(
                            av[:, 128 * qc: 128 * qc + 128],
                            vh[:, 260 * ci + 65 * h: 260 * ci + 65 * h + 65],
                            pt[:, 512 * ci + 128 * qc: 512 * ci + 128 * qc + 128],
                            start=(ci == 0), stop=(ci == nci - 1),
                        )

                def av_q_stage(h, qt, pt, av_q):
                    # out [128 q, 65]: stationary pt q-col block, moving vh
                    # (65th col = ones -> denominators); strips 0..qt are all
                    # complete once block bi=qt has transposed.
                    qc = qt % 4
                    for ci in range(qt + 1):
                        nc.tensor.matmul(
                            av_q[:, 0:65],
                            pt[:, 512 * ci + 128 * qc: 512 * ci + 128 * qc + 128],
                            vh[:, 260 * ci + 65 * h: 260 * ci + 65 * h + 65],
                            start=(ci == 0), stop=(ci == qt),
                        )

                def norm_q_stage(h, qt, av_q, avn2):
                    rec = wk.tile([128, 1], F32, tag="recq", bufs=3,
                                  name=f"recq_h{h}_q{qt}")
                    nc.vector.reciprocal(rec[:], av_q[:, 64:65])
                    ph = h % 2
                    nc.vector.tensor_scalar_mul(
                        avn2[:, 64 * ph: 64 * ph + 64], av_q[:, 0:64],
                        rec[:])

                def tp_q_stage(et, qt, avn2):
                    tp_ = psT.tile([128, 512], BF, tag="T",
                                   name=f"tpq_e{et}_q{qt}")
                    nc.tensor.transpose(tp_[:, 0:128], avn2[:], ident[:])
                    nc.vector.tensor_copy(
                        at[:, 2048 * et + 128 * qt: 2048 * et + 128 * qt + 128],
                        tp_[:, 0:128])

                def norm_stage(h, g, av):
                    et, ph = h // 2, h % 2
                    r0, r1 = 64 * ph, 64 * ph + 64
                    rec = wk.tile([1, 512], F32, tag="rec",
                                  name=f"rec_h{h}_g{g}")
                    nc.vector.reciprocal(rec[:], av[64:65, :])
                    if CFG["bcast"] == "pe":
                        rec_bf = wk.tile([1, 512], BF, tag="recb",
                                         name=f"recb_h{h}_g{g}")
                        nc.vector.tensor_copy(rec_bf[:], rec[:])
                        rec64 = psT.tile([64, 512], F32, tag="T",
                                         name=f"rec64_h{h}_g{g}")
                        nc.tensor.matmul(rec64[:], ones64[:], rec_bf[:],
                                         start=True, stop=True)
                        nc.vector.tensor_tensor(
                            out=at[r0:r1, 2048 * et + 512 * g:
                                   2048 * et + 512 * g + 512],
                            in0=av[0:64, :],
                            in1=rec64[:],
                            op=MULT,
                        )
                    else:
                        rec64 = wk.tile([64, 512], F32, tag="rec64",
                                        bufs=CFG["rec64_bufs"],
                                        name=f"rec64_h{h}_g{g}")
                        nc.gpsimd.partition_broadcast(rec64[:], rec[:])
                        nc.vector.tensor_tensor(
                            out=at[r0:r1, 2048 * et + 512 * g:
                                   2048 * et + 512 * g + 512],
                            in0=av[0:64, :],
                            in1=rec64[:],
                            op=MULT,
                        )

                # software-pipelined schedule over jobs (h, g, bi)
                seq = [(h, g, bi) for g in range(4) for h in range(4)
                       for bi in range(4 * g, 4 * g + 4)]
                deferred = {}   # emit_idx -> list of callables

                def defer(idx, fn):
                    deferred.setdefault(idx, []).append(fn)

                pts = {}
                avs = {}
                avn2s = {}
                staged_tiles = {}
                n = len(seq)
                lead = CFG["rel_lead"]
                for j in range(min(lead, n)):
                    _mark(nc, f"rel_{j}")
                    staged_tiles[seq[j]] = rel_stage(seq[j][0], seq[j][2])
                for idx, (h, g, bi) in enumerate(seq):
                    if (h, g) not in pts:
                        pts[(h, g)] = ptp.tile(
                            [128, 512 * (4 * g + 4)], BF, tag="pt",
                            bufs=CFG["pt_bufs"], name=f"pt_h{h}_g{g}")
                    if idx + lead < n:
                        nxt = seq[idx + lead]
                        _mark(nc, f"rel_{idx+lead}")
                        staged_tiles[nxt] = rel_stage(nxt[0], nxt[2])
                    pn = wk.tile([128, 2048], BF, tag="pn",
                                 bufs=CFG["pn_bufs"], name=f"pn_h{h}_b{bi}")
                    pt = pts[(h, g)]
                    _mark(nc, f"blk_{idx}")
                    block_stage(h, bi, staged_tiles.pop((h, g, bi)), pn,
                                CFG["ra_dve"])
                    if CFG["pt_mode"] == "xbar":
                        xb_idx = idx + CFG["xbar_lag"]
                        defer(xb_idx, lambda bi=bi, pn=pn, pt=pt, _i=idx:
                              (_mark(nc, f"xbar_{_i}"),
                               xbar_stage(bi, pn, pt)))
                    else:
                        xb_idx = idx + CFG["tr_lag"]
                        if CFG["tr_lag"]:
                            defer(xb_idx, lambda h=h, bi=bi, pn=pn, pt=pt,
                                  _i=idx:
                                  (_mark(nc, f"tr_{_i}"),
                                   transp_stage(h, bi, pn, pt)))
                        else:
                            transp_stage(h, bi, pn, pt)
                    if CFG["av_mode"] == "reor":
                        av_idx = xb_idx + CFG["av_lag"]

                        def do_av_q(h=h, g=g, pt=pt, qt=bi):
                            _mark(nc, f"av_{h}_{g}")
                            av_q = psA.tile([128, 65], F32, tag="A",
                                            name=f"avq_h{h}_q{qt}")
                            avs[(h, qt)] = av_q
                            av_q_stage(h, qt, pt, av_q)
                            if qt % 4 == 3:
                                pts.pop((h, g), None)
                        defer(av_idx, do_av_q)
                        nmq_idx = av_idx + CFG["norm_lag"]

                        def do_norm_q(h=h, qt=bi, et=h // 2):
                            _mark(nc, f"nm_{h}_{qt}")
                            if (et, qt) not in avn2s:
                                avn2s[(et, qt)] = wk.tile(
                                    [128, 128], BF, tag="avn2", bufs=8,
                                    name=f"avn2_e{et}_q{qt}")
                            norm_q_stage(h, qt, avs.pop((h, qt)),
                                         avn2s[(et, qt)])
                        defer(nmq_idx, do_norm_q)
                        if h % 2 == 1:
                            defer(nmq_idx + 1, lambda et=h // 2, qt=bi:
                                  tp_q_stage(et, qt, avn2s.pop((et, qt))))
                        if h == 3:
                            def do_op_rt(rt=bi):
                                _mark(nc, f"op_{g}")
                                outproj_rt(rt)
                            defer(nmq_idx + 1 + CFG["op_lag"], do_op_rt)
                    elif CFG["av_split"]:
                        def do_av_part(h=h, g=g, pt=pt, bi=bi):
                            _mark(nc, f"av_{h}_{g}")
                            if (h, g) not in avs:
                                avs[(h, g)] = psA.tile(
                                    [65, 512], F32, tag="A",
                                    name=f"av_h{h}_g{g}")
                            av_col_stage(h, g, pt, avs[(h, g)], bi % 4)
                            if bi % 4 == 3:
                                pts.pop((h, g), None)
                        defer(xb_idx + CFG["av_lag"], do_av_part)
                    if bi == 4 * g + 3 and CFG["av_mode"] != "reor":
                        av_idx = xb_idx + CFG["av_lag"]
                        if not CFG["av_split"]:
                            def do_av(h=h, g=g, pt=pt):
                                _mark(nc, f"av_{h}_{g}")
                                avs[(h, g)] = psA.tile(
                                    [65, 512], F32, tag="A",
                                    name=f"av_h{h}_g{g}")
                                av_stage(h, g, pt, avs[(h, g)])
                                pts.pop((h, g), None)
                            defer(av_idx, do_av)
                        nm_idx = av_idx + CFG["norm_lag"]
                        defer(nm_idx, lambda h=h, g=g:
                              (_mark(nc, f"nm_{h}_{g}"),
                               norm_stage(h, g, avs.pop((h, g)))))
                        if h == 3:
                            def do_op(g=g):
                                _mark(nc, f"op_{g}")
                                for rt in range(4 * g, 4 * g + 4):
                                    outproj_rt(rt)
                            defer(nm_idx + CFG["op_lag"], do_op)
                    for fn in deferred.pop(idx, []):
                        fn()
                # flush remaining deferred work in index order
                for idx in sorted(deferred):
                    for fn in deferred[idx]:
                        fn()
    return nc


# ---------------- host side ----------------

def _bf16(x):
    return np.ascontiguousarray(x).astype(ml_dtypes.bfloat16)


def _col2d(vec256):
    """[256] f32 -> [128, 2] with v2d[p, a] = vec[128a + p]."""
    return np.ascontiguousarray(
        np.asarray(vec256, np.float32).reshape(2, 128).T)


def core_inputs(q_b, k_b, v_b, pos_enc, Wq, bq, Wk, bk, Wv, bv, Wo,
                r_w_bias, r_r_bias, r_kernel, g):
    sl = slice(256 * g, 256 * g + 256)
    rk_cat = np.concatenate([r_kernel[4 * g + i] for i in range(4)], axis=1)
    return {
        "q_in": _bf16(q_b.T),
        "k_in": _bf16(k_b.T),
        "v_in": _bf16(v_b.T),
        "pe_in": _bf16(pos_enc[1:2049].T),
        "wq": _bf16(Wq[:, sl]),
        "wk": _bf16(Wk[:, sl]),
        "wv": _bf16(Wv[:, sl]),
        "rk": _bf16(rk_cat),
        "wo": _bf16(Wo[sl, :]),
        "b1": _col2d(0.125 * (bq[sl] + r_w_bias[4 * g:4 * g + 4].reshape(256))),
        "b2": _col2d(0.125 * (bq[sl] + r_r_bias[4 * g:4 * g + 4].reshape(256))),
        "bk": _col2d(bk[sl]),
        "bv": _col2d(bv[sl]),
    }


_SHAPES = {
    "q_in": ([1024, 2048], BF), "k_in": ([1024, 2048], BF),
    "v_in": ([1024, 2048], BF), "pe_in": ([1024, 2048], BF),
    "wq": ([1024, 256], BF), "wk": ([1024, 256], BF), "wv": ([1024, 256], BF),
    "rk": ([1024, 256], BF), "wo": ([256, 1024], BF),
    "b1": ([128, 2], F32), "b2": ([128, 2], F32),
    "bk": ([128, 2], F32), "bv": ([128, 2], F32),
}

_NC_CACHE = {}


def _build():
    key = tuple(sorted((k, str(v)) for k, v in CFG.items()))
    if key in _NC_CACHE:
        return _NC_CACHE[key]
    MARKS.clear()
    nc = bacc.Bacc("TRN2", target_bir_lowering=False, debug=False,
                   enable_asserts=False)
    ins = {name: nc.dram_tensor(name, shape, dt, kind="ExternalInput").ap()
           for name, (shape, dt) in _SHAPES.items()}
    out = nc.dram_tensor("out", [2048, 1024], BF, kind="ExternalOutput").ap()
    emit_core(nc, ins, out)
    nc.compile()
    nc.m = get_hw_module(nc.m)
    _NC_CACHE[key] = nc
    return nc


def kernel(**inputs):
    global LAST_RESULTS
    inp = {k: np.asarray(v) for k, v in inputs.items()}
    nc = _build()
    in_maps = []
    for c in range(8):
        b, g = c // 4, c % 4
        in_maps.append(core_inputs(
            inp["q"][b], inp["k"][b], inp["v"][b], inp["pos_enc"],
            inp["Wq"], inp["bq"], inp["Wk"], inp["bk"], inp["Wv"], inp["bv"],
            inp["Wo"], inp["r_w_bias"], inp["r_r_bias"], inp["r_kernel"], g))
    res = bass_utils.run_bass_kernel_spmd(
        nc, in_maps, core_ids=list(range(8)), trace=PROFILE)
    LAST_RESULTS = res
    out = np.zeros((2, 2048, 1024), np.float32)
    for c in range(8):
        b = c // 4
        out[b] += np.asarray(res.results[c]["out"]).astype(np.float32)
    out += np.asarray(inp["bo"], np.float32)[None, None, :]
    return out


# revision 32
# speedup vs baseline: 1.7982x; 1.0023x over previous
"""Transformer-XL relative attention (B=2, L=2048, D=1024, H=16) on 8 TRN2
NeuronCores.

Sharding: data-parallel over batch x tensor-parallel over heads.  Core
c = 4*b + g handles batch b, head group g (4 heads).  Wq/Wk/Wv are
column-sharded, Wo row-sharded; each core emits a partial [2048,1024]
output (bf16) which the host sums per batch in f32 (+bo).

Per-core layouts (bf16 in SBUF, head pair = h//2, row = 64*(h%2)+e):
  q1t/q2t/kt : [128, 2*2048]  e-tile h//2 at cols [2048*(h//2)], free = seq
  peht       : [128, 2*2048]  relative-position keys pe[1:2049] @ r_kernel
  vh         : [128, 16*260]  per key-tile: 4 heads' V (64 cols each) + a
                              ones column per head (free row-sum trick)
  at (A^T)   : [128, 2*2048]  normalized attention output, transposed

Rel-shift: for query block bi (rows ri..ri+127) R_s[rr, x] holds
Q2 . peh[xmin + x] (xmin = 1920 - ri); the score chunk at columns
[c0, c0+wc) needs staged[rr, cc] = R_s[rr, 127 - rr + c0 + cc] -- an
anti-diagonal flat access pattern (offset 127+c0, ap=[[PITCH-1,128],[1,wc]])
which only DMA engines can execute (SBUF->SBUF).  R_s columns beyond the
causal edge are padded with -1e9; the anti-diagonal read maps exactly the
strict upper triangle onto that pad, so exp() yields the causal zeros with
no separate masking pass.

Projections run kc-outer with all 8 (et, rc) accumulation groups resident
in PSUM so input-chunk DMAs are consumed as they land.  SBUF pools are
phase-scoped: the input/weight pools are released after the projections,
freeing ~64 KB/partition for deep phase-B pipeline buffers.

The rel pipeline is software-pipelined one query-block ahead, and the
P^T transposes (PE matmuls, or batched DMA-crossbar in pt_mode=xbar),
AV, normalize and output projection are emitted with configurable lags
so their dependencies are complete when each in-order engine queue
reaches them.
"""

import numpy as np
import ml_dtypes
import concourse.bass as bass
import concourse.mybir as mybir
import concourse.tile as tile
from concourse import bacc
from concourse import bass_utils
from concourse.bass_interp import get_hw_module
from concourse.masks import make_identity

BF = mybir.dt.bfloat16
F32 = mybir.dt.float32
EXP = mybir.ActivationFunctionType.Exp
IDENT = mybir.ActivationFunctionType.Identity
MULT = mybir.AluOpType.mult
ADD = mybir.AluOpType.add

L = 2048          # sequence length
RS_W = 2176       # R_s tile width (2048 + 128 pad); anti-diag pitch = RS_W-1

PROFILE = False       # set by test harness to capture a trace
LAST_RESULTS = None   # BassKernelResults of the last run (for profiling)
MARKS = []            # (instruction id watermark, stage label) per build


def _mark(nc, label):
    MARKS.append((nc.next_id(), label))

# tuning knobs (read at build time)
CFG = {
    "diag_eng": "sync",    # engine issuing anti-diagonal staging DMAs
    "load_eng": "sync",    # engine issuing input/weight loads
    "rs_bufs": 5,
    "stg_bufs": 6,
    "pn_bufs": 6,
    "pt_bufs": 2,
    "psS_bufs": 2,
    "psR_bufs": 2,
    "psA_bufs": 2,
    "osb_bufs": 2,
    "rec64_bufs": 2,
    "r_evict": "dve2",    # act | dve | mix | dve2
    "qkv_evict": "act",   # projection eviction engine: act | dve
    "io_bufs": 3,
    "in_split": 4,        # input load DMAs per projection half
    "proj_order": "kpvq",
    "ra_dve": 0,          # rel-add chunks moved to DVE per block (0..4)
    "ra_pool": 0,         # rel-add chunks moved to Pool per block (0..4)
    "pad_eng": "vector",  # pad/ones memset engine: gpsimd | vector
    "pt_mode": "pe",      # pe (transpose matmuls) | xbar (batched DMA)
    "psT_bufs": 2,        # PSUM pool for PE transposes (pt_mode=pe)
    "pt_evict": "dve",    # psT -> pt eviction engine: act | dve | mix
    "tr_lag": 1,          # jobs between block chunks and its PE transposes
    "xbar_lag": 2,        # jobs between block_stage and its crossbar DMA
    "av_mode": "reor",    # reor (query-partition AV) | col (at-orientation)
    "av_split": 1,        # 1: emit AV strips right after each block's tr
    "rel_lead": 2,        # how many jobs ahead the rel stage runs
    "rel_lead0": 2,       # rel lead for the first 8 jobs
    "g_order": "asc",    # group processing order: asc | desc
    "wk_split": 1,        # first weight load chunk count
    "av_lag": 2,          # jobs between last xbar of (h,g) and its AV
    "norm_lag": 1,        # jobs between AV and recip/bcast/normalize
    "op_lag": 1,          # jobs between norm(h=3) and outproj
    "bcast": "gpsimd",    # denominator broadcast: gpsimd | pe
    "store_eng": "sync",  # output store queue: sync | scalar
}


def emit_core(nc, ins, out):
    """ins: dict name->AP (DRAM), out: AP (DRAM [2048,1024] bf16)."""
    with tile.TileContext(nc) as tc:
        deng = getattr(nc, CFG["diag_eng"])
        leng = getattr(nc, CFG["load_eng"])
        peng = getattr(nc, CFG["pad_eng"] if CFG["pad_eng"] != "gpsimd"
                       else "gpsimd")
        with (
            tc.tile_pool(name="per", bufs=1) as per,
            tc.tile_pool(name="psS", bufs=CFG["psS_bufs"], space="PSUM") as psS,
            tc.tile_pool(name="psR", bufs=CFG["psR_bufs"], space="PSUM") as psR,
            tc.tile_pool(name="psA", bufs=CFG["psA_bufs"], space="PSUM") as psA,
            tc.tile_pool(name="psT", bufs=CFG["psT_bufs"], space="PSUM") as psT,
        ):
            # ---------------- persistent tiles ----------------
            q1t = per.tile([128, 2 * L], BF, tag="q1t")
            q2t = per.tile([128, 2 * L], BF, tag="q2t")
            kt = per.tile([128, 2 * L], BF, tag="kt")
            peht = per.tile([128, 2 * L], BF, tag="peht")
            at = per.tile([128, 2 * L], BF, tag="at")
            vh = per.tile([128, 16 * 260], BF, tag="vh")
            wo_s = per.tile([128, 2048], BF, tag="wo")
            b1s = per.tile([128, 2], F32, tag="b1s")
            b2s = per.tile([128, 2], F32, tag="b2s")
            bks = per.tile([128, 2], F32, tag="bks")
            bvs = per.tile([128, 2], F32, tag="bvs")

            ident = per.tile([128, 128], BF, tag="ident")
            make_identity(nc, ident[:])
            ones64 = per.tile([1, 64], BF, tag="ones64")
            nc.vector.memset(ones64[:], 1.0)
            # bias loads go through the Activation HWDGE queue so they don't
            # head-of-line block the SP queue ahead of the weight/input loads
            nc.scalar.dma_start(b1s[:], ins["b1"])
            nc.scalar.dma_start(b2s[:], ins["b2"])
            nc.scalar.dma_start(bks[:], ins["bk"])
            nc.scalar.dma_start(bvs[:], ins["bv"])

            def _pevict(dst, ps, bias, scale):
                if CFG["qkv_evict"] == "dve":
                    nc.vector.tensor_scalar(
                        out=dst, in0=ps[:], scalar1=scale, scalar2=bias,
                        op0=MULT, op1=ADD)
                else:
                    nc.scalar.activation(dst, ps[:], IDENT, bias=bias,
                                         scale=scale)

            def evict_q(et, rc, ps):
                cs = 2048 * et + 512 * rc
                if CFG["qkv_evict"] == "qdve":
                    nc.vector.tensor_scalar(
                        out=q1t[:, cs:cs + 512], in0=ps[:], scalar1=0.125,
                        scalar2=b1s[:, et:et + 1], op0=MULT, op1=ADD)
                    nc.vector.tensor_scalar(
                        out=q2t[:, cs:cs + 512], in0=ps[:], scalar1=0.125,
                        scalar2=b2s[:, et:et + 1], op0=MULT, op1=ADD)
                else:
                    _pevict(q1t[:, cs:cs + 512], ps, b1s[:, et:et + 1], 0.125)
                    _pevict(q2t[:, cs:cs + 512], ps, b2s[:, et:et + 1], 0.125)

            def evict_k(et, rc, ps):
                cs = 2048 * et + 512 * rc
                _pevict(kt[:, cs:cs + 512], ps, bks[:, et:et + 1], 1.0)

            def evict_pe(et, rc, ps):
                cs = 2048 * et + 512 * rc
                nc.vector.tensor_copy(peht[:, cs:cs + 512], ps[:])

            # ---------------- phase A: projections (scoped pools) ---------
            # kc-outer: all 8 (et, rc) accumulation groups live in PSUM
            # simultaneously (borrowing every PSUM pool), so each input
            # chunk DMA is consumed as soon as it lands.
            _proj_pools = [psS, psS, psR, psR, psR, psT, psT, psA]
            _proj_tags = ["S", "S", "R", "R", "R", "T", "T", "A"]

            with (
                tc.tile_pool(name="wgt", bufs=1) as wp,
                tc.tile_pool(name="io", bufs=CFG["io_bufs"]) as iop,
            ):
                vht = iop.tile([128, 2 * L], BF, tag="vht", bufs=1)

                def evict_v(et, rc, ps):
                    cs = 2048 * et + 512 * rc
                    _pevict(vht[:, cs:cs + 512], ps, bvs[:, et:et + 1], 1.0)

                def load_w(name, split=1):
                    t = wp.tile([128, 2048], BF, tag=name, name=f"w_{name}")
                    src = ins[name].rearrange("(a p) e -> p a e", p=128)
                    blk = 8 // split
                    for si in range(split):
                        leng.dma_start(
                            t[:, 256 * blk * si: 256 * blk * (si + 1)],
                            src[:, blk * si: blk * (si + 1), :])
                    return t

                _first_w = ["wq", "wk", "wv", "rk"][
                    "qkvp".index(CFG["proj_order"][0])]

                def project(xname, wname, evict):
                    w_s = load_w(wname, CFG["wk_split"]
                                 if wname == _first_w else 1)
                    src = ins[xname].rearrange("(a p) n -> p a n", p=128)
                    xts = []
                    for half in range(2):
                        xt = iop.tile([128, 4 * L], BF, tag="inT",
                                      name=f"in_{xname}_{half}")
                        ns = CFG["in_split"]
                        blk = 4 // ns
                        for s in range(ns):
                            leng.dma_start(
                                xt[:, 2048 * blk * s: 2048 * blk * (s + 1)],
                                src[:, 4 * half + blk * s:
                                    4 * half + blk * (s + 1), :],
                            )
                        xts.append(xt)
                    pss = {}
                    for i, (et, rc) in enumerate(
                            [(e, r) for e in range(2) for r in range(4)]):
                        pss[(et, rc)] = _proj_pools[i].tile(
                            [128, 512], F32, tag=_proj_tags[i],
                            name=f"ps_{xname}_{et}_{rc}")
                    for kc in range(8):
                        xt = xts[kc // 4]
                        kcc = kc % 4
                        for et in range(2):
                            for rc in range(4):
                                nc.tensor.matmul(
                                    pss[(et, rc)][:],
                                    w_s[:, 256 * kc + 128 * et:
                                        256 * kc + 128 * et + 128],
                                    xt[:, 2048 * kcc + 512 * rc:
                                       2048 * kcc + 512 * rc + 512],
                                    start=(kc == 0),
                                    stop=(kc == 7),
                                )
                    for et in range(2):
                        for rc in range(4):
                            evict(et, rc, pss[(et, rc)])

                projs = {"q": ("q_in", "wq", evict_q),
                         "k": ("k_in", "wk", evict_k),
                         "v": ("v_in", "wv", evict_v),
                         "p": ("pe_in", "rk", evict_pe)}
                for c in CFG["proj_order"]:
                    _mark(nc, f"proj_{c}")
                    project(*projs[c])
                leng.dma_start(
                    wo_s[:], ins["wo"].rearrange("(a p) e -> p a e", p=128))

                _mark(nc, "vh_asm")
                # vh assembly: PE-transpose one [128,128] tile per (ct,
                # pair), evict the two heads' column halves into vh blocks.
                for ct in range(16):
                    for p in range(2):
                        scr = psT.tile([128, 512], BF, tag="T",
                                       name=f"scr_{ct}_{p}")
                        nc.tensor.transpose(
                            scr[:, 0:128],
                            vht[:, 2048 * p + 128 * ct:
                                2048 * p + 128 * ct + 128],
                            ident[:],
                        )
                        for hh in range(2):
                            h4 = 2 * p + hh
                            nc.vector.tensor_copy(
                                vh[:, 260 * ct + 65 * h4:
                                   260 * ct + 65 * h4 + 64],
                                scr[:, 64 * hh: 64 * hh + 64],
                            )
                ones_cols = vh[:].rearrange("p (ct c) -> p ct c", c=260)
                for h in range(4):
                    peng.memset(ones_cols[:, :, 65 * h + 64], 1.0)

            # ---------------- phase B: attention (scoped pools) -----------
            with (
                tc.tile_pool(name="work", bufs=3) as wk,
                tc.tile_pool(name="pt", bufs=CFG["pt_bufs"]) as ptp,
            ):
                def outproj_rt(rt):
                    osb = wk.tile([128, 1024], BF, tag="osb",
                                  bufs=CFG["osb_bufs"], name=f"osb_{rt}")
                    for n in range(2):
                        op_ = psS.tile([128, 512], F32, tag="S",
                                       name=f"op_{rt}_{n}")
                        for hc in range(2):
                            nc.tensor.matmul(
                                op_[:],
                                at[:, 2048 * hc + 128 * rt:
                                   2048 * hc + 128 * rt + 128],
                                wo_s[:, 1024 * hc + 512 * n:
                                     1024 * hc + 512 * n + 512],
                                start=(hc == 0), stop=(hc == 1),
                            )
                        if n == 0:
                            nc.scalar.copy(osb[:, 0:512], op_[:])
                        else:
                            nc.vector.tensor_copy(osb[:, 512:1024], op_[:])
                    getattr(nc, CFG["store_eng"]).dma_start(
                        out[128 * rt: 128 * rt + 128, :], osb[:])

                # rel-score stage for one (h, bi): matmuls -> rs evict ->
                # pad memset -> anti-diagonal staging DMA.
                def rel_stage(h, bi):
                    et, ph = h // 2, h % 2
                    r0, r1 = 64 * ph, 64 * ph + 64
                    ri = 128 * bi
                    Wb = ri + 128
                    nch = (Wb + 511) // 512
                    xmin = 1920 - ri
                    rs = wk.tile([128, RS_W], BF, tag="rs",
                                 bufs=CFG["rs_bufs"], name=f"rs_h{h}_b{bi}")
                    for jc in range(nch):
                        w = min(512, Wb - 512 * jc)
                        rp = psR.tile([128, 512], F32, tag="R",
                                      name=f"rp_h{h}_b{bi}_{jc}")
                        nc.tensor.matmul(
                            rp[:, :w],
                            q2t[r0:r1, 2048 * et + ri: 2048 * et + ri + 128],
                            peht[r0:r1, 2048 * et + xmin + 512 * jc:
                                 2048 * et + xmin + 512 * jc + w],
                            start=True, stop=True,
                        )
                        use_act = (CFG["r_evict"] == "act" or
                                   (CFG["r_evict"] == "mix" and jc % 2 == 0) or
                                   (CFG["r_evict"] == "dve2" and jc % 3 == 0))
                        if use_act:
                            nc.scalar.copy(rs[:, 512 * jc: 512 * jc + w],
                                           rp[:, :w])
                        else:
                            nc.vector.tensor_copy(
                                rs[:, 512 * jc: 512 * jc + w], rp[:, :w])
                    peng.memset(rs[:, Wb:Wb + 128], -1e9)
                    staged = wk.tile([128, 2048], BF, tag="stg",
                                     bufs=CFG["stg_bufs"],
                                     name=f"stg_h{h}_b{bi}")
                    diag = bass.AP(
                        tensor=rs.tensor,
                        offset=rs.offset + 127,
                        ap=[[RS_W - 1, 128], [1, Wb]],
                    )
                    deng.dma_start(staged[:, :Wb], diag)
                    return staged

                # content/ident/exp chunks for one (h, bi)
                def block_stage(h, bi, staged, pn, nrel_dve):
                    et, ph = h // 2, h % 2
                    r0, r1 = 64 * ph, 64 * ph + 64
                    ri = 128 * bi
                    Wb = ri + 128
                    nch = (Wb + 511) // 512
                    for ci_chunk in range(nch):
                        c0 = 512 * ci_chunk
                        wc = min(512, Wb - c0)
                        use_dve = ci_chunk < nrel_dve
                        use_pool = (not use_dve and
                                    ci_chunk < nrel_dve + CFG["ra_pool"])
                        sp = psS.tile([128, 512], F32, tag="S",
                                      name=f"sp_h{h}_b{bi}_{ci_chunk}")
                        nc.tensor.matmul(
                            sp[:, :wc],
                            q1t[r0:r1, 2048 * et + ri: 2048 * et + ri + 128],
                            kt[r0:r1, 2048 * et + c0: 2048 * et + c0 + wc],
                            start=True, stop=use_dve or use_pool,
                        )
                        if use_dve or use_pool:
                            eng = nc.vector if use_dve else nc.gpsimd
                            eng.scalar_tensor_tensor(
                                out=sp[:, :wc], in0=sp[:, :wc], scalar=1.0,
                                in1=staged[:, c0:c0 + wc],
                                op0=MULT, op1=ADD,
                            )
                        else:
                            nc.tensor.matmul(
                                sp[:, :wc],
                                ident[:],
                                staged[:, c0:c0 + wc],
                                start=False, stop=True,
                            )
                        nc.scalar.activation(pn[:, c0:c0 + wc],
                                             sp[:, :wc], EXP)

                # PE transposes for one (h, bi): emitted with a lag so the
                # exps they wait on are complete.
                def transp_stage(h, bi, pn, pt):
                    Wb = 128 * bi + 128
                    nch = (Wb + 511) // 512
                    for ci_chunk in range(nch):
                        c0 = 512 * ci_chunk
                        wc = min(512, Wb - c0)
                        tp_ = psT.tile([128, 512], BF, tag="T",
                                       name=f"tp_h{h}_b{bi}_{ci_chunk}")
                        for s in range(wc // 128):
                            nc.tensor.transpose(
                                tp_[:, 128 * s: 128 * s + 128],
                                pn[:, c0 + 128 * s: c0 + 128 * s + 128],
                                ident[:],
                            )
                        dst = bass.AP(
                            tensor=pt.tensor,
                            offset=pt.offset + 512 * (c0 // 128)
                            + 128 * (bi % 4),
                            ap=[[pt.tensor.shape[-1], 128],
                                [512, wc // 128], [1, 128]],
                        )
                        use_act = (CFG["pt_evict"] == "act" or
                                   (CFG["pt_evict"] == "mix"
                                    and (bi + ci_chunk) % 2 == 0))
                        if use_act:
                            nc.scalar.copy(dst, tp_[:, :wc])
                        else:
                            nc.vector.tensor_copy(dst, tp_[:, :wc])

                def xbar_stage(bi, pn, pt):
                    # batched crossbar transpose: pn [128, Wb] -> pt strips
                    Wb = 128 * bi + 128
                    dst = bass.AP(
                        tensor=pt.tensor,
                        offset=pt.offset + 128 * (bi % 4),
                        ap=[[pt.tensor.shape[-1], 128], [512, Wb // 128],
                            [1, 128]],
                    )
                    nc.sync.dma_start_transpose(dst, pn[:, :Wb])

                def av_stage(h, g, pt, av, ci_lo=0, ci_hi=None):
                    if ci_hi is None:
                        ci_hi = 4 * g + 4
                    for ci in range(ci_lo, ci_hi):
                        o = max(0, 128 * ci - 512 * g)
                        nc.tensor.matmul(
                            av[:, o:512],
                            vh[:, 260 * ci + 65 * h: 260 * ci + 65 * h + 65],
                            pt[:, 512 * ci + o: 512 * ci + 512],
                            start=(ci == 0), stop=(ci == 4 * g + 3),
                        )

                def av_col_stage(h, g, pt, av, qc):
                    # accumulate strips 0..4g+qc into psA columns
                    # [128qc, 128qc+128) -- ready right after block 4g+qc's
                    # transposes (the last writer of those pt columns)
                    nci = 4 * g + qc + 1
                    for ci in range(nci):
                        nc.tensor.matmul(
                            av[:, 128 * qc: 128 * qc + 128],
                            vh[:, 260 * ci + 65 * h: 260 * ci + 65 * h + 65],
                            pt[:, 512 * ci + 128 * qc: 512 * ci + 128 * qc + 128],
                            start=(ci == 0), stop=(ci == nci - 1),
                        )

                def av_q_stage(h, qt, pt, av_q):
                    # out [128 q, 65]: stationary pt q-col block, moving vh
                    # (65th col = ones -> denominators); strips 0..qt are all
                    # complete once block bi=qt has transposed.
                    qc = qt % 4
                    for ci in range(qt + 1):
                        nc.tensor.matmul(
                            av_q[:, 0:65],
                            pt[:, 512 * ci + 128 * qc: 512 * ci + 128 * qc + 128],
                            vh[:, 260 * ci + 65 * h: 260 * ci + 65 * h + 65],
                            start=(ci == 0), stop=(ci == qt),
                        )

                def norm_q_stage(h, qt, av_q, avn2):
                    rec = wk.tile([128, 1], F32, tag="recq", bufs=3,
                                  name=f"recq_h{h}_q{qt}")
                    nc.vector.reciprocal(rec[:], av_q[:, 64:65])
                    ph = h % 2
                    nc.vector.tensor_scalar_mul(
                        avn2[:, 64 * ph: 64 * ph + 64], av_q[:, 0:64],
                        rec[:])

                def tp_q_stage(et, qt, avn2):
                    tp_ = psT.tile([128, 512], BF, tag="T",
                                   name=f"tpq_e{et}_q{qt}")
                    nc.tensor.transpose(tp_[:, 0:128], avn2[:], ident[:])
                    nc.vector.tensor_copy(
                        at[:, 2048 * et + 128 * qt: 2048 * et + 128 * qt + 128],
                        tp_[:, 0:128])

                def norm_stage(h, g, av):
                    et, ph = h // 2, h % 2
                    r0, r1 = 64 * ph, 64 * ph + 64
                    rec = wk.tile([1, 512], F32, tag="rec",
                                  name=f"rec_h{h}_g{g}")
                    nc.vector.reciprocal(rec[:], av[64:65, :])
                    if CFG["bcast"] == "pe":
                        rec_bf = wk.tile([1, 512], BF, tag="recb",
                                         name=f"recb_h{h}_g{g}")
                        nc.vector.tensor_copy(rec_bf[:], rec[:])
                        rec64 = psT.tile([64, 512], F32, tag="T",
                                         name=f"rec64_h{h}_g{g}")
                        nc.tensor.matmul(rec64[:], ones64[:], rec_bf[:],
                                         start=True, stop=True)
                        nc.vector.tensor_tensor(
                            out=at[r0:r1, 2048 * et + 512 * g:
                                   2048 * et + 512 * g + 512],
                            in0=av[0:64, :],
                            in1=rec64[:],
                            op=MULT,
                        )
                    else:
                        rec64 = wk.tile([64, 512], F32, tag="rec64",
                                        bufs=CFG["rec64_bufs"],
                                        name=f"rec64_h{h}_g{g}")
                        nc.gpsimd.partition_broadcast(rec64[:], rec[:])
                        nc.vector.tensor_tensor(
                            out=at[r0:r1, 2048 * et + 512 * g:
                                   2048 * et + 512 * g + 512],
                            in0=av[0:64, :],
                            in1=rec64[:],
                            op=MULT,
                        )

                # software-pipelined schedule over jobs (h, g, bi)
                gs = range(4) if CFG["g_order"] == "asc" else range(3, -1, -1)
                seq = [(h, g, bi) for g in gs for h in range(4)
                       for bi in range(4 * g, 4 * g + 4)]
                deferred = {}   # emit_idx -> list of callables

                def defer(idx, fn):
                    deferred.setdefault(idx, []).append(fn)

                pts = {}
                avs = {}
                avn2s = {}
                staged_tiles = {}
                n = len(seq)
                _rel_next = [0]

                def emit_rel_through(r):
                    while _rel_next[0] <= min(r, n - 1):
                        j = _rel_next[0]
                        _mark(nc, f"rel_{j}")
                        staged_tiles[seq[j]] = rel_stage(seq[j][0], seq[j][2])
                        _rel_next[0] += 1

                emit_rel_through(max(CFG["rel_lead"], CFG["rel_lead0"]) - 1)
                for idx, (h, g, bi) in enumerate(seq):
                    if (h, g) not in pts:
                        pts[(h, g)] = ptp.tile(
                            [128, 512 * (4 * g + 4)], BF, tag="pt",
                            bufs=CFG["pt_bufs"], name=f"pt_h{h}_g{g}")
                    emit_rel_through(idx + CFG["rel_lead"])
                    pn = wk.tile([128, 2048], BF, tag="pn",
                                 bufs=CFG["pn_bufs"], name=f"pn_h{h}_b{bi}")
                    pt = pts[(h, g)]
                    _mark(nc, f"blk_{idx}")
                    block_stage(h, bi, staged_tiles.pop((h, g, bi)), pn,
                                CFG["ra_dve"])
                    if CFG["pt_mode"] == "xbar":
                        xb_idx = idx + CFG["xbar_lag"]
                        defer(xb_idx, lambda bi=bi, pn=pn, pt=pt, _i=idx:
                              (_mark(nc, f"xbar_{_i}"),
                               xbar_stage(bi, pn, pt)))
                    else:
                        xb_idx = idx + CFG["tr_lag"]
                        if CFG["tr_lag"]:
                            defer(xb_idx, lambda h=h, bi=bi, pn=pn, pt=pt,
                                  _i=idx:
                                  (_mark(nc, f"tr_{_i}"),
                                   transp_stage(h, bi, pn, pt)))
                        else:
                            transp_stage(h, bi, pn, pt)
                    if CFG["av_mode"] == "reor":
                        av_idx = xb_idx + CFG["av_lag"]

                        def do_av_q(h=h, g=g, pt=pt, qt=bi):
                            _mark(nc, f"av_{h}_{g}")
                            av_q = psA.tile([128, 65], F32, tag="A",
                                            name=f"avq_h{h}_q{qt}")
                            avs[(h, qt)] = av_q
                            av_q_stage(h, qt, pt, av_q)
                            if qt % 4 == 3:
                                pts.pop((h, g), None)
                        defer(av_idx, do_av_q)
                        nmq_idx = av_idx + CFG["norm_lag"]

                        def do_norm_q(h=h, qt=bi, et=h // 2):
                            _mark(nc, f"nm_{h}_{qt}")
                            if (et, qt) not in avn2s:
                                avn2s[(et, qt)] = wk.tile(
                                    [128, 128], BF, tag="avn2", bufs=8,
                                    name=f"avn2_e{et}_q{qt}")
                            norm_q_stage(h, qt, avs.pop((h, qt)),
                                         avn2s[(et, qt)])
                        defer(nmq_idx, do_norm_q)
                        if h % 2 == 1:
                            defer(nmq_idx + 1, lambda et=h // 2, qt=bi:
                                  tp_q_stage(et, qt, avn2s.pop((et, qt))))
                        if h == 3:
                            def do_op_rt(rt=bi):
                                _mark(nc, f"op_{g}")
                                outproj_rt(rt)
                            defer(nmq_idx + 1 + CFG["op_lag"], do_op_rt)
                    elif CFG["av_split"]:
                        def do_av_part(h=h, g=g, pt=pt, bi=bi):
                            _mark(nc, f"av_{h}_{g}")
                            if (h, g) not in avs:
                                avs[(h, g)] = psA.tile(
                                    [65, 512], F32, tag="A",
                                    name=f"av_h{h}_g{g}")
                            av_col_stage(h, g, pt, avs[(h, g)], bi % 4)
                            if bi % 4 == 3:
                                pts.pop((h, g), None)
                        defer(xb_idx + CFG["av_lag"], do_av_part)
                    if bi == 4 * g + 3 and CFG["av_mode"] != "reor":
                        av_idx = xb_idx + CFG["av_lag"]
                        if not CFG["av_split"]:
                            def do_av(h=h, g=g, pt=pt):
                                _mark(nc, f"av_{h}_{g}")
                                avs[(h, g)] = psA.tile(
                                    [65, 512], F32, tag="A",
                                    name=f"av_h{h}_g{g}")
                                av_stage(h, g, pt, avs[(h, g)])
                                pts.pop((h, g), None)
                            defer(av_idx, do_av)
                        nm_idx = av_idx + CFG["norm_lag"]
                        defer(nm_idx, lambda h=h, g=g:
                              (_mark(nc, f"nm_{h}_{g}"),
                               norm_stage(h, g, avs.pop((h, g)))))
                        if h == 3:
                            def do_op(g=g):
                                _mark(nc, f"op_{g}")
                                for rt in range(4 * g, 4 * g + 4):
                                    outproj_rt(rt)
                            defer(nm_idx + CFG["op_lag"], do_op)
                    for fn in deferred.pop(idx, []):
                        fn()
                # flush remaining deferred work in index order
                for idx in sorted(deferred):
                    for fn in deferred[idx]:
                        fn()
    return nc


# ---------------- host side ----------------

def _bf16(x):
    return np.ascontiguousarray(x).astype(ml_dtypes.bfloat16)


def _col2d(vec256):
    """[256] f32 -> [128, 2] with v2d[p, a] = vec[128a + p]."""
    return np.ascontiguousarray(
        np.asarray(vec256, np.float32).reshape(2, 128).T)


def core_inputs(q_b, k_b, v_b, pos_enc, Wq, bq, Wk, bk, Wv, bv, Wo,
                r_w_bias, r_r_bias, r_kernel, g):
    sl = slice(256 * g, 256 * g + 256)
    rk_cat = np.concatenate([r_kernel[4 * g + i] for i in range(4)], axis=1)
    return {
        "q_in": _bf16(q_b.T),
        "k_in": _bf16(k_b.T),
        "v_in": _bf16(v_b.T),
        "pe_in": _bf16(pos_enc[1:2049].T),
        "wq": _bf16(Wq[:, sl]),
        "wk": _bf16(Wk[:, sl]),
        "wv": _bf16(Wv[:, sl]),
        "rk": _bf16(rk_cat),
        "wo": _bf16(Wo[sl, :]),
        "b1": _col2d(0.125 * (bq[sl] + r_w_bias[4 * g:4 * g + 4].reshape(256))),
        "b2": _col2d(0.125 * (bq[sl] + r_r_bias[4 * g:4 * g + 4].reshape(256))),
        "bk": _col2d(bk[sl]),
        "bv": _col2d(bv[sl]),
    }


_SHAPES = {
    "q_in": ([1024, 2048], BF), "k_in": ([1024, 2048], BF),
    "v_in": ([1024, 2048], BF), "pe_in": ([1024, 2048], BF),
    "wq": ([1024, 256], BF), "wk": ([1024, 256], BF), "wv": ([1024, 256], BF),
    "rk": ([1024, 256], BF), "wo": ([256, 1024], BF),
    "b1": ([128, 2], F32), "b2": ([128, 2], F32),
    "bk": ([128, 2], F32), "bv": ([128, 2], F32),
}

_NC_CACHE = {}


def _build():
    key = tuple(sorted((k, str(v)) for k, v in CFG.items()))
    if key in _NC_CACHE:
        return _NC_CACHE[key]
    MARKS.clear()
    nc = bacc.Bacc("TRN2", target_bir_lowering=False, debug=False,
                   enable_asserts=False)
    ins = {name: nc.dram_tensor(name, shape, dt, kind="ExternalInput").ap()
           for name, (shape, dt) in _SHAPES.items()}
    out = nc.dram_tensor("out", [2048, 1024], BF, kind="ExternalOutput").ap()
    emit_core(nc, ins, out)
    nc.compile()
    nc.m = get_hw_module(nc.m)
    _NC_CACHE[key] = nc
    return nc


def kernel(**inputs):
    global LAST_RESULTS
    inp = {k: np.asarray(v) for k, v in inputs.items()}
    nc = _build()
    in_maps = []
    for c in range(8):
        b, g = c // 4, c % 4
        in_maps.append(core_inputs(
            inp["q"][b], inp["k"][b], inp["v"][b], inp["pos_enc"],
            inp["Wq"], inp["bq"], inp["Wk"], inp["bk"], inp["Wv"], inp["bv"],
            inp["Wo"], inp["r_w_bias"], inp["r_r_bias"], inp["r_kernel"], g))
    res = bass_utils.run_bass_kernel_spmd(
        nc, in_maps, core_ids=list(range(8)), trace=PROFILE)
    LAST_RESULTS = res
    out = np.zeros((2, 2048, 1024), np.float32)
    for c in range(8):
        b = c // 4
        out[b] += np.asarray(res.results[c]["out"]).astype(np.float32)
    out += np.asarray(inp["bo"], np.float32)[None, None, :]
    return out


# revision 33
# speedup vs baseline: 1.8083x; 1.0056x over previous
"""Transformer-XL relative attention (B=2, L=2048, D=1024, H=16) on 8 TRN2
NeuronCores.

Sharding: data-parallel over batch x tensor-parallel over heads.  Core
c = 4*b + g handles batch b, head group g (4 heads).  Wq/Wk/Wv are
column-sharded, Wo row-sharded; each core emits a partial [2048,1024]
output (bf16) which the host sums per batch in f32 (+bo).

Per-core layouts (bf16 in SBUF, head pair = h//2, row = 64*(h%2)+e):
  q1t/q2t/kt : [128, 2*2048]  e-tile h//2 at cols [2048*(h//2)], free = seq
  peht       : [128, 2*2048]  relative-position keys pe[1:2049] @ r_kernel
  vh         : [128, 16*260]  per key-tile: 4 heads' V (64 cols each) + a
                              ones column per head (free row-sum trick)
  at (A^T)   : [128, 2*2048]  normalized attention output, transposed

Rel-shift: for query block bi (rows ri..ri+127) R_s[rr, x] holds
Q2 . peh[xmin + x] (xmin = 1920 - ri); the score chunk at columns
[c0, c0+wc) needs staged[rr, cc] = R_s[rr, 127 - rr + c0 + cc] -- an
anti-diagonal flat access pattern (offset 127+c0, ap=[[PITCH-1,128],[1,wc]])
which only DMA engines can execute (SBUF->SBUF).  R_s columns beyond the
causal edge are padded with -1e9; the anti-diagonal read maps exactly the
strict upper triangle onto that pad, so exp() yields the causal zeros with
no separate masking pass.

Projections run kc-outer with all 8 (et, rc) accumulation groups resident
in PSUM so input-chunk DMAs are consumed as they land.  SBUF pools are
phase-scoped: the input/weight pools are released after the projections,
freeing ~64 KB/partition for deep phase-B pipeline buffers.

The rel pipeline is software-pipelined one query-block ahead, and the
P^T transposes (PE matmuls, or batched DMA-crossbar in pt_mode=xbar),
AV, normalize and output projection are emitted with configurable lags
so their dependencies are complete when each in-order engine queue
reaches them.
"""

import numpy as np
import ml_dtypes
import concourse.bass as bass
import concourse.mybir as mybir
import concourse.tile as tile
from concourse import bacc
from concourse import bass_utils
from concourse.bass_interp import get_hw_module
from concourse.masks import make_identity

BF = mybir.dt.bfloat16
F32 = mybir.dt.float32
EXP = mybir.ActivationFunctionType.Exp
IDENT = mybir.ActivationFunctionType.Identity
MULT = mybir.AluOpType.mult
ADD = mybir.AluOpType.add

L = 2048          # sequence length
RS_W = 2176       # R_s tile width (2048 + 128 pad); anti-diag pitch = RS_W-1

PROFILE = False       # set by test harness to capture a trace
LAST_RESULTS = None   # BassKernelResults of the last run (for profiling)
MARKS = []            # (instruction id watermark, stage label) per build


def _mark(nc, label):
    MARKS.append((nc.next_id(), label))

# tuning knobs (read at build time)
CFG = {
    "diag_eng": "sync",    # engine issuing anti-diagonal staging DMAs
    "load_eng": "sync",    # engine issuing input/weight loads
    "rs_bufs": 6,
    "stg_bufs": 7,
    "pn_bufs": 6,
    "pt_bufs": 2,
    "psS_bufs": 2,
    "psR_bufs": 2,
    "psA_bufs": 2,
    "osb_bufs": 2,
    "rec64_bufs": 2,
    "r_evict": "dve2",    # act | dve | mix | dve2
    "qkv_evict": "act",   # projection eviction engine: act | dve
    "io_bufs": 3,
    "in_split": 4,        # input load DMAs per projection half
    "proj_order": "kpvq",
    "ra_dve": 0,          # rel-add chunks moved to DVE per block (0..4)
    "ra_pool": 0,         # rel-add chunks moved to Pool per block (0..4)
    "pad_eng": "vector",  # pad/ones memset engine: gpsimd | vector
    "pt_mode": "pe",      # pe (transpose matmuls) | xbar (batched DMA)
    "psT_bufs": 2,        # PSUM pool for PE transposes (pt_mode=pe)
    "pt_evict": "dve",    # psT -> pt eviction engine: act | dve | mix
    "tr_lag": 1,          # jobs between block chunks and its PE transposes
    "xbar_lag": 2,        # jobs between block_stage and its crossbar DMA
    "av_mode": "reor",    # reor (query-partition AV) | col (at-orientation)
    "av_split": 1,        # 1: emit AV strips right after each block's tr
    "rel_lead": 3,        # how many jobs ahead the rel stage runs
    "rel_lead0": 2,       # rel lead for the first 8 jobs
    "g_order": "asc",    # group processing order: asc | desc
    "wk_split": 1,        # first weight load chunk count
    "av_lag": 3,          # jobs between last xbar of (h,g) and its AV
    "norm_lag": 1,        # jobs between AV and recip/bcast/normalize
    "op_lag": 1,          # jobs between norm(h=3) and outproj
    "bcast": "gpsimd",    # denominator broadcast: gpsimd | pe
    "store_eng": "sync",  # output store queue: sync | scalar
}


def emit_core(nc, ins, out):
    """ins: dict name->AP (DRAM), out: AP (DRAM [2048,1024] bf16)."""
    with tile.TileContext(nc) as tc:
        deng = getattr(nc, CFG["diag_eng"])
        leng = getattr(nc, CFG["load_eng"])
        peng = getattr(nc, CFG["pad_eng"] if CFG["pad_eng"] != "gpsimd"
                       else "gpsimd")
        with (
            tc.tile_pool(name="per", bufs=1) as per,
            tc.tile_pool(name="psS", bufs=CFG["psS_bufs"], space="PSUM") as psS,
            tc.tile_pool(name="psR", bufs=CFG["psR_bufs"], space="PSUM") as psR,
            tc.tile_pool(name="psA", bufs=CFG["psA_bufs"], space="PSUM") as psA,
            tc.tile_pool(name="psT", bufs=CFG["psT_bufs"], space="PSUM") as psT,
        ):
            # ---------------- persistent tiles ----------------
            q1t = per.tile([128, 2 * L], BF, tag="q1t")
            q2t = per.tile([128, 2 * L], BF, tag="q2t")
            kt = per.tile([128, 2 * L], BF, tag="kt")
            peht = per.tile([128, 2 * L], BF, tag="peht")
            at = per.tile([128, 2 * L], BF, tag="at")
            vh = per.tile([128, 16 * 260], BF, tag="vh")
            wo_s = per.tile([128, 2048], BF, tag="wo")
            b1s = per.tile([128, 2], F32, tag="b1s")
            b2s = per.tile([128, 2], F32, tag="b2s")
            bks = per.tile([128, 2], F32, tag="bks")
            bvs = per.tile([128, 2], F32, tag="bvs")

            ident = per.tile([128, 128], BF, tag="ident")
            make_identity(nc, ident[:])
            ones64 = per.tile([1, 64], BF, tag="ones64")
            nc.vector.memset(ones64[:], 1.0)
            # bias loads go through the Activation HWDGE queue so they don't
            # head-of-line block the SP queue ahead of the weight/input loads
            nc.scalar.dma_start(b1s[:], ins["b1"])
            nc.scalar.dma_start(b2s[:], ins["b2"])
            nc.scalar.dma_start(bks[:], ins["bk"])
            nc.scalar.dma_start(bvs[:], ins["bv"])

            def _pevict(dst, ps, bias, scale):
                if CFG["qkv_evict"] == "dve":
                    nc.vector.tensor_scalar(
                        out=dst, in0=ps[:], scalar1=scale, scalar2=bias,
                        op0=MULT, op1=ADD)
                else:
                    nc.scalar.activation(dst, ps[:], IDENT, bias=bias,
                                         scale=scale)

            def evict_q(et, rc, ps):
                cs = 2048 * et + 512 * rc
                if CFG["qkv_evict"] == "qdve":
                    nc.vector.tensor_scalar(
                        out=q1t[:, cs:cs + 512], in0=ps[:], scalar1=0.125,
                        scalar2=b1s[:, et:et + 1], op0=MULT, op1=ADD)
                    nc.vector.tensor_scalar(
                        out=q2t[:, cs:cs + 512], in0=ps[:], scalar1=0.125,
                        scalar2=b2s[:, et:et + 1], op0=MULT, op1=ADD)
                else:
                    _pevict(q1t[:, cs:cs + 512], ps, b1s[:, et:et + 1], 0.125)
                    _pevict(q2t[:, cs:cs + 512], ps, b2s[:, et:et + 1], 0.125)

            def evict_k(et, rc, ps):
                cs = 2048 * et + 512 * rc
                _pevict(kt[:, cs:cs + 512], ps, bks[:, et:et + 1], 1.0)

            def evict_pe(et, rc, ps):
                cs = 2048 * et + 512 * rc
                nc.vector.tensor_copy(peht[:, cs:cs + 512], ps[:])

            # ---------------- phase A: projections (scoped pools) ---------
            # kc-outer: all 8 (et, rc) accumulation groups live in PSUM
            # simultaneously (borrowing every PSUM pool), so each input
            # chunk DMA is consumed as soon as it lands.
            _proj_pools = [psS, psS, psR, psR, psR, psT, psT, psA]
            _proj_tags = ["S", "S", "R", "R", "R", "T", "T", "A"]

            with (
                tc.tile_pool(name="wgt", bufs=1) as wp,
                tc.tile_pool(name="io", bufs=CFG["io_bufs"]) as iop,
            ):
                vht = iop.tile([128, 2 * L], BF, tag="vht", bufs=1)

                def evict_v(et, rc, ps):
                    cs = 2048 * et + 512 * rc
                    _pevict(vht[:, cs:cs + 512], ps, bvs[:, et:et + 1], 1.0)

                def load_w(name, split=1):
                    t = wp.tile([128, 2048], BF, tag=name, name=f"w_{name}")
                    src = ins[name].rearrange("(a p) e -> p a e", p=128)
                    blk = 8 // split
                    for si in range(split):
                        leng.dma_start(
                            t[:, 256 * blk * si: 256 * blk * (si + 1)],
                            src[:, blk * si: blk * (si + 1), :])
                    return t

                _first_w = ["wq", "wk", "wv", "rk"][
                    "qkvp".index(CFG["proj_order"][0])]

                def project(xname, wname, evict):
                    w_s = load_w(wname, CFG["wk_split"]
                                 if wname == _first_w else 1)
                    src = ins[xname].rearrange("(a p) n -> p a n", p=128)
                    xts = []
                    for half in range(2):
                        xt = iop.tile([128, 4 * L], BF, tag="inT",
                                      name=f"in_{xname}_{half}")
                        ns = CFG["in_split"]
                        blk = 4 // ns
                        for s in range(ns):
                            leng.dma_start(
                                xt[:, 2048 * blk * s: 2048 * blk * (s + 1)],
                                src[:, 4 * half + blk * s:
                                    4 * half + blk * (s + 1), :],
                            )
                        xts.append(xt)
                    pss = {}
                    for i, (et, rc) in enumerate(
                            [(e, r) for e in range(2) for r in range(4)]):
                        pss[(et, rc)] = _proj_pools[i].tile(
                            [128, 512], F32, tag=_proj_tags[i],
                            name=f"ps_{xname}_{et}_{rc}")
                    for kc in range(8):
                        xt = xts[kc // 4]
                        kcc = kc % 4
                        for et in range(2):
                            for rc in range(4):
                                nc.tensor.matmul(
                                    pss[(et, rc)][:],
                                    w_s[:, 256 * kc + 128 * et:
                                        256 * kc + 128 * et + 128],
                                    xt[:, 2048 * kcc + 512 * rc:
                                       2048 * kcc + 512 * rc + 512],
                                    start=(kc == 0),
                                    stop=(kc == 7),
                                )
                    for et in range(2):
                        for rc in range(4):
                            evict(et, rc, pss[(et, rc)])

                projs = {"q": ("q_in", "wq", evict_q),
                         "k": ("k_in", "wk", evict_k),
                         "v": ("v_in", "wv", evict_v),
                         "p": ("pe_in", "rk", evict_pe)}
                for c in CFG["proj_order"]:
                    _mark(nc, f"proj_{c}")
                    project(*projs[c])
                leng.dma_start(
                    wo_s[:], ins["wo"].rearrange("(a p) e -> p a e", p=128))

                _mark(nc, "vh_asm")
                # vh assembly: PE-transpose one [128,128] tile per (ct,
                # pair), evict the two heads' column halves into vh blocks.
                for ct in range(16):
                    for p in range(2):
                        scr = psT.tile([128, 512], BF, tag="T",
                                       name=f"scr_{ct}_{p}")
                        nc.tensor.transpose(
                            scr[:, 0:128],
                            vht[:, 2048 * p + 128 * ct:
                                2048 * p + 128 * ct + 128],
                            ident[:],
                        )
                        for hh in range(2):
                            h4 = 2 * p + hh
                            nc.vector.tensor_copy(
                                vh[:, 260 * ct + 65 * h4:
                                   260 * ct + 65 * h4 + 64],
                                scr[:, 64 * hh: 64 * hh + 64],
                            )
                ones_cols = vh[:].rearrange("p (ct c) -> p ct c", c=260)
                for h in range(4):
                    peng.memset(ones_cols[:, :, 65 * h + 64], 1.0)

            # ---------------- phase B: attention (scoped pools) -----------
            with (
                tc.tile_pool(name="work", bufs=3) as wk,
                tc.tile_pool(name="pt", bufs=CFG["pt_bufs"]) as ptp,
            ):
                def outproj_rt(rt):
                    osb = wk.tile([128, 1024], BF, tag="osb",
                                  bufs=CFG["osb_bufs"], name=f"osb_{rt}")
                    for n in range(2):
                        op_ = psS.tile([128, 512], F32, tag="S",
                                       name=f"op_{rt}_{n}")
                        for hc in range(2):
                            nc.tensor.matmul(
                                op_[:],
                                at[:, 2048 * hc + 128 * rt:
                                   2048 * hc + 128 * rt + 128],
                                wo_s[:, 1024 * hc + 512 * n:
                                     1024 * hc + 512 * n + 512],
                                start=(hc == 0), stop=(hc == 1),
                            )
                        if n == 0:
                            nc.scalar.copy(osb[:, 0:512], op_[:])
                        else:
                            nc.vector.tensor_copy(osb[:, 512:1024], op_[:])
                    getattr(nc, CFG["store_eng"]).dma_start(
                        out[128 * rt: 128 * rt + 128, :], osb[:])

                # rel-score stage for one (h, bi): matmuls -> rs evict ->
                # pad memset -> anti-diagonal staging DMA.
                def rel_stage(h, bi):
                    et, ph = h // 2, h % 2
                    r0, r1 = 64 * ph, 64 * ph + 64
                    ri = 128 * bi
                    Wb = ri + 128
                    nch = (Wb + 511) // 512
                    xmin = 1920 - ri
                    rs = wk.tile([128, RS_W], BF, tag="rs",
                                 bufs=CFG["rs_bufs"], name=f"rs_h{h}_b{bi}")
                    for jc in range(nch):
                        w = min(512, Wb - 512 * jc)
                        rp = psR.tile([128, 512], F32, tag="R",
                                      name=f"rp_h{h}_b{bi}_{jc}")
                        nc.tensor.matmul(
                            rp[:, :w],
                            q2t[r0:r1, 2048 * et + ri: 2048 * et + ri + 128],
                            peht[r0:r1, 2048 * et + xmin + 512 * jc:
                                 2048 * et + xmin + 512 * jc + w],
                            start=True, stop=True,
                        )
                        use_act = (CFG["r_evict"] == "act" or
                                   (CFG["r_evict"] == "mix" and jc % 2 == 0) or
                                   (CFG["r_evict"] == "dve2" and jc % 3 == 0))
                        if use_act:
                            nc.scalar.copy(rs[:, 512 * jc: 512 * jc + w],
                                           rp[:, :w])
                        else:
                            nc.vector.tensor_copy(
                                rs[:, 512 * jc: 512 * jc + w], rp[:, :w])
                    peng.memset(rs[:, Wb:Wb + 128], -1e9)
                    staged = wk.tile([128, 2048], BF, tag="stg",
                                     bufs=CFG["stg_bufs"],
                                     name=f"stg_h{h}_b{bi}")
                    diag = bass.AP(
                        tensor=rs.tensor,
                        offset=rs.offset + 127,
                        ap=[[RS_W - 1, 128], [1, Wb]],
                    )
                    deng.dma_start(staged[:, :Wb], diag)
                    return staged

                # content/ident/exp chunks for one (h, bi)
                def block_stage(h, bi, staged, pn, nrel_dve):
                    et, ph = h // 2, h % 2
                    r0, r1 = 64 * ph, 64 * ph + 64
                    ri = 128 * bi
                    Wb = ri + 128
                    nch = (Wb + 511) // 512
                    for ci_chunk in range(nch):
                        c0 = 512 * ci_chunk
                        wc = min(512, Wb - c0)
                        use_dve = ci_chunk < nrel_dve
                        use_pool = (not use_dve and
                                    ci_chunk < nrel_dve + CFG["ra_pool"])
                        sp = psS.tile([128, 512], F32, tag="S",
                                      name=f"sp_h{h}_b{bi}_{ci_chunk}")
                        nc.tensor.matmul(
                            sp[:, :wc],
                            q1t[r0:r1, 2048 * et + ri: 2048 * et + ri + 128],
                            kt[r0:r1, 2048 * et + c0: 2048 * et + c0 + wc],
                            start=True, stop=use_dve or use_pool,
                        )
                        if use_dve or use_pool:
                            eng = nc.vector if use_dve else nc.gpsimd
                            eng.scalar_tensor_tensor(
                                out=sp[:, :wc], in0=sp[:, :wc], scalar=1.0,
                                in1=staged[:, c0:c0 + wc],
                                op0=MULT, op1=ADD,
                            )
                        else:
                            nc.tensor.matmul(
                                sp[:, :wc],
                                ident[:],
                                staged[:, c0:c0 + wc],
                                start=False, stop=True,
                            )
                        nc.scalar.activation(pn[:, c0:c0 + wc],
                                             sp[:, :wc], EXP)

                # PE transposes for one (h, bi): emitted with a lag so the
                # exps they wait on are complete.
                def transp_stage(h, bi, pn, pt):
                    Wb = 128 * bi + 128
                    nch = (Wb + 511) // 512
                    for ci_chunk in range(nch):
                        c0 = 512 * ci_chunk
                        wc = min(512, Wb - c0)
                        tp_ = psT.tile([128, 512], BF, tag="T",
                                       name=f"tp_h{h}_b{bi}_{ci_chunk}")
                        for s in range(wc // 128):
                            nc.tensor.transpose(
                                tp_[:, 128 * s: 128 * s + 128],
                                pn[:, c0 + 128 * s: c0 + 128 * s + 128],
                                ident[:],
                            )
                        dst = bass.AP(
                            tensor=pt.tensor,
                            offset=pt.offset + 512 * (c0 // 128)
                            + 128 * (bi % 4),
                            ap=[[pt.tensor.shape[-1], 128],
                                [512, wc // 128], [1, 128]],
                        )
                        use_act = (CFG["pt_evict"] == "act" or
                                   (CFG["pt_evict"] == "mix"
                                    and (bi + ci_chunk) % 2 == 0))
                        if use_act:
                            nc.scalar.copy(dst, tp_[:, :wc])
                        else:
                            nc.vector.tensor_copy(dst, tp_[:, :wc])

                def xbar_stage(bi, pn, pt):
                    # batched crossbar transpose: pn [128, Wb] -> pt strips
                    Wb = 128 * bi + 128
                    dst = bass.AP(
                        tensor=pt.tensor,
                        offset=pt.offset + 128 * (bi % 4),
                        ap=[[pt.tensor.shape[-1], 128], [512, Wb // 128],
                            [1, 128]],
                    )
                    nc.sync.dma_start_transpose(dst, pn[:, :Wb])

                def av_stage(h, g, pt, av, ci_lo=0, ci_hi=None):
                    if ci_hi is None:
                        ci_hi = 4 * g + 4
                    for ci in range(ci_lo, ci_hi):
                        o = max(0, 128 * ci - 512 * g)
                        nc.tensor.matmul(
                            av[:, o:512],
                            vh[:, 260 * ci + 65 * h: 260 * ci + 65 * h + 65],
                            pt[:, 512 * ci + o: 512 * ci + 512],
                            start=(ci == 0), stop=(ci == 4 * g + 3),
                        )

                def av_col_stage(h, g, pt, av, qc):
                    # accumulate strips 0..4g+qc into psA columns
                    # [128qc, 128qc+128) -- ready right after block 4g+qc's
                    # transposes (the last writer of those pt columns)
                    nci = 4 * g + qc + 1
                    for ci in range(nci):
                        nc.tensor.matmul(
                            av[:, 128 * qc: 128 * qc + 128],
                            vh[:, 260 * ci + 65 * h: 260 * ci + 65 * h + 65],
                            pt[:, 512 * ci + 128 * qc: 512 * ci + 128 * qc + 128],
                            start=(ci == 0), stop=(ci == nci - 1),
                        )

                def av_q_stage(h, qt, pt, av_q):
                    # out [128 q, 65]: stationary pt q-col block, moving vh
                    # (65th col = ones -> denominators); strips 0..qt are all
                    # complete once block bi=qt has transposed.
                    qc = qt % 4
                    for ci in range(qt + 1):
                        nc.tensor.matmul(
                            av_q[:, 0:65],
                            pt[:, 512 * ci + 128 * qc: 512 * ci + 128 * qc + 128],
                            vh[:, 260 * ci + 65 * h: 260 * ci + 65 * h + 65],
                            start=(ci == 0), stop=(ci == qt),
                        )

                def norm_q_stage(h, qt, av_q, avn2):
                    rec = wk.tile([128, 1], F32, tag="recq", bufs=3,
                                  name=f"recq_h{h}_q{qt}")
                    nc.vector.reciprocal(rec[:], av_q[:, 64:65])
                    ph = h % 2
                    nc.vector.tensor_scalar_mul(
                        avn2[:, 64 * ph: 64 * ph + 64], av_q[:, 0:64],
                        rec[:])

                def tp_q_stage(et, qt, avn2):
                    tp_ = psT.tile([128, 512], BF, tag="T",
                                   name=f"tpq_e{et}_q{qt}")
                    nc.tensor.transpose(tp_[:, 0:128], avn2[:], ident[:])
                    nc.vector.tensor_copy(
                        at[:, 2048 * et + 128 * qt: 2048 * et + 128 * qt + 128],
                        tp_[:, 0:128])

                def norm_stage(h, g, av):
                    et, ph = h // 2, h % 2
                    r0, r1 = 64 * ph, 64 * ph + 64
                    rec = wk.tile([1, 512], F32, tag="rec",
                                  name=f"rec_h{h}_g{g}")
                    nc.vector.reciprocal(rec[:], av[64:65, :])
                    if CFG["bcast"] == "pe":
                        rec_bf = wk.tile([1, 512], BF, tag="recb",
                                         name=f"recb_h{h}_g{g}")
                        nc.vector.tensor_copy(rec_bf[:], rec[:])
                        rec64 = psT.tile([64, 512], F32, tag="T",
                                         name=f"rec64_h{h}_g{g}")
                        nc.tensor.matmul(rec64[:], ones64[:], rec_bf[:],
                                         start=True, stop=True)
                        nc.vector.tensor_tensor(
                            out=at[r0:r1, 2048 * et + 512 * g:
                                   2048 * et + 512 * g + 512],
                            in0=av[0:64, :],
                            in1=rec64[:],
                            op=MULT,
                        )
                    else:
                        rec64 = wk.tile([64, 512], F32, tag="rec64",
                                        bufs=CFG["rec64_bufs"],
                                        name=f"rec64_h{h}_g{g}")
                        nc.gpsimd.partition_broadcast(rec64[:], rec[:])
                        nc.vector.tensor_tensor(
                            out=at[r0:r1, 2048 * et + 512 * g:
                                   2048 * et + 512 * g + 512],
                            in0=av[0:64, :],
                            in1=rec64[:],
                            op=MULT,
                        )

                # software-pipelined schedule over jobs (h, g, bi)
                gs = range(4) if CFG["g_order"] == "asc" else range(3, -1, -1)
                seq = [(h, g, bi) for g in gs for h in range(4)
                       for bi in range(4 * g, 4 * g + 4)]
                deferred = {}   # emit_idx -> list of callables

                def defer(idx, fn):
                    deferred.setdefault(idx, []).append(fn)

                pts = {}
                avs = {}
                avn2s = {}
                staged_tiles = {}
                n = len(seq)
                _rel_next = [0]

                def emit_rel_through(r):
                    while _rel_next[0] <= min(r, n - 1):
                        j = _rel_next[0]
                        _mark(nc, f"rel_{j}")
                        staged_tiles[seq[j]] = rel_stage(seq[j][0], seq[j][2])
                        _rel_next[0] += 1

                emit_rel_through(max(CFG["rel_lead"], CFG["rel_lead0"]) - 1)
                for idx, (h, g, bi) in enumerate(seq):
                    if (h, g) not in pts:
                        pts[(h, g)] = ptp.tile(
                            [128, 512 * (4 * g + 4)], BF, tag="pt",
                            bufs=CFG["pt_bufs"], name=f"pt_h{h}_g{g}")
                    emit_rel_through(idx + CFG["rel_lead"])
                    pn = wk.tile([128, 2048], BF, tag="pn",
                                 bufs=CFG["pn_bufs"], name=f"pn_h{h}_b{bi}")
                    pt = pts[(h, g)]
                    _mark(nc, f"blk_{idx}")
                    block_stage(h, bi, staged_tiles.pop((h, g, bi)), pn,
                                CFG["ra_dve"])
                    if CFG["pt_mode"] == "xbar":
                        xb_idx = idx + CFG["xbar_lag"]
                        defer(xb_idx, lambda bi=bi, pn=pn, pt=pt, _i=idx:
                              (_mark(nc, f"xbar_{_i}"),
                               xbar_stage(bi, pn, pt)))
                    else:
                        xb_idx = idx + CFG["tr_lag"]
                        if CFG["tr_lag"]:
                            defer(xb_idx, lambda h=h, bi=bi, pn=pn, pt=pt,
                                  _i=idx:
                                  (_mark(nc, f"tr_{_i}"),
                                   transp_stage(h, bi, pn, pt)))
                        else:
                            transp_stage(h, bi, pn, pt)
                    if CFG["av_mode"] == "reor":
                        av_idx = xb_idx + CFG["av_lag"]

                        def do_av_q(h=h, g=g, pt=pt, qt=bi):
                            _mark(nc, f"av_{h}_{g}")
                            av_q = psA.tile([128, 65], F32, tag="A",
                                            name=f"avq_h{h}_q{qt}")
                            avs[(h, qt)] = av_q
                            av_q_stage(h, qt, pt, av_q)
                            if qt % 4 == 3:
                                pts.pop((h, g), None)
                        defer(av_idx, do_av_q)
                        nmq_idx = av_idx + CFG["norm_lag"]

                        def do_norm_q(h=h, qt=bi, et=h // 2):
                            _mark(nc, f"nm_{h}_{qt}")
                            if (et, qt) not in avn2s:
                                avn2s[(et, qt)] = wk.tile(
                                    [128, 128], BF, tag="avn2", bufs=8,
                                    name=f"avn2_e{et}_q{qt}")
                            norm_q_stage(h, qt, avs.pop((h, qt)),
                                         avn2s[(et, qt)])
                        defer(nmq_idx, do_norm_q)
                        if h % 2 == 1:
                            defer(nmq_idx + 1, lambda et=h // 2, qt=bi:
                                  tp_q_stage(et, qt, avn2s.pop((et, qt))))
                        if h == 3:
                            def do_op_rt(rt=bi):
                                _mark(nc, f"op_{g}")
                                outproj_rt(rt)
                            defer(nmq_idx + 1 + CFG["op_lag"], do_op_rt)
                    elif CFG["av_split"]:
                        def do_av_part(h=h, g=g, pt=pt, bi=bi):
                            _mark(nc, f"av_{h}_{g}")
                            if (h, g) not in avs:
                                avs[(h, g)] = psA.tile(
                                    [65, 512], F32, tag="A",
                                    name=f"av_h{h}_g{g}")
                            av_col_stage(h, g, pt, avs[(h, g)], bi % 4)
                            if bi % 4 == 3:
                                pts.pop((h, g), None)
                        defer(xb_idx + CFG["av_lag"], do_av_part)
                    if bi == 4 * g + 3 and CFG["av_mode"] != "reor":
                        av_idx = xb_idx + CFG["av_lag"]
                        if not CFG["av_split"]:
                            def do_av(h=h, g=g, pt=pt):
                                _mark(nc, f"av_{h}_{g}")
                                avs[(h, g)] = psA.tile(
                                    [65, 512], F32, tag="A",
                                    name=f"av_h{h}_g{g}")
                                av_stage(h, g, pt, avs[(h, g)])
                                pts.pop((h, g), None)
                            defer(av_idx, do_av)
                        nm_idx = av_idx + CFG["norm_lag"]
                        defer(nm_idx, lambda h=h, g=g:
                              (_mark(nc, f"nm_{h}_{g}"),
                               norm_stage(h, g, avs.pop((h, g)))))
                        if h == 3:
                            def do_op(g=g):
                                _mark(nc, f"op_{g}")
                                for rt in range(4 * g, 4 * g + 4):
                                    outproj_rt(rt)
                            defer(nm_idx + CFG["op_lag"], do_op)
                    for fn in deferred.pop(idx, []):
                        fn()
                # flush remaining deferred work in index order
                for idx in sorted(deferred):
                    for fn in deferred[idx]:
                        fn()
    return nc


# ---------------- host side ----------------

def _bf16(x):
    return np.ascontiguousarray(x).astype(ml_dtypes.bfloat16)


def _col2d(vec256):
    """[256] f32 -> [128, 2] with v2d[p, a] = vec[128a + p]."""
    return np.ascontiguousarray(
        np.asarray(vec256, np.float32).reshape(2, 128).T)


def core_inputs(q_b, k_b, v_b, pos_enc, Wq, bq, Wk, bk, Wv, bv, Wo,
                r_w_bias, r_r_bias, r_kernel, g):
    sl = slice(256 * g, 256 * g + 256)
    rk_cat = np.concatenate([r_kernel[4 * g + i] for i in range(4)], axis=1)
    return {
        "q_in": _bf16(q_b.T),
        "k_in": _bf16(k_b.T),
        "v_in": _bf16(v_b.T),
        "pe_in": _bf16(pos_enc[1:2049].T),
        "wq": _bf16(Wq[:, sl]),
        "wk": _bf16(Wk[:, sl]),
        "wv": _bf16(Wv[:, sl]),
        "rk": _bf16(rk_cat),
        "wo": _bf16(Wo[sl, :]),
        "b1": _col2d(0.125 * (bq[sl] + r_w_bias[4 * g:4 * g + 4].reshape(256))),
        "b2": _col2d(0.125 * (bq[sl] + r_r_bias[4 * g:4 * g + 4].reshape(256))),
        "bk": _col2d(bk[sl]),
        "bv": _col2d(bv[sl]),
    }


_SHAPES = {
    "q_in": ([1024, 2048], BF), "k_in": ([1024, 2048], BF),
    "v_in": ([1024, 2048], BF), "pe_in": ([1024, 2048], BF),
    "wq": ([1024, 256], BF), "wk": ([1024, 256], BF), "wv": ([1024, 256], BF),
    "rk": ([1024, 256], BF), "wo": ([256, 1024], BF),
    "b1": ([128, 2], F32), "b2": ([128, 2], F32),
    "bk": ([128, 2], F32), "bv": ([128, 2], F32),
}

_NC_CACHE = {}


def _build():
    key = tuple(sorted((k, str(v)) for k, v in CFG.items()))
    if key in _NC_CACHE:
        return _NC_CACHE[key]
    MARKS.clear()
    nc = bacc.Bacc("TRN2", target_bir_lowering=False, debug=False,
                   enable_asserts=False)
    ins = {name: nc.dram_tensor(name, shape, dt, kind="ExternalInput").ap()
           for name, (shape, dt) in _SHAPES.items()}
    out = nc.dram_tensor("out", [2048, 1024], BF, kind="ExternalOutput").ap()
    emit_core(nc, ins, out)
    nc.compile()
    nc.m = get_hw_module(nc.m)
    _NC_CACHE[key] = nc
    return nc


def kernel(**inputs):
    global LAST_RESULTS
    inp = {k: np.asarray(v) for k, v in inputs.items()}
    nc = _build()
    in_maps = []
    for c in range(8):
        b, g = c // 4, c % 4
        in_maps.append(core_inputs(
            inp["q"][b], inp["k"][b], inp["v"][b], inp["pos_enc"],
            inp["Wq"], inp["bq"], inp["Wk"], inp["bk"], inp["Wv"], inp["bv"],
            inp["Wo"], inp["r_w_bias"], inp["r_r_bias"], inp["r_kernel"], g))
    res = bass_utils.run_bass_kernel_spmd(
        nc, in_maps, core_ids=list(range(8)), trace=PROFILE)
    LAST_RESULTS = res
    out = np.zeros((2, 2048, 1024), np.float32)
    for c in range(8):
        b = c // 4
        out[b] += np.asarray(res.results[c]["out"]).astype(np.float32)
    out += np.asarray(inp["bo"], np.float32)[None, None, :]
    return out


# revision 34
# speedup vs baseline: 1.8115x; 1.0018x over previous
"""Transformer-XL relative attention (B=2, L=2048, D=1024, H=16) on 8 TRN2
NeuronCores.

Sharding: data-parallel over batch x tensor-parallel over heads.  Core
c = 4*b + g handles batch b, head group g (4 heads).  Wq/Wk/Wv are
column-sharded, Wo row-sharded; each core emits a partial [2048,1024]
output (bf16) which the host sums per batch in f32 (+bo).

Per-core layouts (bf16 in SBUF, head pair = h//2, row = 64*(h%2)+e):
  q1t/q2t/kt : [128, 2*2048]  e-tile h//2 at cols [2048*(h//2)], free = seq
  peht       : [128, 2*2048]  relative-position keys pe[1:2049] @ r_kernel
  vh         : [128, 16*260]  per key-tile: 4 heads' V (64 cols each) + a
                              ones column per head (free row-sum trick)
  at (A^T)   : [128, 2*2048]  normalized attention output, transposed

Rel-shift: for query block bi (rows ri..ri+127) R_s[rr, x] holds
Q2 . peh[xmin + x] (xmin = 1920 - ri); the score chunk at columns
[c0, c0+wc) needs staged[rr, cc] = R_s[rr, 127 - rr + c0 + cc] -- an
anti-diagonal flat access pattern (offset 127+c0, ap=[[PITCH-1,128],[1,wc]])
which only DMA engines can execute (SBUF->SBUF).  R_s columns beyond the
causal edge are padded with -1e9; the anti-diagonal read maps exactly the
strict upper triangle onto that pad, so exp() yields the causal zeros with
no separate masking pass.

Projections run kc-outer with all 8 (et, rc) accumulation groups resident
in PSUM so input-chunk DMAs are consumed as they land.  SBUF pools are
phase-scoped: the input/weight pools are released after the projections,
freeing ~64 KB/partition for deep phase-B pipeline buffers.

The rel pipeline is software-pipelined one query-block ahead, and the
P^T transposes (PE matmuls, or batched DMA-crossbar in pt_mode=xbar),
AV, normalize and output projection are emitted with configurable lags
so their dependencies are complete when each in-order engine queue
reaches them.
"""

import numpy as np
import ml_dtypes
import concourse.bass as bass
import concourse.mybir as mybir
import concourse.tile as tile
from concourse import bacc
from concourse import bass_utils
from concourse.bass_interp import get_hw_module
from concourse.masks import make_identity

BF = mybir.dt.bfloat16
F32 = mybir.dt.float32
EXP = mybir.ActivationFunctionType.Exp
IDENT = mybir.ActivationFunctionType.Identity
MULT = mybir.AluOpType.mult
ADD = mybir.AluOpType.add

L = 2048          # sequence length
RS_W = 2176       # R_s tile width (2048 + 128 pad); anti-diag pitch = RS_W-1

PROFILE = False       # set by test harness to capture a trace
LAST_RESULTS = None   # BassKernelResults of the last run (for profiling)
MARKS = []            # (instruction id watermark, stage label) per build


def _mark(nc, label):
    MARKS.append((nc.next_id(), label))

# tuning knobs (read at build time)
CFG = {
    "diag_eng": "sync",    # engine issuing anti-diagonal staging DMAs
    "load_eng": "sync",    # engine issuing input/weight loads
    "rs_bufs": 6,
    "stg_bufs": 7,
    "pn_bufs": 4,
    "pt_bufs": 2,
    "psS_bufs": 2,
    "psR_bufs": 2,
    "psA_bufs": 2,
    "osb_bufs": 3,
    "rec64_bufs": 2,
    "r_evict": "dve2",    # act | dve | mix | dve2
    "qkv_evict": "act",   # projection eviction engine: act | dve
    "io_bufs": 3,
    "in_split": 4,        # input load DMAs per projection half
    "proj_order": "kpvq",
    "ra_dve": 0,          # rel-add chunks moved to DVE per block (0..4)
    "ra_pool": 0,         # rel-add chunks moved to Pool per block (0..4)
    "pad_eng": "vector",  # pad/ones memset engine: gpsimd | vector
    "pt_mode": "pe",      # pe (transpose matmuls) | xbar (batched DMA)
    "psT_bufs": 2,        # PSUM pool for PE transposes (pt_mode=pe)
    "pt_evict": "dve",    # psT -> pt eviction engine: act | dve | mix
    "tr_lag": 1,          # jobs between block chunks and its PE transposes
    "xbar_lag": 2,        # jobs between block_stage and its crossbar DMA
    "av_mode": "reor",    # reor (query-partition AV) | col (at-orientation)
    "av_split": 1,        # 1: emit AV strips right after each block's tr
    "rel_lead": 3,        # how many jobs ahead the rel stage runs
    "rel_lead0": 2,       # rel lead for the first 8 jobs
    "g_order": "asc",    # group processing order: asc | desc
    "wk_split": 1,        # first weight load chunk count
    "av_lag": 3,          # jobs between last xbar of (h,g) and its AV
    "norm_lag": 1,        # jobs between AV and recip/bcast/normalize
    "op_lag": 1,          # jobs between norm(h=3) and outproj
    "bcast": "gpsimd",    # denominator broadcast: gpsimd | pe
    "store_eng": "sync",  # output store queue: sync | scalar
}


def emit_core(nc, ins, out):
    """ins: dict name->AP (DRAM), out: AP (DRAM [2048,1024] bf16)."""
    with tile.TileContext(nc) as tc:
        deng = getattr(nc, CFG["diag_eng"])
        leng = getattr(nc, CFG["load_eng"])
        peng = getattr(nc, CFG["pad_eng"] if CFG["pad_eng"] != "gpsimd"
                       else "gpsimd")
        with (
            tc.tile_pool(name="per", bufs=1) as per,
            tc.tile_pool(name="psS", bufs=CFG["psS_bufs"], space="PSUM") as psS,
            tc.tile_pool(name="psR", bufs=CFG["psR_bufs"], space="PSUM") as psR,
            tc.tile_pool(name="psA", bufs=CFG["psA_bufs"], space="PSUM") as psA,
            tc.tile_pool(name="psT", bufs=CFG["psT_bufs"], space="PSUM") as psT,
        ):
            # ---------------- persistent tiles ----------------
            q1t = per.tile([128, 2 * L], BF, tag="q1t")
            q2t = per.tile([128, 2 * L], BF, tag="q2t")
            kt = per.tile([128, 2 * L], BF, tag="kt")
            peht = per.tile([128, 2 * L], BF, tag="peht")
            at = per.tile([128, 2 * L], BF, tag="at")
            vh = per.tile([128, 16 * 260], BF, tag="vh")
            wo_s = per.tile([128, 2048], BF, tag="wo")
            b1s = per.tile([128, 2], F32, tag="b1s")
            b2s = per.tile([128, 2], F32, tag="b2s")
            bks = per.tile([128, 2], F32, tag="bks")
            bvs = per.tile([128, 2], F32, tag="bvs")

            ident = per.tile([128, 128], BF, tag="ident")
            make_identity(nc, ident[:])
            ones64 = per.tile([1, 64], BF, tag="ones64")
            nc.vector.memset(ones64[:], 1.0)
            # bias loads go through the Activation HWDGE queue so they don't
            # head-of-line block the SP queue ahead of the weight/input loads
            nc.scalar.dma_start(b1s[:], ins["b1"])
            nc.scalar.dma_start(b2s[:], ins["b2"])
            nc.scalar.dma_start(bks[:], ins["bk"])
            nc.scalar.dma_start(bvs[:], ins["bv"])

            def _pevict(dst, ps, bias, scale):
                if CFG["qkv_evict"] == "dve":
                    nc.vector.tensor_scalar(
                        out=dst, in0=ps[:], scalar1=scale, scalar2=bias,
                        op0=MULT, op1=ADD)
                else:
                    nc.scalar.activation(dst, ps[:], IDENT, bias=bias,
                                         scale=scale)

            def evict_q(et, rc, ps):
                cs = 2048 * et + 512 * rc
                if CFG["qkv_evict"] == "qdve":
                    nc.vector.tensor_scalar(
                        out=q1t[:, cs:cs + 512], in0=ps[:], scalar1=0.125,
                        scalar2=b1s[:, et:et + 1], op0=MULT, op1=ADD)
                    nc.vector.tensor_scalar(
                        out=q2t[:, cs:cs + 512], in0=ps[:], scalar1=0.125,
                        scalar2=b2s[:, et:et + 1], op0=MULT, op1=ADD)
                else:
                    _pevict(q1t[:, cs:cs + 512], ps, b1s[:, et:et + 1], 0.125)
                    _pevict(q2t[:, cs:cs + 512], ps, b2s[:, et:et + 1], 0.125)

            def evict_k(et, rc, ps):
                cs = 2048 * et + 512 * rc
                _pevict(kt[:, cs:cs + 512], ps, bks[:, et:et + 1], 1.0)

            def evict_pe(et, rc, ps):
                cs = 2048 * et + 512 * rc
                nc.vector.tensor_copy(peht[:, cs:cs + 512], ps[:])

            # ---------------- phase A: projections (scoped pools) ---------
            # kc-outer: all 8 (et, rc) accumulation groups live in PSUM
            # simultaneously (borrowing every PSUM pool), so each input
            # chunk DMA is consumed as soon as it lands.
            _proj_pools = [psS, psS, psR, psR, psR, psT, psT, psA]
            _proj_tags = ["S", "S", "R", "R", "R", "T", "T", "A"]

            with (
                tc.tile_pool(name="wgt", bufs=1) as wp,
                tc.tile_pool(name="io", bufs=CFG["io_bufs"]) as iop,
            ):
                vht = iop.tile([128, 2 * L], BF, tag="vht", bufs=1)

                def evict_v(et, rc, ps):
                    cs = 2048 * et + 512 * rc
                    _pevict(vht[:, cs:cs + 512], ps, bvs[:, et:et + 1], 1.0)

                def load_w(name, split=1):
                    t = wp.tile([128, 2048], BF, tag=name, name=f"w_{name}")
                    src = ins[name].rearrange("(a p) e -> p a e", p=128)
                    blk = 8 // split
                    for si in range(split):
                        leng.dma_start(
                            t[:, 256 * blk * si: 256 * blk * (si + 1)],
                            src[:, blk * si: blk * (si + 1), :])
                    return t

                _first_w = ["wq", "wk", "wv", "rk"][
                    "qkvp".index(CFG["proj_order"][0])]

                def project(xname, wname, evict):
                    w_s = load_w(wname, CFG["wk_split"]
                                 if wname == _first_w else 1)
                    src = ins[xname].rearrange("(a p) n -> p a n", p=128)
                    xts = []
                    for half in range(2):
                        xt = iop.tile([128, 4 * L], BF, tag="inT",
                                      name=f"in_{xname}_{half}")
                        ns = CFG["in_split"]
                        blk = 4 // ns
                        for s in range(ns):
                            leng.dma_start(
                                xt[:, 2048 * blk * s: 2048 * blk * (s + 1)],
                                src[:, 4 * half + blk * s:
                                    4 * half + blk * (s + 1), :],
                            )
                        xts.append(xt)
                    pss = {}
                    for i, (et, rc) in enumerate(
                            [(e, r) for e in range(2) for r in range(4)]):
                        pss[(et, rc)] = _proj_pools[i].tile(
                            [128, 512], F32, tag=_proj_tags[i],
                            name=f"ps_{xname}_{et}_{rc}")
                    for kc in range(8):
                        xt = xts[kc // 4]
                        kcc = kc % 4
                        for et in range(2):
                            for rc in range(4):
                                nc.tensor.matmul(
                                    pss[(et, rc)][:],
                                    w_s[:, 256 * kc + 128 * et:
                                        256 * kc + 128 * et + 128],
                                    xt[:, 2048 * kcc + 512 * rc:
                                       2048 * kcc + 512 * rc + 512],
                                    start=(kc == 0),
                                    stop=(kc == 7),
                                )
                    for et in range(2):
                        for rc in range(4):
                            evict(et, rc, pss[(et, rc)])

                projs = {"q": ("q_in", "wq", evict_q),
                         "k": ("k_in", "wk", evict_k),
                         "v": ("v_in", "wv", evict_v),
                         "p": ("pe_in", "rk", evict_pe)}
                for c in CFG["proj_order"]:
                    _mark(nc, f"proj_{c}")
                    project(*projs[c])
                leng.dma_start(
                    wo_s[:], ins["wo"].rearrange("(a p) e -> p a e", p=128))

                _mark(nc, "vh_asm")
                # vh assembly: PE-transpose one [128,128] tile per (ct,
                # pair), evict the two heads' column halves into vh blocks.
                for ct in range(16):
                    for p in range(2):
                        scr = psT.tile([128, 512], BF, tag="T",
                                       name=f"scr_{ct}_{p}")
                        nc.tensor.transpose(
                            scr[:, 0:128],
                            vht[:, 2048 * p + 128 * ct:
                                2048 * p + 128 * ct + 128],
                            ident[:],
                        )
                        for hh in range(2):
                            h4 = 2 * p + hh
                            nc.vector.tensor_copy(
                                vh[:, 260 * ct + 65 * h4:
                                   260 * ct + 65 * h4 + 64],
                                scr[:, 64 * hh: 64 * hh + 64],
                            )
                ones_cols = vh[:].rearrange("p (ct c) -> p ct c", c=260)
                for h in range(4):
                    peng.memset(ones_cols[:, :, 65 * h + 64], 1.0)

            # ---------------- phase B: attention (scoped pools) -----------
            with (
                tc.tile_pool(name="work", bufs=3) as wk,
                tc.tile_pool(name="pt", bufs=CFG["pt_bufs"]) as ptp,
            ):
                def outproj_rt(rt):
                    osb = wk.tile([128, 1024], BF, tag="osb",
                                  bufs=CFG["osb_bufs"], name=f"osb_{rt}")
                    for n in range(2):
                        op_ = psS.tile([128, 512], F32, tag="S",
                                       name=f"op_{rt}_{n}")
                        for hc in range(2):
                            nc.tensor.matmul(
                                op_[:],
                                at[:, 2048 * hc + 128 * rt:
                                   2048 * hc + 128 * rt + 128],
                                wo_s[:, 1024 * hc + 512 * n:
                                     1024 * hc + 512 * n + 512],
                                start=(hc == 0), stop=(hc == 1),
                            )
                        if n == 0:
                            nc.scalar.copy(osb[:, 0:512], op_[:])
                        else:
                            nc.vector.tensor_copy(osb[:, 512:1024], op_[:])
                    getattr(nc, CFG["store_eng"]).dma_start(
                        out[128 * rt: 128 * rt + 128, :], osb[:])

                # rel-score stage for one (h, bi): matmuls -> rs evict ->
                # pad memset -> anti-diagonal staging DMA.
                def rel_stage(h, bi):
                    et, ph = h // 2, h % 2
                    r0, r1 = 64 * ph, 64 * ph + 64
                    ri = 128 * bi
                    Wb = ri + 128
                    nch = (Wb + 511) // 512
                    xmin = 1920 - ri
                    rs = wk.tile([128, RS_W], BF, tag="rs",
                                 bufs=CFG["rs_bufs"], name=f"rs_h{h}_b{bi}")
                    for jc in range(nch):
                        w = min(512, Wb - 512 * jc)
                        rp = psR.tile([128, 512], F32, tag="R",
                                      name=f"rp_h{h}_b{bi}_{jc}")
                        nc.tensor.matmul(
                            rp[:, :w],
                            q2t[r0:r1, 2048 * et + ri: 2048 * et + ri + 128],
                            peht[r0:r1, 2048 * et + xmin + 512 * jc:
                                 2048 * et + xmin + 512 * jc + w],
                            start=True, stop=True,
                        )
                        use_act = (CFG["r_evict"] == "act" or
                                   (CFG["r_evict"] == "mix" and jc % 2 == 0) or
                                   (CFG["r_evict"] == "dve2" and jc % 3 == 0))
                        if use_act:
                            nc.scalar.copy(rs[:, 512 * jc: 512 * jc + w],
                                           rp[:, :w])
                        else:
                            nc.vector.tensor_copy(
                                rs[:, 512 * jc: 512 * jc + w], rp[:, :w])
                    peng.memset(rs[:, Wb:Wb + 128], -1e9)
                    staged = wk.tile([128, 2048], BF, tag="stg",
                                     bufs=CFG["stg_bufs"],
                                     name=f"stg_h{h}_b{bi}")
                    diag = bass.AP(
                        tensor=rs.tensor,
                        offset=rs.offset + 127,
                        ap=[[RS_W - 1, 128], [1, Wb]],
                    )
                    deng.dma_start(staged[:, :Wb], diag)
                    return staged

                # content/ident/exp chunks for one (h, bi)
                def block_stage(h, bi, staged, pn, nrel_dve):
                    et, ph = h // 2, h % 2
                    r0, r1 = 64 * ph, 64 * ph + 64
                    ri = 128 * bi
                    Wb = ri + 128
                    nch = (Wb + 511) // 512
                    for ci_chunk in range(nch):
                        c0 = 512 * ci_chunk
                        wc = min(512, Wb - c0)
                        use_dve = ci_chunk < nrel_dve
                        use_pool = (not use_dve and
                                    ci_chunk < nrel_dve + CFG["ra_pool"])
                        sp = psS.tile([128, 512], F32, tag="S",
                                      name=f"sp_h{h}_b{bi}_{ci_chunk}")
                        nc.tensor.matmul(
                            sp[:, :wc],
                            q1t[r0:r1, 2048 * et + ri: 2048 * et + ri + 128],
                            kt[r0:r1, 2048 * et + c0: 2048 * et + c0 + wc],
                            start=True, stop=use_dve or use_pool,
                        )
                        if use_dve or use_pool:
                            eng = nc.vector if use_dve else nc.gpsimd
                            eng.scalar_tensor_tensor(
                                out=sp[:, :wc], in0=sp[:, :wc], scalar=1.0,
                                in1=staged[:, c0:c0 + wc],
                                op0=MULT, op1=ADD,
                            )
                        else:
                            nc.tensor.matmul(
                                sp[:, :wc],
                                ident[:],
                                staged[:, c0:c0 + wc],
                                start=False, stop=True,
                            )
                        nc.scalar.activation(pn[:, c0:c0 + wc],
                                             sp[:, :wc], EXP)

                # PE transposes for one (h, bi): emitted with a lag so the
                # exps they wait on are complete.
                def transp_stage(h, bi, pn, pt):
                    Wb = 128 * bi + 128
                    nch = (Wb + 511) // 512
                    for ci_chunk in range(nch):
                        c0 = 512 * ci_chunk
                        wc = min(512, Wb - c0)
                        tp_ = psT.tile([128, 512], BF, tag="T",
                                       name=f"tp_h{h}_b{bi}_{ci_chunk}")
                        for s in range(wc // 128):
                            nc.tensor.transpose(
                                tp_[:, 128 * s: 128 * s + 128],
                                pn[:, c0 + 128 * s: c0 + 128 * s + 128],
                                ident[:],
                            )
                        dst = bass.AP(
                            tensor=pt.tensor,
                            offset=pt.offset + 512 * (c0 // 128)
                            + 128 * (bi % 4),
                            ap=[[pt.tensor.shape[-1], 128],
                                [512, wc // 128], [1, 128]],
                        )
                        use_act = (CFG["pt_evict"] == "act" or
                                   (CFG["pt_evict"] == "mix"
                                    and (bi + ci_chunk) % 2 == 0))
                        if use_act:
                            nc.scalar.copy(dst, tp_[:, :wc])
                        else:
                            nc.vector.tensor_copy(dst, tp_[:, :wc])

                def xbar_stage(bi, pn, pt):
                    # batched crossbar transpose: pn [128, Wb] -> pt strips
                    Wb = 128 * bi + 128
                    dst = bass.AP(
                        tensor=pt.tensor,
                        offset=pt.offset + 128 * (bi % 4),
                        ap=[[pt.tensor.shape[-1], 128], [512, Wb // 128],
                            [1, 128]],
                    )
                    nc.sync.dma_start_transpose(dst, pn[:, :Wb])

                def av_stage(h, g, pt, av, ci_lo=0, ci_hi=None):
                    if ci_hi is None:
                        ci_hi = 4 * g + 4
                    for ci in range(ci_lo, ci_hi):
                        o = max(0, 128 * ci - 512 * g)
                        nc.tensor.matmul(
                            av[:, o:512],
                            vh[:, 260 * ci + 65 * h: 260 * ci + 65 * h + 65],
                            pt[:, 512 * ci + o: 512 * ci + 512],
                            start=(ci == 0), stop=(ci == 4 * g + 3),
                        )

                def av_col_stage(h, g, pt, av, qc):
                    # accumulate strips 0..4g+qc into psA columns
                    # [128qc, 128qc+128) -- ready right after block 4g+qc's
                    # transposes (the last writer of those pt columns)
                    nci = 4 * g + qc + 1
                    for ci in range(nci):
                        nc.tensor.matmul(
                            av[:, 128 * qc: 128 * qc + 128],
                            vh[:, 260 * ci + 65 * h: 260 * ci + 65 * h + 65],
                            pt[:, 512 * ci + 128 * qc: 512 * ci + 128 * qc + 128],
                            start=(ci == 0), stop=(ci == nci - 1),
                        )

                def av_q_stage(h, qt, pt, av_q):
                    # out [128 q, 65]: stationary pt q-col block, moving vh
                    # (65th col = ones -> denominators); strips 0..qt are all
                    # complete once block bi=qt has transposed.
                    qc = qt % 4
                    for ci in range(qt + 1):
                        nc.tensor.matmul(
                            av_q[:, 0:65],
                            pt[:, 512 * ci + 128 * qc: 512 * ci + 128 * qc + 128],
                            vh[:, 260 * ci + 65 * h: 260 * ci + 65 * h + 65],
                            start=(ci == 0), stop=(ci == qt),
                        )

                def norm_q_stage(h, qt, av_q, avn2):
                    rec = wk.tile([128, 1], F32, tag="recq", bufs=3,
                                  name=f"recq_h{h}_q{qt}")
                    nc.vector.reciprocal(rec[:], av_q[:, 64:65])
                    ph = h % 2
                    nc.vector.tensor_scalar_mul(
                        avn2[:, 64 * ph: 64 * ph + 64], av_q[:, 0:64],
                        rec[:])

                def tp_q_stage(et, qt, avn2):
                    tp_ = psT.tile([128, 512], BF, tag="T",
                                   name=f"tpq_e{et}_q{qt}")
                    nc.tensor.transpose(tp_[:, 0:128], avn2[:], ident[:])
                    nc.vector.tensor_copy(
                        at[:, 2048 * et + 128 * qt: 2048 * et + 128 * qt + 128],
                        tp_[:, 0:128])

                def norm_stage(h, g, av):
                    et, ph = h // 2, h % 2
                    r0, r1 = 64 * ph, 64 * ph + 64
                    rec = wk.tile([1, 512], F32, tag="rec",
                                  name=f"rec_h{h}_g{g}")
                    nc.vector.reciprocal(rec[:], av[64:65, :])
                    if CFG["bcast"] == "pe":
                        rec_bf = wk.tile([1, 512], BF, tag="recb",
                                         name=f"recb_h{h}_g{g}")
                        nc.vector.tensor_copy(rec_bf[:], rec[:])
                        rec64 = psT.tile([64, 512], F32, tag="T",
                                         name=f"rec64_h{h}_g{g}")
                        nc.tensor.matmul(rec64[:], ones64[:], rec_bf[:],
                                         start=True, stop=True)
                        nc.vector.tensor_tensor(
                            out=at[r0:r1, 2048 * et + 512 * g:
                                   2048 * et + 512 * g + 512],
                            in0=av[0:64, :],
                            in1=rec64[:],
                            op=MULT,
                        )
                    else:
                        rec64 = wk.tile([64, 512], F32, tag="rec64",
                                        bufs=CFG["rec64_bufs"],
                                        name=f"rec64_h{h}_g{g}")
                        nc.gpsimd.partition_broadcast(rec64[:], rec[:])
                        nc.vector.tensor_tensor(
                            out=at[r0:r1, 2048 * et + 512 * g:
                                   2048 * et + 512 * g + 512],
                            in0=av[0:64, :],
                            in1=rec64[:],
                            op=MULT,
                        )

                # software-pipelined schedule over jobs (h, g, bi)
                gs = range(4) if CFG["g_order"] == "asc" else range(3, -1, -1)
                seq = [(h, g, bi) for g in gs for h in range(4)
                       for bi in range(4 * g, 4 * g + 4)]
                deferred = {}   # emit_idx -> list of callables

                def defer(idx, fn):
                    deferred.setdefault(idx, []).append(fn)

                pts = {}
                avs = {}
                avn2s = {}
                staged_tiles = {}
                n = len(seq)
                _rel_next = [0]

                def emit_rel_through(r):
                    while _rel_next[0] <= min(r, n - 1):
                        j = _rel_next[0]
                        _mark(nc, f"rel_{j}")
                        staged_tiles[seq[j]] = rel_stage(seq[j][0], seq[j][2])
                        _rel_next[0] += 1

                emit_rel_through(max(CFG["rel_lead"], CFG["rel_lead0"]) - 1)
                for idx, (h, g, bi) in enumerate(seq):
                    if (h, g) not in pts:
                        pts[(h, g)] = ptp.tile(
                            [128, 512 * (4 * g + 4)], BF, tag="pt",
                            bufs=CFG["pt_bufs"], name=f"pt_h{h}_g{g}")
                    emit_rel_through(idx + CFG["rel_lead"])
                    pn = wk.tile([128, 2048], BF, tag="pn",
                                 bufs=CFG["pn_bufs"], name=f"pn_h{h}_b{bi}")
                    pt = pts[(h, g)]
                    _mark(nc, f"blk_{idx}")
                    block_stage(h, bi, staged_tiles.pop((h, g, bi)), pn,
                                CFG["ra_dve"])
                    if CFG["pt_mode"] == "xbar":
                        xb_idx = idx + CFG["xbar_lag"]
                        defer(xb_idx, lambda bi=bi, pn=pn, pt=pt, _i=idx:
                              (_mark(nc, f"xbar_{_i}"),
                               xbar_stage(bi, pn, pt)))
                    else:
                        xb_idx = idx + CFG["tr_lag"]
                        if CFG["tr_lag"]:
                            defer(xb_idx, lambda h=h, bi=bi, pn=pn, pt=pt,
                                  _i=idx:
                                  (_mark(nc, f"tr_{_i}"),
                                   transp_stage(h, bi, pn, pt)))
                        else:
                            transp_stage(h, bi, pn, pt)
                    if CFG["av_mode"] == "reor":
                        av_idx = xb_idx + CFG["av_lag"]

                        def do_av_q(h=h, g=g, pt=pt, qt=bi):
                            _mark(nc, f"av_{h}_{g}")
                            av_q = psA.tile([128, 65], F32, tag="A",
                                            name=f"avq_h{h}_q{qt}")
                            avs[(h, qt)] = av_q
                            av_q_stage(h, qt, pt, av_q)
                            if qt % 4 == 3:
                                pts.pop((h, g), None)
                        defer(av_idx, do_av_q)
                        nmq_idx = av_idx + CFG["norm_lag"]

                        def do_norm_q(h=h, qt=bi, et=h // 2):
                            _mark(nc, f"nm_{h}_{qt}")
                            if (et, qt) not in avn2s:
                                avn2s[(et, qt)] = wk.tile(
                                    [128, 128], BF, tag="avn2", bufs=8,
                                    name=f"avn2_e{et}_q{qt}")
                            norm_q_stage(h, qt, avs.pop((h, qt)),
                                         avn2s[(et, qt)])
                        defer(nmq_idx, do_norm_q)
                        if h % 2 == 1:
                            defer(nmq_idx + 1, lambda et=h // 2, qt=bi:
                                  tp_q_stage(et, qt, avn2s.pop((et, qt))))
                        if h == 3:
                            def do_op_rt(rt=bi):
                                _mark(nc, f"op_{g}")
                                outproj_rt(rt)
                            defer(nmq_idx + 1 + CFG["op_lag"], do_op_rt)
                    elif CFG["av_split"]:
                        def do_av_part(h=h, g=g, pt=pt, bi=bi):
                            _mark(nc, f"av_{h}_{g}")
                            if (h, g) not in avs:
                                avs[(h, g)] = psA.tile(
                                    [65, 512], F32, tag="A",
                                    name=f"av_h{h}_g{g}")
                            av_col_stage(h, g, pt, avs[(h, g)], bi % 4)
                            if bi % 4 == 3:
                                pts.pop((h, g), None)
                        defer(xb_idx + CFG["av_lag"], do_av_part)
                    if bi == 4 * g + 3 and CFG["av_mode"] != "reor":
                        av_idx = xb_idx + CFG["av_lag"]
                        if not CFG["av_split"]:
                            def do_av(h=h, g=g, pt=pt):
                                _mark(nc, f"av_{h}_{g}")
                                avs[(h, g)] = psA.tile(
                                    [65, 512], F32, tag="A",
                                    name=f"av_h{h}_g{g}")
                                av_stage(h, g, pt, avs[(h, g)])
                                pts.pop((h, g), None)
                            defer(av_idx, do_av)
                        nm_idx = av_idx + CFG["norm_lag"]
                        defer(nm_idx, lambda h=h, g=g:
                              (_mark(nc, f"nm_{h}_{g}"),
                               norm_stage(h, g, avs.pop((h, g)))))
                        if h == 3:
                            def do_op(g=g):
                                _mark(nc, f"op_{g}")
                                for rt in range(4 * g, 4 * g + 4):
                                    outproj_rt(rt)
                            defer(nm_idx + CFG["op_lag"], do_op)
                    for fn in deferred.pop(idx, []):
                        fn()
                # flush remaining deferred work in index order
                for idx in sorted(deferred):
                    for fn in deferred[idx]:
                        fn()
    return nc


# ---------------- host side ----------------

def _bf16(x):
    return np.ascontiguousarray(x).astype(ml_dtypes.bfloat16)


def _col2d(vec256):
    """[256] f32 -> [128, 2] with v2d[p, a] = vec[128a + p]."""
    return np.ascontiguousarray(
        np.asarray(vec256, np.float32).reshape(2, 128).T)


def core_inputs(q_b, k_b, v_b, pos_enc, Wq, bq, Wk, bk, Wv, bv, Wo,
                r_w_bias, r_r_bias, r_kernel, g):
    sl = slice(256 * g, 256 * g + 256)
    rk_cat = np.concatenate([r_kernel[4 * g + i] for i in range(4)], axis=1)
    return {
        "q_in": _bf16(q_b.T),
        "k_in": _bf16(k_b.T),
        "v_in": _bf16(v_b.T),
        "pe_in": _bf16(pos_enc[1:2049].T),
        "wq": _bf16(Wq[:, sl]),
        "wk": _bf16(Wk[:, sl]),
        "wv": _bf16(Wv[:, sl]),
        "rk": _bf16(rk_cat),
        "wo": _bf16(Wo[sl, :]),
        "b1": _col2d(0.125 * (bq[sl] + r_w_bias[4 * g:4 * g + 4].reshape(256))),
        "b2": _col2d(0.125 * (bq[sl] + r_r_bias[4 * g:4 * g + 4].reshape(256))),
        "bk": _col2d(bk[sl]),
        "bv": _col2d(bv[sl]),
    }


_SHAPES = {
    "q_in": ([1024, 2048], BF), "k_in": ([1024, 2048], BF),
    "v_in": ([1024, 2048], BF), "pe_in": ([1024, 2048], BF),
    "wq": ([1024, 256], BF), "wk": ([1024, 256], BF), "wv": ([1024, 256], BF),
    "rk": ([1024, 256], BF), "wo": ([256, 1024], BF),
    "b1": ([128, 2], F32), "b2": ([128, 2], F32),
    "bk": ([128, 2], F32), "bv": ([128, 2], F32),
}

_NC_CACHE = {}


def _build():
    key = tuple(sorted((k, str(v)) for k, v in CFG.items()))
    if key in _NC_CACHE:
        return _NC_CACHE[key]
    MARKS.clear()
    nc = bacc.Bacc("TRN2", target_bir_lowering=False, debug=False,
                   enable_asserts=False)
    ins = {name: nc.dram_tensor(name, shape, dt, kind="ExternalInput").ap()
           for name, (shape, dt) in _SHAPES.items()}
    out = nc.dram_tensor("out", [2048, 1024], BF, kind="ExternalOutput").ap()
    emit_core(nc, ins, out)
    nc.compile()
    nc.m = get_hw_module(nc.m)
    _NC_CACHE[key] = nc
    return nc


def kernel(**inputs):
    global LAST_RESULTS
    inp = {k: np.asarray(v) for k, v in inputs.items()}
    nc = _build()
    in_maps = []
    for c in range(8):
        b, g = c // 4, c % 4
        in_maps.append(core_inputs(
            inp["q"][b], inp["k"][b], inp["v"][b], inp["pos_enc"],
            inp["Wq"], inp["bq"], inp["Wk"], inp["bk"], inp["Wv"], inp["bv"],
            inp["Wo"], inp["r_w_bias"], inp["r_r_bias"], inp["r_kernel"], g))
    res = bass_utils.run_bass_kernel_spmd(
        nc, in_maps, core_ids=list(range(8)), trace=PROFILE)
    LAST_RESULTS = res
    out = np.zeros((2, 2048, 1024), np.float32)
    for c in range(8):
        b = c // 4
        out[b] += np.asarray(res.results[c]["out"]).astype(np.float32)
    out += np.asarray(inp["bo"], np.float32)[None, None, :]
    return out
